# revision 16
# baseline (speedup 1.0000x reference)
"""AcidSynth Trainium2 kernel: 32 voices sharded 4-per-core across 8 NeuronCores.

Pipeline per core (4 voices, T=131072):
  - VCO sin/cos from host-reduced phase (ACT Sin), tanh square-saw morph
  - ADSR envelope (ACT Ln/Exp power curves)
  - RBJ time-varying lowpass biquad coefficients (ACT Exp/Sin + DVE)
  - IIR scan: block-parallel 3-channel method (particular + 2 homogeneous
    runs per block; linear correction), in-block steps vectorized over
    128 partitions x (4 voices x 8 chunks); carry chain composed
    intra-partition, then across partitions via PE shift-matmul
    Hillis-Steele scan
  - tanh distortion
"""
import numpy as np

f32 = np.float32
SR = 48000.0
T = 131072
P = 128
NV = 4            # voices per core
C = 8             # chunks per partition per voice
L = 1024 // C     # in-block scan length (128)
NCORES = 8

MIN_W = 2.0 * np.pi * 20.0 / SR
MAX_W = 2.0 * np.pi * 8000.0 / SR
MIN_Q = 0.7071
MAX_Q = 8.0

# voice-const column indices (vc tensor [128, NV*NK], col = v*NK + k)
K_TANH, K_S, K_DRYSC, K_IATT, K_ALPHA, K_IDEC, K_BDEC, K_N1MS, K_IREL, \
    K_BREL, K_DIST = range(11)
NK = 11

_CACHE = {}


def _build_module():
    import concourse.bacc as bacc
    import concourse.mybir as mybir
    from concourse import tile

    dt = mybir.dt.float32
    A = mybir.AluOpType
    F = mybir.ActivationFunctionType

    nc = bacc.Bacc()
    rph_d = nc.dram_tensor("rph", [NV, T], dt, kind="ExternalInput")
    fc_d = nc.dram_tensor("fcm", [NV, T], dt, kind="ExternalInput")
    qm_d = nc.dram_tensor("qmm", [NV, T], dt, kind="ExternalInput")
    ts_d = nc.dram_tensor("tsec", [P, 1024], dt, kind="ExternalInput")
    vc_d = nc.dram_tensor("vc", [P, NV * NK], dt, kind="ExternalInput")
    sh_d = nc.dram_tensor("shm", [P, 8 * P], dt, kind="ExternalInput")
    import os as _os
    DBG = bool(int(_os.environ.get("KDEBUG", "0")))
    dry_d = nc.dram_tensor("dry", [NV, T], dt, kind="ExternalOutput")
    wet_d = nc.dram_tensor("wet", [NV, T], dt, kind="ExternalOutput")
    env_d = nc.dram_tensor("env", [NV, T], dt, kind="ExternalOutput")
    if DBG:
        dbgw_d = nc.dram_tensor("dbgw", [NV, T], dt, kind="ExternalOutput")
        dbga_d = nc.dram_tensor("dbga", [NV, T], dt, kind="ExternalOutput")
        dbgy_d = nc.dram_tensor("dbgy", [P, NV * C * 3 * (L + 2)], dt,
                                kind="ExternalOutput")
        dbgab_d = nc.dram_tensor("dbgab", [P, 3 * NV * C], dt,
                                 kind="ExternalOutput")

    def dview(d):  # DRAM [NV, T] as [128, NV, 1024]
        return d[:, :].rearrange("v (p j) -> p v j", p=P)

    K1 = float(np.log(MAX_W / MIN_W))
    K0 = float(np.log(MIN_W))
    Kq1 = float(np.log(MAX_Q / MIN_Q))
    Kq0 = float(np.log(MIN_Q))
    PI = float(np.pi)
    BQ = -(Kq0 + float(np.log(2.0)))

    # ACT float biases need pre-registered const APs
    for val in (PI / 2, K0, BQ):
        t = nc.alloc_sbuf_tensor(f"constap-{val}", [128, 1], dt)
        nc.gpsimd.memset(t.ap(), val)
        nc.const_aps.aps[(dt, float(val))] = t.ap()
    nc.all_engine_barrier()

    with tile.TileContext(nc) as tc:
        with (
            tc.tile_pool(name="big", bufs=8) as bigp,
            tc.tile_pool(name="ybuf", bufs=1) as ypool,
            tc.tile_pool(name="keep", bufs=1) as keepp,
            tc.tile_pool(name="tmps", bufs=10) as tmpp,
            tc.tile_pool(name="psum", bufs=2, space="PSUM") as psp,
        ):
            def big():
                return bigp.tile([P, NV * 1024], dt, tag="big", name="bigt")

            def keep(name, shape):
                return keepp.tile([P, *shape] if shape else [P, 1], dt,
                                  tag=name, name=name)

            def vsl(t, v):
                return t[:, v * 1024:(v + 1) * 1024]

            def vc1(v, k):
                return vc_t[:, v * NK + k: v * NK + k + 1]

            # ---------- small persistent loads ----------
            ts_t = keepp.tile([P, 1024], dt, tag="tsec")
            nc.sync.dma_start(ts_t[:], ts_d[:, :])
            vc_t = keepp.tile([P, NV * NK], dt, tag="vc")
            nc.sync.dma_start(vc_t[:], vc_d[:, :])
            sh_t = keepp.tile([P, 8 * P], dt, tag="shm")
            nc.sync.dma_start(sh_t[:], sh_d[:, :])

            # ---------- phase -> sin, cos ----------
            rph = big()
            nc.sync.dma_start(rph[:].rearrange("p (v j) -> p v j", v=NV),
                              dview(rph_d))
            sn = big()
            nc.scalar.activation(sn[:], rph[:], F.Sin)
            ab = big()
            nc.scalar.activation(ab[:], rph[:], F.Abs)
            cs = big()
            nc.scalar.activation(cs[:], ab[:], F.Sin, bias=PI / 2, scale=-1.0)

            # ---------- ADSR envelope ----------
            env = big()
            tA = ab  # reuse slot logically: Tile tracks deps; use fresh tiles
            tA = big()
            for v in range(NV):
                # att -> env_v
                nc.vector.tensor_scalar(vsl(env, v)[:, 0:1024], ts_t[:],
                                        vc1(v, K_IATT), 1.0, op0=A.mult,
                                        op1=A.min)
                nc.scalar.activation(vsl(env, v), vsl(env, v), F.Ln)
                nc.scalar.activation(vsl(env, v), vsl(env, v), F.Exp,
                                     scale=vc1(v, K_ALPHA))
                # dec -> tA_v
                nc.scalar.activation(vsl(tA, v), ts_t[:], F.Relu,
                                     bias=vc1(v, K_BDEC),
                                     scale=vc1(v, K_IDEC))
                nc.vector.tensor_scalar(vsl(tA, v), vsl(tA, v), 1.0, None,
                                        op0=A.min)
                nc.scalar.activation(vsl(tA, v), vsl(tA, v), F.Ln)
                nc.scalar.activation(vsl(tA, v), vsl(tA, v), F.Exp,
                                     scale=vc1(v, K_ALPHA))
                nc.scalar.activation(vsl(tA, v), vsl(tA, v), F.Identity,
                                     bias=1.0, scale=vc1(v, K_N1MS))
            nc.vector.tensor_tensor(env[:], env[:], tA[:], op=A.mult)
            for v in range(NV):
                # rel -> tA_v
                nc.scalar.activation(vsl(tA, v), ts_t[:], F.Relu,
                                     bias=vc1(v, K_BREL),
                                     scale=vc1(v, K_IREL))
                nc.vector.tensor_scalar(vsl(tA, v), vsl(tA, v), 1.0, None,
                                        op0=A.min)
                nc.scalar.activation(vsl(tA, v), vsl(tA, v), F.Ln)
                nc.scalar.activation(vsl(tA, v), vsl(tA, v), F.Exp,
                                     scale=vc1(v, K_ALPHA))
                nc.scalar.activation(vsl(tA, v), vsl(tA, v), F.Identity,
                                     bias=1.0, scale=-1.0)
            nc.vector.tensor_tensor(env[:], env[:], tA[:], op=A.mult)
            nc.sync.dma_start(dview(env_d),
                              env[:].rearrange("p (v j) -> p v j", v=NV))

            # ---------- VCO mix -> dry ----------
            sq = big()
            mx1 = big()
            for v in range(NV):
                nc.scalar.activation(vsl(sq, v), vsl(sn, v), F.Tanh,
                                     scale=vc1(v, K_TANH))
                nc.scalar.activation(vsl(mx1, v), vsl(cs, v), F.Identity,
                                     bias=1.0, scale=vc1(v, K_S))
            m2 = big()
            nc.gpsimd.tensor_tensor(m2[:], sq[:], mx1[:], op=A.mult)
            dry = big()
            for v in range(NV):
                nc.vector.scalar_tensor_tensor(
                    vsl(dry, v), vsl(m2, v), vc1(v, K_DRYSC), vsl(env, v),
                    op0=A.mult, op1=A.mult)
            nc.sync.dma_start(dview(dry_d),
                              dry[:].rearrange("p (v j) -> p v j", v=NV))

            # ---------- biquad coefficients ----------
            fc = big()
            nc.sync.dma_start(fc[:].rearrange("p (v j) -> p v j", v=NV),
                              dview(fc_d))
            qm = big()
            nc.sync.dma_start(qm[:].rearrange("p (v j) -> p v j", v=NV),
                              dview(qm_d))
            w0 = big()
            nc.scalar.activation(w0[:], fc[:], F.Exp, bias=K0, scale=K1)
            ee = big()
            nc.scalar.activation(ee[:], qm[:], F.Exp, bias=BQ, scale=-Kq1)
            sn0 = big()
            nc.scalar.activation(sn0[:], w0[:], F.Sin)
            cw = big()
            nc.scalar.activation(cw[:], w0[:], F.Sin, bias=PI / 2, scale=1.0)
            al = big()
            nc.gpsimd.tensor_tensor(al[:], sn0[:], ee[:], op=A.mult)
            a0 = big()
            nc.scalar.activation(a0[:], al[:], F.Identity, bias=1.0,
                                 scale=1.0)
            ra = big()
            rs = big()
            nc.vector.reciprocal_approx_accurate(ra[:], a0[:], rs[:])
            nb1 = big()
            nc.vector.scalar_tensor_tensor(nb1[:], cw[:], 1.0, ra[:],
                                           op0=A.subtract, op1=A.mult)
            na1 = big()
            nc.vector.scalar_tensor_tensor(na1[:], cw[:], 2.0, ra[:],
                                           op0=A.mult, op1=A.mult)
            na2 = big()
            nc.vector.scalar_tensor_tensor(na2[:], al[:], 1.0, ra[:],
                                           op0=A.subtract, op1=A.mult)

            # ---------- FIR: w = b1*(0.5*(x + x2) + x1) = nb1*(-0.5*(x+x2)-x1)
            x1 = big()
            x2 = big()
            for v in range(NV):
                nc.gpsimd.tensor_copy(vsl(x1, v)[:, 1:1024],
                                      vsl(dry, v)[:, 0:1023])
                nc.vector.memset(x1[0:1, v * 1024:v * 1024 + 1], 0.0)
                nc.sync.dma_start(x1[1:P, v * 1024:v * 1024 + 1],
                                  dry[0:P - 1, (v + 1) * 1024 - 1:
                                      (v + 1) * 1024])
            for v in range(NV):
                nc.gpsimd.tensor_copy(vsl(x2, v)[:, 1:1024],
                                      vsl(x1, v)[:, 0:1023])
                nc.vector.memset(x2[0:1, v * 1024:v * 1024 + 1], 0.0)
                nc.sync.dma_start(x2[1:P, v * 1024:v * 1024 + 1],
                                  x1[0:P - 1, (v + 1) * 1024 - 1:
                                     (v + 1) * 1024])
            ssum = big()
            nc.gpsimd.tensor_tensor(ssum[:], dry[:], x2[:], op=A.add)
            uu = big()
            nc.vector.scalar_tensor_tensor(uu[:], ssum[:], -0.5, x1[:],
                                           op0=A.mult, op1=A.subtract)
            wf = big()
            nc.vector.tensor_tensor(wf[:], uu[:], nb1[:], op=A.mult)

            if DBG:
                nc.sync.dma_start(dview(dbgw_d),
                                  wf[:].rearrange("p (v j) -> p v j", v=NV))
                nc.sync.dma_start(dview(dbga_d),
                                  na1[:].rearrange("p (v j) -> p v j", v=NV))
            # ---------- blocked IIR scan, level 1 ----------
            Y = ypool.tile([P, NV * C * 3 * (L + 2)], dt, tag="Y")
            Yv = Y[:].rearrange("p (v c ch i) -> p v c ch i", v=NV, c=C,
                                ch=3)
            nc.vector.memset(Yv[:, :, :, 0, 0:2], 0.0)
            nc.vector.memset(Yv[:, :, :, 1, 0:1], 0.0)
            nc.vector.memset(Yv[:, :, :, 1, 1:2], 1.0)
            nc.vector.memset(Yv[:, :, :, 2, 0:1], 1.0)
            nc.vector.memset(Yv[:, :, :, 2, 1:2], 0.0)

            na1v = na1[:].rearrange("p (v c i) -> p v c i", v=NV, c=C)
            na2v = na2[:].rearrange("p (v c i) -> p v c i", v=NV, c=C)
            wfv = wf[:].rearrange("p (v c i) -> p v c i", v=NV, c=C)

            def bc3(view, i):
                return view[:, :, :, i].unsqueeze(3).to_broadcast(
                    (P, NV, C, 3))

            import os as _os2
            _SKIP = bool(int(_os2.environ.get("KSKIP_SCAN", "0")))
            for i in range(0 if _SKIP else L):
                t1 = tmpp.tile([P, NV * C * 3], dt, tag="sc", name="t1")
                t1v = t1[:].rearrange("p (v c ch) -> p v c ch", v=NV, c=C)
                t2 = tmpp.tile([P, NV * C * 3], dt, tag="sc", name="t2")
                t2v = t2[:].rearrange("p (v c ch) -> p v c ch", v=NV, c=C)
                nc.vector.tensor_tensor(t1v, Yv[:, :, :, :, i + 1],
                                        bc3(na1v, i), op=A.mult)
                nc.gpsimd.tensor_tensor(t2v, Yv[:, :, :, :, i],
                                        bc3(na2v, i), op=A.mult)
                nc.vector.tensor_tensor(t1v, t1v, t2v, op=A.add)
                nc.vector.tensor_tensor(Yv[:, :, :, :, i + 2], t1v,
                                        bc3(wfv, i), op=A.add)

            # ---------- block transfers ----------
            # T entries [128, NV, C]: h1e h2e h1e2 h2e2 ue ue2
            TmE = keep("TmE", [NV * C * 6])
            TmEv = TmE[:].rearrange("p (e v c) -> p e v c", e=6, v=NV)
            for (e, ch, col) in ((0, 1, L + 1), (1, 2, L + 1), (2, 1, L),
                                 (3, 2, L)):
                nc.vector.tensor_tensor(TmEv[:, e], Yv[:, :, :, ch, col],
                                        Yv[:, :, :, 0, col], op=A.subtract)
            nc.vector.tensor_copy(TmEv[:, 4], Yv[:, :, :, 0, L + 1])
            nc.vector.tensor_copy(TmEv[:, 5], Yv[:, :, :, 0, L])

            # ---------- intra-partition exclusive prefixes over c ----------
            # Pm00 Pm01 Pm10 Pm11 Pu0 Pu1: [128, 6, NV, C]
            PmE = keep("PmE", [6 * NV * C])
            PmEv = PmE[:].rearrange("p (e v c) -> p e v c", e=6, v=NV)
            for e in range(6):
                init = 1.0 if e in (0, 3) else 0.0
                nc.vector.memset(PmEv[:, e, :, 0:1], init)

            def compose_step(dst, dcc, S, scc, Tt, tcc):
                """dst[:,:,dcc] = T[:,:,tcc] o S[:,:,scc] (T applied second)"""
                tq = tmpp.tile([P, NV], dt, tag="c4", name="tq")
                tq2 = tmpp.tile([P, NV], dt, tag="c4", name="tq2")
                # m: new00 = T00*S00 + T01*S10 ; new01 = T00*S01 + T01*S11
                #    new10 = T10*S00 + T11*S10 ; new11 = T10*S01 + T11*S11
                for (de, a1, b1_, a2, b2_) in ((0, 0, 0, 1, 2), (1, 0, 1, 1, 3),
                                               (2, 2, 0, 3, 2), (3, 2, 1, 3, 3)):
                    nc.vector.tensor_tensor(tq[:], Tt[:, a1, :, tcc],
                                            S[:, b1_, :, scc], op=A.mult)
                    nc.vector.tensor_tensor(tq2[:], Tt[:, a2, :, tcc],
                                            S[:, b2_, :, scc], op=A.mult)
                    nc.vector.tensor_tensor(dst[:, de, :, dcc], tq[:], tq2[:],
                                            op=A.add)
                # u: newu0 = T00*Su0 + T01*Su1 + Tu0 ; newu1 = T10.. + Tu1
                tq3 = tmpp.tile([P, NV], dt, tag="c4", name="tq3")
                tq4 = tmpp.tile([P, NV], dt, tag="c4", name="tq4")
                for (de, a1, a2, au) in ((4, 0, 1, 4), (5, 2, 3, 5)):
                    nc.gpsimd.tensor_tensor(tq3[:], Tt[:, a1, :, tcc],
                                            S[:, 4, :, scc], op=A.mult)
                    nc.gpsimd.tensor_tensor(tq4[:], Tt[:, a2, :, tcc],
                                            S[:, 5, :, scc], op=A.mult)
                    nc.gpsimd.tensor_tensor(tq3[:], tq3[:], tq4[:], op=A.add)
                    nc.gpsimd.tensor_tensor(dst[:, de, :, dcc], tq3[:],
                                            Tt[:, au, :, tcc], op=A.add)

            for cc in range(1, C):
                compose_step(PmEv, cc, PmEv, cc - 1, TmEv, cc - 1)

            # full-partition transfer F = T_{C-1} o P_{C-1}, packed [128,NV,6]
            Fp = keep("Fp", [NV * 6])
            FpE = Fp[:].rearrange("p (e v c) -> p e v c", e=6, v=NV, c=1)
            compose_step(FpE, 0, PmEv, C - 1, TmEv, C - 1)

            # ---------- Hillis-Steele across partitions via PE shifts -------
            # pack to [128, NV*6] with (v, e) layout for matmul rhs
            Xp = keep("Xp", [NV * 6])
            Xv = Xp[:].rearrange("p (v e) -> p v e", v=NV)
            for e in range(6):
                nc.vector.tensor_copy(Xv[:, :, e], FpE[:, e, :, 0])

            cur = Xp
            for si, s in enumerate((1, 2, 4, 8, 16, 32, 64)):
                sh = tmpp.tile([P, NV * 6], dt, tag="hs", name="sh")
                nc.vector.memset(sh[0:s, :], 0.0)
                nc.sync.dma_start(sh[s:P, :], cur[0:P - s, :])
                shv = sh[:].rearrange("p (v e) -> p v e", v=NV)
                nc.vector.memset(shv[0:s, :, 0:1], 1.0)
                nc.vector.memset(shv[0:s, :, 3:4], 1.0)
                nxt = tmpp.tile([P, NV * 6], dt, tag="hs", name="nxt")
                nxtv = nxt[:].rearrange("p (v e) -> p v e", v=NV)
                curv = cur[:].rearrange("p (v e) -> p v e", v=NV)
                tq = tmpp.tile([P, NV], dt, tag="c4", name="tq")
                tq2 = tmpp.tile([P, NV], dt, tag="c4", name="tq2")
                for (de, a1, b1_, a2, b2_) in ((0, 0, 0, 1, 2), (1, 0, 1, 1, 3),
                                               (2, 2, 0, 3, 2), (3, 2, 1, 3, 3)):
                    nc.vector.tensor_tensor(tq[:], curv[:, :, a1],
                                            shv[:, :, b1_], op=A.mult)
                    nc.vector.tensor_tensor(tq2[:], curv[:, :, a2],
                                            shv[:, :, b2_], op=A.mult)
                    nc.vector.tensor_tensor(nxtv[:, :, de], tq[:], tq2[:],
                                            op=A.add)
                tq3 = tmpp.tile([P, NV], dt, tag="c4", name="tq3")
                tq4 = tmpp.tile([P, NV], dt, tag="c4", name="tq4")
                for (de, a1, a2) in ((4, 0, 1), (5, 2, 3)):
                    nc.gpsimd.tensor_tensor(tq3[:], curv[:, :, a1],
                                            shv[:, :, 4], op=A.mult)
                    nc.gpsimd.tensor_tensor(tq4[:], curv[:, :, a2],
                                            shv[:, :, 5], op=A.mult)
                    nc.gpsimd.tensor_tensor(tq3[:], tq3[:], tq4[:], op=A.add)
                    nc.gpsimd.tensor_tensor(nxtv[:, :, de], tq3[:],
                                            curv[:, :, de], op=A.add)
                cur = nxt

            # exclusive shift by 1: s0 = shift1(cur); rows<1 -> zeros
            s0 = keep("s0", [NV * 6])
            nc.vector.memset(s0[0:1, :], 0.0)
            nc.sync.dma_start(s0[1:P, :], cur[0:P - 1, :])
            s0v = s0[:].rearrange("p (v e) -> p v e", v=NV)

            # ---------- per-block incoming states: ab = Pm@s0u + Pu --------
            abt = keep("abt", [3 * NV * C])
            abv = abt[:].rearrange("p (e v c) -> p e v c", e=3, v=NV)
            s0u0b = s0v[:, :, 4].unsqueeze(2).to_broadcast((P, NV, C))
            s0u1b = s0v[:, :, 5].unsqueeze(2).to_broadcast((P, NV, C))
            tqa = tmpp.tile([P, NV * C], dt, tag="c32", name="tqa")
            tqav = tqa[:].rearrange("p (v c) -> p v c", v=NV)
            tqb = tmpp.tile([P, NV * C], dt, tag="c32", name="tqb")
            tqbv = tqb[:].rearrange("p (v c) -> p v c", v=NV)
            for (de, a1, a2, au) in ((0, 0, 1, 4), (1, 2, 3, 5)):
                nc.vector.tensor_tensor(tqav, PmEv[:, a1], s0u0b, op=A.mult)
                nc.vector.tensor_tensor(tqbv, PmEv[:, a2], s0u1b, op=A.mult)
                nc.vector.tensor_tensor(tqav, tqav, tqbv, op=A.add)
                nc.vector.tensor_tensor(abv[:, de], tqav, PmEv[:, au],
                                        op=A.add)
            # gamma = 1 - a - b
            nc.vector.tensor_tensor(tqav, abv[:, 0], abv[:, 1], op=A.add)
            nc.vector.tensor_scalar(abv[:, 2], tqav, -1.0, 1.0,
                                    op0=A.mult, op1=A.add)

            # ---------- correction + wet ----------
            yout = big()
            youtv = yout[:].rearrange("p (v c i) -> p v c i", v=NV, c=C)
            a_b = abv[:, 0].unsqueeze(3).to_broadcast((P, NV, C, L))
            b_b = abv[:, 1].unsqueeze(3).to_broadcast((P, NV, C, L))
            g_b = abv[:, 2].unsqueeze(3).to_broadcast((P, NV, C, L))
            ct1 = big()
            ct1v = ct1[:].rearrange("p (v c i) -> p v c i", v=NV, c=C)
            ct2 = big()
            ct2v = ct2[:].rearrange("p (v c i) -> p v c i", v=NV, c=C)
            nc.vector.tensor_tensor(ct1v, Yv[:, :, :, 1, 2:L + 2], a_b,
                                    op=A.mult)
            nc.vector.tensor_tensor(ct2v, Yv[:, :, :, 2, 2:L + 2], b_b,
                                    op=A.mult)
            nc.vector.tensor_tensor(ct1v, ct1v, ct2v, op=A.add)
            nc.vector.tensor_tensor(ct2v, Yv[:, :, :, 0, 2:L + 2], g_b,
                                    op=A.mult)
            nc.vector.tensor_tensor(youtv, ct1v, ct2v, op=A.add)
            wet = big()
            for v in range(NV):
                nc.scalar.activation(vsl(wet, v), vsl(yout, v), F.Tanh,
                                     scale=vc1(v, K_DIST))
            nc.sync.dma_start(dview(wet_d),
                              wet[:].rearrange("p (v j) -> p v j", v=NV))

    nc.compile()
    return nc


def _host_prep(inputs):
    """Per-core input maps. Heavy phase cumsum matches eager CPU-XLA jax."""
    import jax
    import jax.numpy as jnp

    cpu = jax.devices("cpu")[0]
    f0 = np.asarray(inputs["f0_hz"], f32)
    s = np.asarray(inputs["osc_shape"], f32)
    note_on = np.asarray(inputs["note_on_duration"], f32)
    fcm = np.asarray(inputs["fc_mod_sig"], f32)
    qmm = np.asarray(inputs["q_mod_sig"], f32)
    dist = np.asarray(inputs["dist_gain"], f32)
    att = np.asarray(inputs["attack"], f32)
    dec = np.asarray(inputs["decay"], f32)
    sus = np.asarray(inputs["sustain"], f32)
    rel = np.asarray(inputs["release"], f32)
    alpha = np.asarray(inputs["alpha"], f32)
    Bn = f0.shape[0]

    inc = (f0 / f32(SR)).astype(f32)
    incb = jax.device_put(
        np.broadcast_to(inc[:, None], (Bn, T)).astype(f32), cpu)
    cx = np.asarray(jax.jit(lambda a: jnp.cumsum(a, axis=1))(incb))
    p32 = (f32(2 * np.pi) * cx).astype(f32)
    r64 = np.mod(p32.astype(np.float64), 2 * np.pi)
    r64 = np.where(r64 > np.pi, r64 - 2 * np.pi, r64)
    rph = r64.astype(f32)

    partials = (f32(12000.0) / (f0 * np.log10(f0))).astype(f32)
    vc_cols = np.zeros((Bn, NK), f32)
    vc_cols[:, K_TANH] = f32(np.pi) * partials / f32(2.0)
    vc_cols[:, K_S] = s
    vc_cols[:, K_DRYSC] = f32(0.5) * (f32(1.0) - s / f32(2.0))
    vc_cols[:, K_IATT] = f32(1.0) / att
    vc_cols[:, K_ALPHA] = alpha
    vc_cols[:, K_IDEC] = f32(1.0) / dec
    vc_cols[:, K_BDEC] = -(att / dec)
    vc_cols[:, K_N1MS] = -(f32(1.0) - sus)
    vc_cols[:, K_IREL] = f32(1.0) / rel
    vc_cols[:, K_BREL] = -(note_on / rel)
    vc_cols[:, K_DIST] = dist

    tsec = (np.arange(T, dtype=f32) / f32(SR)).astype(f32).reshape(P, 1024)

    shm = np.zeros((P, 8 * P), f32)
    for si, sh in enumerate((1, 2, 4, 8, 16, 32, 64)):
        m = np.zeros((P, P), f32)
        m[np.arange(P - sh), np.arange(sh, P)] = 1.0
        shm[:, si * P:(si + 1) * P] = m

    in_maps = []
    for core in range(NCORES):
        vs = slice(core * NV, (core + 1) * NV)
        vc = np.ascontiguousarray(
            np.broadcast_to(vc_cols[vs].reshape(1, NV * NK), (P, NV * NK))
        ).astype(f32)
        in_maps.append(dict(
            rph=np.ascontiguousarray(rph[vs]),
            fcm=np.ascontiguousarray(fcm[vs]),
            qmm=np.ascontiguousarray(qmm[vs]),
            tsec=tsec, vc=vc, shm=shm,
        ))
    return in_maps


def run(inputs, trace=False):
    from concourse.bass_utils import run_bass_kernel_spmd

    if "nc" not in _CACHE:
        _CACHE["nc"] = _build_module()
    nc = _CACHE["nc"]
    in_maps = _host_prep(inputs)
    res = run_bass_kernel_spmd(nc, in_maps, core_ids=list(range(NCORES)),
                               trace=trace)
    dry = np.concatenate([r["dry"] for r in res.results], axis=0)
    wet = np.concatenate([r["wet"] for r in res.results], axis=0)
    env = np.concatenate([r["env"] for r in res.results], axis=0)
    return (dry, wet, env), res


def kernel(**inputs):
    out, _ = run(inputs, trace=False)
    return out


# revision 17
# speedup vs baseline: 1.0454x; 1.0454x over previous
"""AcidSynth Trainium2 kernel: 32 voices sharded 4-per-core across 8 NeuronCores.

Pipeline per core (4 voices, T=131072):
  - VCO sin/cos from host-reduced phase (ACT Sin), tanh square-saw morph
  - ADSR envelope (ACT Ln/Exp power curves)
  - RBJ time-varying lowpass biquad coefficients (ACT Exp/Sin + DVE)
  - IIR scan: block-parallel 3-channel method (particular + 2 homogeneous
    runs per block; linear correction), in-block steps vectorized over
    128 partitions x (4 voices x 8 chunks); carry chain composed
    intra-partition, then across partitions via PE shift-matmul
    Hillis-Steele scan
  - tanh distortion
"""
import numpy as np

f32 = np.float32
SR = 48000.0
T = 131072
P = 128
NV = 4            # voices per core
C = 8             # chunks per partition per voice
L = 1024 // C     # in-block scan length (128)
NCORES = 8

MIN_W = 2.0 * np.pi * 20.0 / SR
MAX_W = 2.0 * np.pi * 8000.0 / SR
MIN_Q = 0.7071
MAX_Q = 8.0

# voice-const column indices (vc tensor [128, NV*NK], col = v*NK + k)
K_TANH, K_S, K_DRYSC, K_IATT, K_ALPHA, K_IDEC, K_BDEC, K_N1MS, K_IREL, \
    K_BREL, K_DIST = range(11)
NK = 11

_CACHE = {}


def _build_module():
    import concourse.bacc as bacc
    import concourse.mybir as mybir
    from concourse import tile

    dt = mybir.dt.float32
    A = mybir.AluOpType
    F = mybir.ActivationFunctionType

    nc = bacc.Bacc()
    rph_d = nc.dram_tensor("rph", [NV, T], dt, kind="ExternalInput")
    fc_d = nc.dram_tensor("fcm", [NV, T], dt, kind="ExternalInput")
    qm_d = nc.dram_tensor("qmm", [NV, T], dt, kind="ExternalInput")
    ts_d = nc.dram_tensor("tsec", [P, 1024], dt, kind="ExternalInput")
    vc_d = nc.dram_tensor("vc", [P, NV * NK], dt, kind="ExternalInput")
    sh_d = nc.dram_tensor("shm", [P, 8 * P], dt, kind="ExternalInput")
    import os as _os
    DBG = bool(int(_os.environ.get("KDEBUG", "0")))
    dry_d = nc.dram_tensor("dry", [NV, T], dt, kind="ExternalOutput")
    wet_d = nc.dram_tensor("wet", [NV, T], dt, kind="ExternalOutput")
    env_d = nc.dram_tensor("env", [NV, T], dt, kind="ExternalOutput")
    if DBG:
        dbgw_d = nc.dram_tensor("dbgw", [NV, T], dt, kind="ExternalOutput")
        dbga_d = nc.dram_tensor("dbga", [NV, T], dt, kind="ExternalOutput")
        dbgy_d = nc.dram_tensor("dbgy", [P, NV * C * 3 * (L + 2)], dt,
                                kind="ExternalOutput")
        dbgab_d = nc.dram_tensor("dbgab", [P, 3 * NV * C], dt,
                                 kind="ExternalOutput")

    def dview(d):  # DRAM [NV, T] as [128, NV, 1024]
        return d[:, :].rearrange("v (p j) -> p v j", p=P)

    K1 = float(np.log(MAX_W / MIN_W))
    K0 = float(np.log(MIN_W))
    Kq1 = float(np.log(MAX_Q / MIN_Q))
    Kq0 = float(np.log(MIN_Q))
    PI = float(np.pi)
    BQ = -(Kq0 + float(np.log(2.0)))

    # ACT float biases need pre-registered const APs
    for val in (PI / 2, K0, BQ):
        t = nc.alloc_sbuf_tensor(f"constap-{val}", [128, 1], dt)
        nc.gpsimd.memset(t.ap(), val)
        nc.const_aps.aps[(dt, float(val))] = t.ap()
    nc.all_engine_barrier()

    with tile.TileContext(nc) as tc:
        with (
            tc.tile_pool(name="big", bufs=8) as bigp,
            tc.tile_pool(name="ybuf", bufs=1) as ypool,
            tc.tile_pool(name="keep", bufs=1) as keepp,
            tc.tile_pool(name="tmps", bufs=10) as tmpp,
            tc.tile_pool(name="psum", bufs=2, space="PSUM") as psp,
        ):
            def big():
                return bigp.tile([P, NV * 1024], dt, tag="big", name="bigt")

            def keep(name, shape):
                return keepp.tile([P, *shape] if shape else [P, 1], dt,
                                  tag=name, name=name)

            def vsl(t, v):
                return t[:, v * 1024:(v + 1) * 1024]

            def vc1(v, k):
                return vc_t[:, v * NK + k: v * NK + k + 1]

            # ---------- small persistent loads ----------
            ts_t = keepp.tile([P, 1024], dt, tag="tsec")
            nc.sync.dma_start(ts_t[:], ts_d[:, :])
            vc_t = keepp.tile([P, NV * NK], dt, tag="vc")
            nc.sync.dma_start(vc_t[:], vc_d[:, :])
            sh_t = keepp.tile([P, 8 * P], dt, tag="shm")
            nc.sync.dma_start(sh_t[:], sh_d[:, :])

            # ---------- phase -> sin, cos ----------
            rph = big()
            nc.sync.dma_start(rph[:].rearrange("p (v j) -> p v j", v=NV),
                              dview(rph_d))
            sn = big()
            nc.scalar.activation(sn[:], rph[:], F.Sin)
            ab = big()
            nc.scalar.activation(ab[:], rph[:], F.Abs)
            cs = big()
            nc.scalar.activation(cs[:], ab[:], F.Sin, bias=PI / 2, scale=-1.0)

            # ---------- ADSR envelope ----------
            env = big()
            tA = ab  # reuse slot logically: Tile tracks deps; use fresh tiles
            tA = big()
            for v in range(NV):
                # att -> env_v
                nc.vector.tensor_scalar(vsl(env, v)[:, 0:1024], ts_t[:],
                                        vc1(v, K_IATT), 1.0, op0=A.mult,
                                        op1=A.min)
                nc.scalar.activation(vsl(env, v), vsl(env, v), F.Ln)
                nc.scalar.activation(vsl(env, v), vsl(env, v), F.Exp,
                                     scale=vc1(v, K_ALPHA))
                # dec -> tA_v
                nc.scalar.activation(vsl(tA, v), ts_t[:], F.Relu,
                                     bias=vc1(v, K_BDEC),
                                     scale=vc1(v, K_IDEC))
                nc.vector.tensor_scalar(vsl(tA, v), vsl(tA, v), 1.0, None,
                                        op0=A.min)
                nc.scalar.activation(vsl(tA, v), vsl(tA, v), F.Ln)
                nc.scalar.activation(vsl(tA, v), vsl(tA, v), F.Exp,
                                     scale=vc1(v, K_ALPHA))
                nc.scalar.activation(vsl(tA, v), vsl(tA, v), F.Identity,
                                     bias=1.0, scale=vc1(v, K_N1MS))
            nc.vector.tensor_tensor(env[:], env[:], tA[:], op=A.mult)
            for v in range(NV):
                # rel -> tA_v
                nc.scalar.activation(vsl(tA, v), ts_t[:], F.Relu,
                                     bias=vc1(v, K_BREL),
                                     scale=vc1(v, K_IREL))
                nc.vector.tensor_scalar(vsl(tA, v), vsl(tA, v), 1.0, None,
                                        op0=A.min)
                nc.scalar.activation(vsl(tA, v), vsl(tA, v), F.Ln)
                nc.scalar.activation(vsl(tA, v), vsl(tA, v), F.Exp,
                                     scale=vc1(v, K_ALPHA))
                nc.scalar.activation(vsl(tA, v), vsl(tA, v), F.Identity,
                                     bias=1.0, scale=-1.0)
            nc.vector.tensor_tensor(env[:], env[:], tA[:], op=A.mult)
            nc.sync.dma_start(dview(env_d),
                              env[:].rearrange("p (v j) -> p v j", v=NV))

            # ---------- VCO mix -> dry ----------
            sq = big()
            mx1 = big()
            for v in range(NV):
                nc.scalar.activation(vsl(sq, v), vsl(sn, v), F.Tanh,
                                     scale=vc1(v, K_TANH))
                nc.scalar.activation(vsl(mx1, v), vsl(cs, v), F.Identity,
                                     bias=1.0, scale=vc1(v, K_S))
            m2 = big()
            nc.gpsimd.tensor_tensor(m2[:], sq[:], mx1[:], op=A.mult)
            dry = big()
            for v in range(NV):
                nc.vector.scalar_tensor_tensor(
                    vsl(dry, v), vsl(m2, v), vc1(v, K_DRYSC), vsl(env, v),
                    op0=A.mult, op1=A.mult)
            nc.sync.dma_start(dview(dry_d),
                              dry[:].rearrange("p (v j) -> p v j", v=NV))

            # ---------- biquad coefficients ----------
            fc = big()
            nc.sync.dma_start(fc[:].rearrange("p (v j) -> p v j", v=NV),
                              dview(fc_d))
            qm = big()
            nc.sync.dma_start(qm[:].rearrange("p (v j) -> p v j", v=NV),
                              dview(qm_d))
            w0 = big()
            nc.scalar.activation(w0[:], fc[:], F.Exp, bias=K0, scale=K1)
            ee = big()
            nc.scalar.activation(ee[:], qm[:], F.Exp, bias=BQ, scale=-Kq1)
            sn0 = big()
            nc.scalar.activation(sn0[:], w0[:], F.Sin)
            cw = big()
            nc.scalar.activation(cw[:], w0[:], F.Sin, bias=PI / 2, scale=1.0)
            al = big()
            nc.gpsimd.tensor_tensor(al[:], sn0[:], ee[:], op=A.mult)
            a0 = big()
            nc.scalar.activation(a0[:], al[:], F.Identity, bias=1.0,
                                 scale=1.0)
            ra = big()
            rs = big()
            nc.vector.reciprocal_approx_accurate(ra[:], a0[:], rs[:])
            nb1 = big()
            nc.vector.scalar_tensor_tensor(nb1[:], cw[:], 1.0, ra[:],
                                           op0=A.subtract, op1=A.mult)
            na1 = big()
            nc.vector.scalar_tensor_tensor(na1[:], cw[:], 2.0, ra[:],
                                           op0=A.mult, op1=A.mult)
            na2 = big()
            nc.vector.scalar_tensor_tensor(na2[:], al[:], 1.0, ra[:],
                                           op0=A.subtract, op1=A.mult)

            # ---------- FIR: w = b1*(0.5*(x + x2) + x1) = nb1*(-0.5*(x+x2)-x1)
            x1 = big()
            x2 = big()
            for v in range(NV):
                nc.gpsimd.tensor_copy(vsl(x1, v)[:, 1:1024],
                                      vsl(dry, v)[:, 0:1023])
                nc.vector.memset(x1[0:1, v * 1024:v * 1024 + 1], 0.0)
                nc.sync.dma_start(x1[1:P, v * 1024:v * 1024 + 1],
                                  dry[0:P - 1, (v + 1) * 1024 - 1:
                                      (v + 1) * 1024])
            for v in range(NV):
                nc.gpsimd.tensor_copy(vsl(x2, v)[:, 1:1024],
                                      vsl(x1, v)[:, 0:1023])
                nc.vector.memset(x2[0:1, v * 1024:v * 1024 + 1], 0.0)
                nc.sync.dma_start(x2[1:P, v * 1024:v * 1024 + 1],
                                  x1[0:P - 1, (v + 1) * 1024 - 1:
                                     (v + 1) * 1024])
            ssum = big()
            nc.gpsimd.tensor_tensor(ssum[:], dry[:], x2[:], op=A.add)
            uu = big()
            nc.vector.scalar_tensor_tensor(uu[:], ssum[:], -0.5, x1[:],
                                           op0=A.mult, op1=A.subtract)
            wf = big()
            nc.vector.tensor_tensor(wf[:], uu[:], nb1[:], op=A.mult)

            if DBG:
                nc.sync.dma_start(dview(dbgw_d),
                                  wf[:].rearrange("p (v j) -> p v j", v=NV))
                nc.sync.dma_start(dview(dbga_d),
                                  na1[:].rearrange("p (v j) -> p v j", v=NV))
            # ---------- blocked IIR scan, level 1 ----------
            Y = ypool.tile([P, NV * C * 3 * (L + 2)], dt, tag="Y")
            Yv = Y[:].rearrange("p (v c ch i) -> p v c ch i", v=NV, c=C,
                                ch=3)
            nc.vector.memset(Yv[:, :, :, 0, 0:2], 0.0)
            nc.vector.memset(Yv[:, :, :, 1, 0:1], 0.0)
            nc.vector.memset(Yv[:, :, :, 1, 1:2], 1.0)
            nc.vector.memset(Yv[:, :, :, 2, 0:1], 1.0)
            nc.vector.memset(Yv[:, :, :, 2, 1:2], 0.0)

            # pair-step coefficients: y[2k+1] = PA*y[2k-1] + PB*y[2k-2] + PR
            #   PA = na1'*na1 + na2',  PB = na1'*na2,  PR = na1'*w + w'
            # packed as [p, g(32), k(64), r(2)] with r = (even formula, pair)
            na1p = na1[:].rearrange("p (g k r) -> p g k r", g=NV * C, k=L // 2)
            na2p = na2[:].rearrange("p (g k r) -> p g k r", g=NV * C, k=L // 2)
            wfp = wf[:].rearrange("p (g k r) -> p g k r", g=NV * C, k=L // 2)
            pk1 = big()
            pk1v = pk1[:].rearrange("p (g k r) -> p g k r", g=NV * C,
                                    k=L // 2)
            nc.gpsimd.tensor_copy(pk1v[:, :, :, 0], na1p[:, :, :, 0])
            nc.vector.tensor_tensor(pk1v[:, :, :, 1], na1p[:, :, :, 1],
                                    na1p[:, :, :, 0], op=A.mult)
            nc.vector.tensor_tensor(pk1v[:, :, :, 1], pk1v[:, :, :, 1],
                                    na2p[:, :, :, 1], op=A.add)
            pk2 = big()
            pk2v = pk2[:].rearrange("p (g k r) -> p g k r", g=NV * C,
                                    k=L // 2)
            nc.gpsimd.tensor_copy(pk2v[:, :, :, 0], na2p[:, :, :, 0])
            nc.vector.tensor_tensor(pk2v[:, :, :, 1], na1p[:, :, :, 1],
                                    na2p[:, :, :, 0], op=A.mult)
            pkw = big()
            pkwv = pkw[:].rearrange("p (g k r) -> p g k r", g=NV * C,
                                    k=L // 2)
            nc.gpsimd.tensor_copy(pkwv[:, :, :, 0], wfp[:, :, :, 0])
            nc.vector.tensor_tensor(pkwv[:, :, :, 1], na1p[:, :, :, 1],
                                    wfp[:, :, :, 0], op=A.mult)
            nc.vector.tensor_tensor(pkwv[:, :, :, 1], pkwv[:, :, :, 1],
                                    wfp[:, :, :, 1], op=A.add)

            Yg = Y[:].rearrange("p (g ch i) -> p g ch i", g=NV * C, ch=3)
            G = NV * C

            def bcy(col):
                return Yg[:, :, :, col].unsqueeze(3).to_broadcast(
                    (P, G, 3, 2))

            def bck(pkv, k):
                return pkv[:, :, k, :].unsqueeze(2).to_broadcast(
                    (P, G, 3, 2))

            import os as _os2
            _SKIP = bool(int(_os2.environ.get("KSKIP_SCAN", "0")))
            for k in range(0 if _SKIP else L // 2):
                c0 = 2 * k
                t1 = tmpp.tile([P, G * 3 * 2], dt, tag="sc", name="t1")
                t1v = t1[:].rearrange("p (g ch r) -> p g ch r", g=G, ch=3)
                t2 = tmpp.tile([P, G * 3 * 2], dt, tag="sc", name="t2")
                t2v = t2[:].rearrange("p (g ch r) -> p g ch r", g=G, ch=3)
                nc.vector.tensor_tensor(t1v, bcy(c0 + 1), bck(pk1v, k),
                                        op=A.mult)
                nc.gpsimd.tensor_tensor(t2v, bcy(c0), bck(pk2v, k),
                                        op=A.mult)
                nc.vector.tensor_tensor(t1v, t1v, t2v, op=A.add)
                nc.vector.tensor_tensor(Yg[:, :, :, c0 + 2:c0 + 4], t1v,
                                        bck(pkwv, k), op=A.add)

            # ---------- block transfers ----------
            # T entries [128, NV, C]: h1e h2e h1e2 h2e2 ue ue2
            TmE = keep("TmE", [NV * C * 6])
            TmEv = TmE[:].rearrange("p (e v c) -> p e v c", e=6, v=NV)
            for (e, ch, col) in ((0, 1, L + 1), (1, 2, L + 1), (2, 1, L),
                                 (3, 2, L)):
                nc.vector.tensor_tensor(TmEv[:, e], Yv[:, :, :, ch, col],
                                        Yv[:, :, :, 0, col], op=A.subtract)
            nc.vector.tensor_copy(TmEv[:, 4], Yv[:, :, :, 0, L + 1])
            nc.vector.tensor_copy(TmEv[:, 5], Yv[:, :, :, 0, L])

            # ---------- intra-partition exclusive prefixes over c ----------
            # Pm00 Pm01 Pm10 Pm11 Pu0 Pu1: [128, 6, NV, C]
            PmE = keep("PmE", [6 * NV * C])
            PmEv = PmE[:].rearrange("p (e v c) -> p e v c", e=6, v=NV)
            for e in range(6):
                init = 1.0 if e in (0, 3) else 0.0
                nc.vector.memset(PmEv[:, e, :, 0:1], init)

            def compose_step(dst, dcc, S, scc, Tt, tcc):
                """dst[:,:,dcc] = T[:,:,tcc] o S[:,:,scc] (T applied second)"""
                tq = tmpp.tile([P, NV], dt, tag="c4", name="tq")
                tq2 = tmpp.tile([P, NV], dt, tag="c4", name="tq2")
                # m: new00 = T00*S00 + T01*S10 ; new01 = T00*S01 + T01*S11
                #    new10 = T10*S00 + T11*S10 ; new11 = T10*S01 + T11*S11
                for (de, a1, b1_, a2, b2_) in ((0, 0, 0, 1, 2), (1, 0, 1, 1, 3),
                                               (2, 2, 0, 3, 2), (3, 2, 1, 3, 3)):
                    nc.vector.tensor_tensor(tq[:], Tt[:, a1, :, tcc],
                                            S[:, b1_, :, scc], op=A.mult)
                    nc.vector.tensor_tensor(tq2[:], Tt[:, a2, :, tcc],
                                            S[:, b2_, :, scc], op=A.mult)
                    nc.vector.tensor_tensor(dst[:, de, :, dcc], tq[:], tq2[:],
                                            op=A.add)
                # u: newu0 = T00*Su0 + T01*Su1 + Tu0 ; newu1 = T10.. + Tu1
                tq3 = tmpp.tile([P, NV], dt, tag="c4", name="tq3")
                tq4 = tmpp.tile([P, NV], dt, tag="c4", name="tq4")
                for (de, a1, a2, au) in ((4, 0, 1, 4), (5, 2, 3, 5)):
                    nc.gpsimd.tensor_tensor(tq3[:], Tt[:, a1, :, tcc],
                                            S[:, 4, :, scc], op=A.mult)
                    nc.gpsimd.tensor_tensor(tq4[:], Tt[:, a2, :, tcc],
                                            S[:, 5, :, scc], op=A.mult)
                    nc.gpsimd.tensor_tensor(tq3[:], tq3[:], tq4[:], op=A.add)
                    nc.gpsimd.tensor_tensor(dst[:, de, :, dcc], tq3[:],
                                            Tt[:, au, :, tcc], op=A.add)

            for cc in range(1, C):
                compose_step(PmEv, cc, PmEv, cc - 1, TmEv, cc - 1)

            # full-partition transfer F = T_{C-1} o P_{C-1}, packed [128,NV,6]
            Fp = keep("Fp", [NV * 6])
            FpE = Fp[:].rearrange("p (e v c) -> p e v c", e=6, v=NV, c=1)
            compose_step(FpE, 0, PmEv, C - 1, TmEv, C - 1)

            # ---------- Hillis-Steele across partitions via PE shifts -------
            # pack to [128, NV*6] with (v, e) layout for matmul rhs
            Xp = keep("Xp", [NV * 6])
            Xv = Xp[:].rearrange("p (v e) -> p v e", v=NV)
            for e in range(6):
                nc.vector.tensor_copy(Xv[:, :, e], FpE[:, e, :, 0])

            cur = Xp
            for si, s in enumerate((1, 2, 4, 8, 16, 32, 64)):
                sh = tmpp.tile([P, NV * 6], dt, tag="hs", name="sh")
                nc.vector.memset(sh[0:s, :], 0.0)
                nc.sync.dma_start(sh[s:P, :], cur[0:P - s, :])
                shv = sh[:].rearrange("p (v e) -> p v e", v=NV)
                nc.vector.memset(shv[0:s, :, 0:1], 1.0)
                nc.vector.memset(shv[0:s, :, 3:4], 1.0)
                nxt = tmpp.tile([P, NV * 6], dt, tag="hs", name="nxt")
                nxtv = nxt[:].rearrange("p (v e) -> p v e", v=NV)
                curv = cur[:].rearrange("p (v e) -> p v e", v=NV)
                tq = tmpp.tile([P, NV], dt, tag="c4", name="tq")
                tq2 = tmpp.tile([P, NV], dt, tag="c4", name="tq2")
                for (de, a1, b1_, a2, b2_) in ((0, 0, 0, 1, 2), (1, 0, 1, 1, 3),
                                               (2, 2, 0, 3, 2), (3, 2, 1, 3, 3)):
                    nc.vector.tensor_tensor(tq[:], curv[:, :, a1],
                                            shv[:, :, b1_], op=A.mult)
                    nc.vector.tensor_tensor(tq2[:], curv[:, :, a2],
                                            shv[:, :, b2_], op=A.mult)
                    nc.vector.tensor_tensor(nxtv[:, :, de], tq[:], tq2[:],
                                            op=A.add)
                tq3 = tmpp.tile([P, NV], dt, tag="c4", name="tq3")
                tq4 = tmpp.tile([P, NV], dt, tag="c4", name="tq4")
                for (de, a1, a2) in ((4, 0, 1), (5, 2, 3)):
                    nc.gpsimd.tensor_tensor(tq3[:], curv[:, :, a1],
                                            shv[:, :, 4], op=A.mult)
                    nc.gpsimd.tensor_tensor(tq4[:], curv[:, :, a2],
                                            shv[:, :, 5], op=A.mult)
                    nc.gpsimd.tensor_tensor(tq3[:], tq3[:], tq4[:], op=A.add)
                    nc.gpsimd.tensor_tensor(nxtv[:, :, de], tq3[:],
                                            curv[:, :, de], op=A.add)
                cur = nxt

            # exclusive shift by 1: s0 = shift1(cur); rows<1 -> zeros
            s0 = keep("s0", [NV * 6])
            nc.vector.memset(s0[0:1, :], 0.0)
            nc.sync.dma_start(s0[1:P, :], cur[0:P - 1, :])
            s0v = s0[:].rearrange("p (v e) -> p v e", v=NV)

            # ---------- per-block incoming states: ab = Pm@s0u + Pu --------
            abt = keep("abt", [3 * NV * C])
            abv = abt[:].rearrange("p (e v c) -> p e v c", e=3, v=NV)
            s0u0b = s0v[:, :, 4].unsqueeze(2).to_broadcast((P, NV, C))
            s0u1b = s0v[:, :, 5].unsqueeze(2).to_broadcast((P, NV, C))
            tqa = tmpp.tile([P, NV * C], dt, tag="c32", name="tqa")
            tqav = tqa[:].rearrange("p (v c) -> p v c", v=NV)
            tqb = tmpp.tile([P, NV * C], dt, tag="c32", name="tqb")
            tqbv = tqb[:].rearrange("p (v c) -> p v c", v=NV)
            for (de, a1, a2, au) in ((0, 0, 1, 4), (1, 2, 3, 5)):
                nc.vector.tensor_tensor(tqav, PmEv[:, a1], s0u0b, op=A.mult)
                nc.vector.tensor_tensor(tqbv, PmEv[:, a2], s0u1b, op=A.mult)
                nc.vector.tensor_tensor(tqav, tqav, tqbv, op=A.add)
                nc.vector.tensor_tensor(abv[:, de], tqav, PmEv[:, au],
                                        op=A.add)
            # gamma = 1 - a - b
            nc.vector.tensor_tensor(tqav, abv[:, 0], abv[:, 1], op=A.add)
            nc.vector.tensor_scalar(abv[:, 2], tqav, -1.0, 1.0,
                                    op0=A.mult, op1=A.add)

            # ---------- correction + wet ----------
            yout = big()
            youtv = yout[:].rearrange("p (v c i) -> p v c i", v=NV, c=C)
            a_b = abv[:, 0].unsqueeze(3).to_broadcast((P, NV, C, L))
            b_b = abv[:, 1].unsqueeze(3).to_broadcast((P, NV, C, L))
            g_b = abv[:, 2].unsqueeze(3).to_broadcast((P, NV, C, L))
            ct1 = big()
            ct1v = ct1[:].rearrange("p (v c i) -> p v c i", v=NV, c=C)
            ct2 = big()
            ct2v = ct2[:].rearrange("p (v c i) -> p v c i", v=NV, c=C)
            nc.vector.tensor_tensor(ct1v, Yv[:, :, :, 1, 2:L + 2], a_b,
                                    op=A.mult)
            nc.vector.tensor_tensor(ct2v, Yv[:, :, :, 2, 2:L + 2], b_b,
                                    op=A.mult)
            nc.vector.tensor_tensor(ct1v, ct1v, ct2v, op=A.add)
            nc.vector.tensor_tensor(ct2v, Yv[:, :, :, 0, 2:L + 2], g_b,
                                    op=A.mult)
            nc.vector.tensor_tensor(youtv, ct1v, ct2v, op=A.add)
            wet = big()
            for v in range(NV):
                nc.scalar.activation(vsl(wet, v), vsl(yout, v), F.Tanh,
                                     scale=vc1(v, K_DIST))
            nc.sync.dma_start(dview(wet_d),
                              wet[:].rearrange("p (v j) -> p v j", v=NV))

    nc.compile()
    return nc


def _host_prep(inputs):
    """Per-core input maps. Heavy phase cumsum matches eager CPU-XLA jax."""
    import jax
    import jax.numpy as jnp

    cpu = jax.devices("cpu")[0]
    f0 = np.asarray(inputs["f0_hz"], f32)
    s = np.asarray(inputs["osc_shape"], f32)
    note_on = np.asarray(inputs["note_on_duration"], f32)
    fcm = np.asarray(inputs["fc_mod_sig"], f32)
    qmm = np.asarray(inputs["q_mod_sig"], f32)
    dist = np.asarray(inputs["dist_gain"], f32)
    att = np.asarray(inputs["attack"], f32)
    dec = np.asarray(inputs["decay"], f32)
    sus = np.asarray(inputs["sustain"], f32)
    rel = np.asarray(inputs["release"], f32)
    alpha = np.asarray(inputs["alpha"], f32)
    Bn = f0.shape[0]

    inc = (f0 / f32(SR)).astype(f32)
    incb = jax.device_put(
        np.broadcast_to(inc[:, None], (Bn, T)).astype(f32), cpu)
    cx = np.asarray(jax.jit(lambda a: jnp.cumsum(a, axis=1))(incb))
    p32 = (f32(2 * np.pi) * cx).astype(f32)
    r64 = np.mod(p32.astype(np.float64), 2 * np.pi)
    r64 = np.where(r64 > np.pi, r64 - 2 * np.pi, r64)
    rph = r64.astype(f32)

    partials = (f32(12000.0) / (f0 * np.log10(f0))).astype(f32)
    vc_cols = np.zeros((Bn, NK), f32)
    vc_cols[:, K_TANH] = f32(np.pi) * partials / f32(2.0)
    vc_cols[:, K_S] = s
    vc_cols[:, K_DRYSC] = f32(0.5) * (f32(1.0) - s / f32(2.0))
    vc_cols[:, K_IATT] = f32(1.0) / att
    vc_cols[:, K_ALPHA] = alpha
    vc_cols[:, K_IDEC] = f32(1.0) / dec
    vc_cols[:, K_BDEC] = -(att / dec)
    vc_cols[:, K_N1MS] = -(f32(1.0) - sus)
    vc_cols[:, K_IREL] = f32(1.0) / rel
    vc_cols[:, K_BREL] = -(note_on / rel)
    vc_cols[:, K_DIST] = dist

    tsec = (np.arange(T, dtype=f32) / f32(SR)).astype(f32).reshape(P, 1024)

    shm = np.zeros((P, 8 * P), f32)
    for si, sh in enumerate((1, 2, 4, 8, 16, 32, 64)):
        m = np.zeros((P, P), f32)
        m[np.arange(P - sh), np.arange(sh, P)] = 1.0
        shm[:, si * P:(si + 1) * P] = m

    in_maps = []
    for core in range(NCORES):
        vs = slice(core * NV, (core + 1) * NV)
        vc = np.ascontiguousarray(
            np.broadcast_to(vc_cols[vs].reshape(1, NV * NK), (P, NV * NK))
        ).astype(f32)
        in_maps.append(dict(
            rph=np.ascontiguousarray(rph[vs]),
            fcm=np.ascontiguousarray(fcm[vs]),
            qmm=np.ascontiguousarray(qmm[vs]),
            tsec=tsec, vc=vc, shm=shm,
        ))
    return in_maps


def run(inputs, trace=False):
    from concourse.bass_utils import run_bass_kernel_spmd

    if "nc" not in _CACHE:
        _CACHE["nc"] = _build_module()
    nc = _CACHE["nc"]
    in_maps = _host_prep(inputs)
    res = run_bass_kernel_spmd(nc, in_maps, core_ids=list(range(NCORES)),
                               trace=trace)
    dry = np.concatenate([r["dry"] for r in res.results], axis=0)
    wet = np.concatenate([r["wet"] for r in res.results], axis=0)
    env = np.concatenate([r["env"] for r in res.results], axis=0)
    return (dry, wet, env), res


def kernel(**inputs):
    out, _ = run(inputs, trace=False)
    return out


# revision 22
# speedup vs baseline: 1.0461x; 1.0007x over previous
"""AcidSynth Trainium2 kernel: 32 voices sharded 4-per-core across 8 NeuronCores.

Pipeline per core (4 voices, T=131072):
  - VCO sin/cos from host-reduced phase (ACT Sin), tanh square-saw morph
  - ADSR envelope (ACT Ln/Exp power curves)
  - RBJ time-varying lowpass biquad coefficients (ACT Exp/Sin + DVE)
  - IIR scan: block-parallel 3-channel method (particular + 2 homogeneous
    runs per block; linear correction), in-block steps vectorized over
    128 partitions x (4 voices x 8 chunks); carry chain composed
    intra-partition, then across partitions via PE shift-matmul
    Hillis-Steele scan
  - tanh distortion
"""
import numpy as np

f32 = np.float32
SR = 48000.0
T = 131072
P = 128
NV = 4            # voices per core
C = 8             # chunks per partition per voice
L = 1024 // C     # in-block scan length (128)
NCORES = 8

MIN_W = 2.0 * np.pi * 20.0 / SR
MAX_W = 2.0 * np.pi * 8000.0 / SR
MIN_Q = 0.7071
MAX_Q = 8.0

# voice-const column indices (vc tensor [128, NV*NK], col = v*NK + k)
K_TANH, K_S, K_DRYSC, K_IATT, K_ALPHA, K_IDEC, K_BDEC, K_N1MS, K_IREL, \
    K_BREL, K_DIST = range(11)
NK = 11

_CACHE = {}


def _build_module():
    import concourse.bacc as bacc
    import concourse.mybir as mybir
    from concourse import tile

    dt = mybir.dt.float32
    A = mybir.AluOpType
    F = mybir.ActivationFunctionType

    nc = bacc.Bacc()
    rph_d = nc.dram_tensor("rph", [NV, T], dt, kind="ExternalInput")
    fc_d = nc.dram_tensor("fcm", [NV, T], dt, kind="ExternalInput")
    qm_d = nc.dram_tensor("qmm", [NV, T], dt, kind="ExternalInput")
    ts_d = nc.dram_tensor("tsec", [P, 1024], dt, kind="ExternalInput")
    vc_d = nc.dram_tensor("vc", [P, NV * NK], dt, kind="ExternalInput")
    sh_d = nc.dram_tensor("shm", [P, 8 * P], dt, kind="ExternalInput")
    import os as _os
    DBG = bool(int(_os.environ.get("KDEBUG", "0")))
    dry_d = nc.dram_tensor("dry", [NV, T], dt, kind="ExternalOutput")
    wet_d = nc.dram_tensor("wet", [NV, T], dt, kind="ExternalOutput")
    env_d = nc.dram_tensor("env", [NV, T], dt, kind="ExternalOutput")
    if DBG:
        dbgw_d = nc.dram_tensor("dbgw", [NV, T], dt, kind="ExternalOutput")
        dbga_d = nc.dram_tensor("dbga", [NV, T], dt, kind="ExternalOutput")
        dbgy_d = nc.dram_tensor("dbgy", [P, NV * C * 3 * (L + 2)], dt,
                                kind="ExternalOutput")
        dbgab_d = nc.dram_tensor("dbgab", [P, 3 * NV * C], dt,
                                 kind="ExternalOutput")

    def dview(d):  # DRAM [NV, T] as [128, NV, 1024]
        return d[:, :].rearrange("v (p j) -> p v j", p=P)

    K1 = float(np.log(MAX_W / MIN_W))
    K0 = float(np.log(MIN_W))
    Kq1 = float(np.log(MAX_Q / MIN_Q))
    Kq0 = float(np.log(MIN_Q))
    PI = float(np.pi)
    BQ = -(Kq0 + float(np.log(2.0)))

    # ACT float biases need pre-registered const APs
    for val in (PI / 2, K0, BQ):
        t = nc.alloc_sbuf_tensor(f"constap-{val}", [128, 1], dt)
        nc.gpsimd.memset(t.ap(), val)
        nc.const_aps.aps[(dt, float(val))] = t.ap()
    nc.all_engine_barrier()

    with tile.TileContext(nc) as tc:
        with (
            tc.tile_pool(name="big", bufs=8) as bigp,
            tc.tile_pool(name="ybuf", bufs=1) as ypool,
            tc.tile_pool(name="keep", bufs=1) as keepp,
            tc.tile_pool(name="tmps", bufs=16) as tmpp,
            tc.tile_pool(name="psum", bufs=2, space="PSUM") as psp,
        ):
            def big():
                return bigp.tile([P, NV * 1024], dt, tag="big", name="bigt")

            def keep(name, shape):
                return keepp.tile([P, *shape] if shape else [P, 1], dt,
                                  tag=name, name=name)

            def vsl(t, v):
                return t[:, v * 1024:(v + 1) * 1024]

            def vc1(v, k):
                return vc_t[:, v * NK + k: v * NK + k + 1]

            # ---------- small persistent loads ----------
            ts_t = keepp.tile([P, 1024], dt, tag="tsec")
            nc.sync.dma_start(ts_t[:], ts_d[:, :])
            vc_t = keepp.tile([P, NV * NK], dt, tag="vc")
            nc.sync.dma_start(vc_t[:], vc_d[:, :])
            sh_t = keepp.tile([P, 8 * P], dt, tag="shm")
            nc.sync.dma_start(sh_t[:], sh_d[:, :])

            # ---------- phase -> sin, cos ----------
            rph = big()
            nc.sync.dma_start(rph[:].rearrange("p (v j) -> p v j", v=NV),
                              dview(rph_d))
            sn = big()
            nc.scalar.activation(sn[:], rph[:], F.Sin)
            ab = big()
            nc.scalar.activation(ab[:], rph[:], F.Abs)
            cs = big()
            nc.scalar.activation(cs[:], ab[:], F.Sin, bias=PI / 2, scale=-1.0)

            # ---------- ADSR envelope ----------
            env = big()
            tA = ab  # reuse slot logically: Tile tracks deps; use fresh tiles
            tA = big()
            for v in range(NV):
                # att -> env_v
                nc.vector.tensor_scalar(vsl(env, v)[:, 0:1024], ts_t[:],
                                        vc1(v, K_IATT), 1.0, op0=A.mult,
                                        op1=A.min)
                nc.scalar.activation(vsl(env, v), vsl(env, v), F.Ln)
                nc.scalar.activation(vsl(env, v), vsl(env, v), F.Exp,
                                     scale=vc1(v, K_ALPHA))
                # dec -> tA_v
                nc.scalar.activation(vsl(tA, v), ts_t[:], F.Relu,
                                     bias=vc1(v, K_BDEC),
                                     scale=vc1(v, K_IDEC))
                nc.vector.tensor_scalar(vsl(tA, v), vsl(tA, v), 1.0, None,
                                        op0=A.min)
                nc.scalar.activation(vsl(tA, v), vsl(tA, v), F.Ln)
                nc.scalar.activation(vsl(tA, v), vsl(tA, v), F.Exp,
                                     scale=vc1(v, K_ALPHA))
                nc.scalar.activation(vsl(tA, v), vsl(tA, v), F.Identity,
                                     bias=1.0, scale=vc1(v, K_N1MS))
            nc.vector.tensor_tensor(env[:], env[:], tA[:], op=A.mult)
            for v in range(NV):
                # rel -> tA_v
                nc.scalar.activation(vsl(tA, v), ts_t[:], F.Relu,
                                     bias=vc1(v, K_BREL),
                                     scale=vc1(v, K_IREL))
                nc.vector.tensor_scalar(vsl(tA, v), vsl(tA, v), 1.0, None,
                                        op0=A.min)
                nc.scalar.activation(vsl(tA, v), vsl(tA, v), F.Ln)
                nc.scalar.activation(vsl(tA, v), vsl(tA, v), F.Exp,
                                     scale=vc1(v, K_ALPHA))
                nc.scalar.activation(vsl(tA, v), vsl(tA, v), F.Identity,
                                     bias=1.0, scale=-1.0)
            nc.vector.tensor_tensor(env[:], env[:], tA[:], op=A.mult)
            nc.sync.dma_start(dview(env_d),
                              env[:].rearrange("p (v j) -> p v j", v=NV))

            # ---------- VCO mix -> dry ----------
            sq = big()
            mx1 = big()
            for v in range(NV):
                nc.scalar.activation(vsl(sq, v), vsl(sn, v), F.Tanh,
                                     scale=vc1(v, K_TANH))
                nc.scalar.activation(vsl(mx1, v), vsl(cs, v), F.Identity,
                                     bias=1.0, scale=vc1(v, K_S))
            m2 = big()
            nc.gpsimd.tensor_tensor(m2[:], sq[:], mx1[:], op=A.mult)
            dry = big()
            for v in range(NV):
                nc.vector.scalar_tensor_tensor(
                    vsl(dry, v), vsl(m2, v), vc1(v, K_DRYSC), vsl(env, v),
                    op0=A.mult, op1=A.mult)
            nc.sync.dma_start(dview(dry_d),
                              dry[:].rearrange("p (v j) -> p v j", v=NV))

            # ---------- biquad coefficients ----------
            fc = big()
            nc.sync.dma_start(fc[:].rearrange("p (v j) -> p v j", v=NV),
                              dview(fc_d))
            qm = big()
            nc.sync.dma_start(qm[:].rearrange("p (v j) -> p v j", v=NV),
                              dview(qm_d))
            w0 = big()
            nc.scalar.activation(w0[:], fc[:], F.Exp, bias=K0, scale=K1)
            ee = big()
            nc.scalar.activation(ee[:], qm[:], F.Exp, bias=BQ, scale=-Kq1)
            sn0 = big()
            nc.scalar.activation(sn0[:], w0[:], F.Sin)
            cw = big()
            nc.scalar.activation(cw[:], w0[:], F.Sin, bias=PI / 2, scale=1.0)
            al = big()
            nc.gpsimd.tensor_tensor(al[:], sn0[:], ee[:], op=A.mult)
            a0 = big()
            nc.scalar.activation(a0[:], al[:], F.Identity, bias=1.0,
                                 scale=1.0)
            ra = big()
            rs = big()
            nc.vector.reciprocal_approx_accurate(ra[:], a0[:], rs[:])
            nb1 = big()
            nc.vector.scalar_tensor_tensor(nb1[:], cw[:], 1.0, ra[:],
                                           op0=A.subtract, op1=A.mult)
            na1 = big()
            nc.vector.scalar_tensor_tensor(na1[:], cw[:], 2.0, ra[:],
                                           op0=A.mult, op1=A.mult)
            na2 = big()
            nc.vector.scalar_tensor_tensor(na2[:], al[:], 1.0, ra[:],
                                           op0=A.subtract, op1=A.mult)

            # ---------- FIR: w = b1*(0.5*(x + x2) + x1) = nb1*(-0.5*(x+x2)-x1)
            x1 = big()
            x2 = big()
            for v in range(NV):
                nc.gpsimd.tensor_copy(vsl(x1, v)[:, 1:1024],
                                      vsl(dry, v)[:, 0:1023])
                nc.vector.memset(x1[0:1, v * 1024:v * 1024 + 1], 0.0)
                nc.sync.dma_start(x1[1:P, v * 1024:v * 1024 + 1],
                                  dry[0:P - 1, (v + 1) * 1024 - 1:
                                      (v + 1) * 1024])
            for v in range(NV):
                nc.gpsimd.tensor_copy(vsl(x2, v)[:, 1:1024],
                                      vsl(x1, v)[:, 0:1023])
                nc.vector.memset(x2[0:1, v * 1024:v * 1024 + 1], 0.0)
                nc.sync.dma_start(x2[1:P, v * 1024:v * 1024 + 1],
                                  x1[0:P - 1, (v + 1) * 1024 - 1:
                                     (v + 1) * 1024])
            ssum = big()
            nc.gpsimd.tensor_tensor(ssum[:], dry[:], x2[:], op=A.add)
            uu = big()
            nc.vector.scalar_tensor_tensor(uu[:], ssum[:], -0.5, x1[:],
                                           op0=A.mult, op1=A.subtract)
            wf = big()
            nc.vector.tensor_tensor(wf[:], uu[:], nb1[:], op=A.mult)

            if DBG:
                nc.sync.dma_start(dview(dbgw_d),
                                  wf[:].rearrange("p (v j) -> p v j", v=NV))
                nc.sync.dma_start(dview(dbga_d),
                                  na1[:].rearrange("p (v j) -> p v j", v=NV))
            # ---------- blocked IIR scan, level 1 ----------
            Y = ypool.tile([P, NV * C * 3 * (L + 2)], dt, tag="Y")
            Yv = Y[:].rearrange("p (v c ch i) -> p v c ch i", v=NV, c=C,
                                ch=3)
            nc.vector.memset(Yv[:, :, :, 0, 0:2], 0.0)
            nc.vector.memset(Yv[:, :, :, 1, 0:1], 0.0)
            nc.vector.memset(Yv[:, :, :, 1, 1:2], 1.0)
            nc.vector.memset(Yv[:, :, :, 2, 0:1], 1.0)
            nc.vector.memset(Yv[:, :, :, 2, 1:2], 0.0)

            # pair-step coefficients: y[2k+1] = PA*y[2k-1] + PB*y[2k-2] + PR
            #   PA = na1'*na1 + na2',  PB = na1'*na2,  PR = na1'*w + w'
            # packed as [p, g(32), k(64), r(2)] with r = (even formula, pair)
            na1p = na1[:].rearrange("p (g k r) -> p g k r", g=NV * C, k=L // 2)
            na2p = na2[:].rearrange("p (g k r) -> p g k r", g=NV * C, k=L // 2)
            wfp = wf[:].rearrange("p (g k r) -> p g k r", g=NV * C, k=L // 2)
            pk1 = big()
            pk1v = pk1[:].rearrange("p (g k r) -> p g k r", g=NV * C,
                                    k=L // 2)
            nc.gpsimd.tensor_copy(pk1v[:, :, :, 0], na1p[:, :, :, 0])
            nc.vector.tensor_tensor(pk1v[:, :, :, 1], na1p[:, :, :, 1],
                                    na1p[:, :, :, 0], op=A.mult)
            nc.vector.tensor_tensor(pk1v[:, :, :, 1], pk1v[:, :, :, 1],
                                    na2p[:, :, :, 1], op=A.add)
            pk2 = big()
            pk2v = pk2[:].rearrange("p (g k r) -> p g k r", g=NV * C,
                                    k=L // 2)
            nc.gpsimd.tensor_copy(pk2v[:, :, :, 0], na2p[:, :, :, 0])
            nc.vector.tensor_tensor(pk2v[:, :, :, 1], na1p[:, :, :, 1],
                                    na2p[:, :, :, 0], op=A.mult)
            pkw = big()
            pkwv = pkw[:].rearrange("p (g k r) -> p g k r", g=NV * C,
                                    k=L // 2)
            nc.gpsimd.tensor_copy(pkwv[:, :, :, 0], wfp[:, :, :, 0])
            nc.vector.tensor_tensor(pkwv[:, :, :, 1], na1p[:, :, :, 1],
                                    wfp[:, :, :, 0], op=A.mult)
            nc.vector.tensor_tensor(pkwv[:, :, :, 1], pkwv[:, :, :, 1],
                                    wfp[:, :, :, 1], op=A.add)

            Yg = Y[:].rearrange("p (g ch i) -> p g ch i", g=NV * C, ch=3)
            G = NV * C

            def bcy(col):
                return Yg[:, :, :, col].unsqueeze(3).to_broadcast(
                    (P, G, 3, 2))

            def bck(pkv, k):
                return pkv[:, :, k, :].unsqueeze(2).to_broadcast(
                    (P, G, 3, 2))

            import os as _os2
            _SKIP = bool(int(_os2.environ.get("KSKIP_SCAN", "0")))
            for k in range(0 if _SKIP else L // 2):
                c0 = 2 * k
                t1 = tmpp.tile([P, G * 3 * 2], dt, tag="sc", name="t1")
                t1v = t1[:].rearrange("p (g ch r) -> p g ch r", g=G, ch=3)
                t2 = tmpp.tile([P, G * 3 * 2], dt, tag="sc", name="t2")
                t2v = t2[:].rearrange("p (g ch r) -> p g ch r", g=G, ch=3)
                nc.vector.tensor_tensor(t1v, bcy(c0 + 1), bck(pk1v, k),
                                        op=A.mult)
                nc.gpsimd.tensor_tensor(t2v, bcy(c0), bck(pk2v, k),
                                        op=A.mult)
                nc.vector.tensor_tensor(t1v, t1v, t2v, op=A.add)
                nc.vector.tensor_tensor(Yg[:, :, :, c0 + 2:c0 + 4], t1v,
                                        bck(pkwv, k), op=A.add)

            # ---------- block transfers ----------
            # T entries [128, NV, C]: h1e h2e h1e2 h2e2 ue ue2
            TmE = keep("TmE", [NV * C * 6])
            TmEv = TmE[:].rearrange("p (e v c) -> p e v c", e=6, v=NV)
            for (e, ch, col) in ((0, 1, L + 1), (1, 2, L + 1), (2, 1, L),
                                 (3, 2, L)):
                nc.vector.tensor_tensor(TmEv[:, e], Yv[:, :, :, ch, col],
                                        Yv[:, :, :, 0, col], op=A.subtract)
            nc.vector.tensor_copy(TmEv[:, 4], Yv[:, :, :, 0, L + 1])
            nc.vector.tensor_copy(TmEv[:, 5], Yv[:, :, :, 0, L])

            # ---------- intra-partition exclusive prefixes over c ----------
            # Pm00 Pm01 Pm10 Pm11 Pu0 Pu1: [128, 6, NV, C]
            PmE = keep("PmE", [6 * NV * C])
            PmEv = PmE[:].rearrange("p (e v c) -> p e v c", e=6, v=NV)
            for e in range(6):
                init = 1.0 if e in (0, 3) else 0.0
                nc.vector.memset(PmEv[:, e, :, 0:1], init)

            def compose_step(dst, dcc, S, scc, Tt, tcc):
                """dst[:,:,dcc] = T[:,:,tcc] o S[:,:,scc] (T applied second)"""
                tq = tmpp.tile([P, NV], dt, tag="c4", name="tq")
                tq2 = tmpp.tile([P, NV], dt, tag="c4", name="tq2")
                # m: new00 = T00*S00 + T01*S10 ; new01 = T00*S01 + T01*S11
                #    new10 = T10*S00 + T11*S10 ; new11 = T10*S01 + T11*S11
                for (de, a1, b1_, a2, b2_) in ((0, 0, 0, 1, 2), (1, 0, 1, 1, 3),
                                               (2, 2, 0, 3, 2), (3, 2, 1, 3, 3)):
                    nc.vector.tensor_tensor(tq[:], Tt[:, a1, :, tcc],
                                            S[:, b1_, :, scc], op=A.mult)
                    nc.vector.tensor_tensor(tq2[:], Tt[:, a2, :, tcc],
                                            S[:, b2_, :, scc], op=A.mult)
                    nc.vector.tensor_tensor(dst[:, de, :, dcc], tq[:], tq2[:],
                                            op=A.add)
                # u: newu0 = T00*Su0 + T01*Su1 + Tu0 ; newu1 = T10.. + Tu1
                tq3 = tmpp.tile([P, NV], dt, tag="c4", name="tq3")
                tq4 = tmpp.tile([P, NV], dt, tag="c4", name="tq4")
                for (de, a1, a2, au) in ((4, 0, 1, 4), (5, 2, 3, 5)):
                    nc.gpsimd.tensor_tensor(tq3[:], Tt[:, a1, :, tcc],
                                            S[:, 4, :, scc], op=A.mult)
                    nc.gpsimd.tensor_tensor(tq4[:], Tt[:, a2, :, tcc],
                                            S[:, 5, :, scc], op=A.mult)
                    nc.gpsimd.tensor_tensor(tq3[:], tq3[:], tq4[:], op=A.add)
                    nc.gpsimd.tensor_tensor(dst[:, de, :, dcc], tq3[:],
                                            Tt[:, au, :, tcc], op=A.add)

            for cc in range(1, C):
                compose_step(PmEv, cc, PmEv, cc - 1, TmEv, cc - 1)

            # full-partition transfer F = T_{C-1} o P_{C-1}, packed [128,NV,6]
            Fp = keep("Fp", [NV * 6])
            FpE = Fp[:].rearrange("p (e v c) -> p e v c", e=6, v=NV, c=1)
            compose_step(FpE, 0, PmEv, C - 1, TmEv, C - 1)

            # ---------- Hillis-Steele across partitions via PE shifts -------
            # pack to [128, NV*6] with (v, e) layout for matmul rhs
            Xp = keep("Xp", [NV * 6])
            Xv = Xp[:].rearrange("p (v e) -> p v e", v=NV)
            for e in range(6):
                nc.vector.tensor_copy(Xv[:, :, e], FpE[:, e, :, 0])

            cur = Xp
            for si, s in enumerate((1, 2, 4, 8, 16, 32, 64)):
                sh = tmpp.tile([P, NV * 6], dt, tag="hs", name="sh")
                nc.vector.memset(sh[0:s, :], 0.0)
                nc.sync.dma_start(sh[s:P, :], cur[0:P - s, :])
                shv = sh[:].rearrange("p (v e) -> p v e", v=NV)
                nc.vector.memset(shv[0:s, :, 0:1], 1.0)
                nc.vector.memset(shv[0:s, :, 3:4], 1.0)
                nxt = tmpp.tile([P, NV * 6], dt, tag="hs", name="nxt")
                nxtv = nxt[:].rearrange("p (v e) -> p v e", v=NV)
                curv = cur[:].rearrange("p (v e) -> p v e", v=NV)
                tq = tmpp.tile([P, NV], dt, tag="c4", name="tq")
                tq2 = tmpp.tile([P, NV], dt, tag="c4", name="tq2")
                for (de, a1, b1_, a2, b2_) in ((0, 0, 0, 1, 2), (1, 0, 1, 1, 3),
                                               (2, 2, 0, 3, 2), (3, 2, 1, 3, 3)):
                    nc.vector.tensor_tensor(tq[:], curv[:, :, a1],
                                            shv[:, :, b1_], op=A.mult)
                    nc.vector.tensor_tensor(tq2[:], curv[:, :, a2],
                                            shv[:, :, b2_], op=A.mult)
                    nc.vector.tensor_tensor(nxtv[:, :, de], tq[:], tq2[:],
                                            op=A.add)
                tq3 = tmpp.tile([P, NV], dt, tag="c4", name="tq3")
                tq4 = tmpp.tile([P, NV], dt, tag="c4", name="tq4")
                for (de, a1, a2) in ((4, 0, 1), (5, 2, 3)):
                    nc.gpsimd.tensor_tensor(tq3[:], curv[:, :, a1],
                                            shv[:, :, 4], op=A.mult)
                    nc.gpsimd.tensor_tensor(tq4[:], curv[:, :, a2],
                                            shv[:, :, 5], op=A.mult)
                    nc.gpsimd.tensor_tensor(tq3[:], tq3[:], tq4[:], op=A.add)
                    nc.gpsimd.tensor_tensor(nxtv[:, :, de], tq3[:],
                                            curv[:, :, de], op=A.add)
                cur = nxt

            # exclusive shift by 1: s0 = shift1(cur); rows<1 -> zeros
            s0 = keep("s0", [NV * 6])
            nc.vector.memset(s0[0:1, :], 0.0)
            nc.sync.dma_start(s0[1:P, :], cur[0:P - 1, :])
            s0v = s0[:].rearrange("p (v e) -> p v e", v=NV)

            # ---------- per-block incoming states: ab = Pm@s0u + Pu --------
            abt = keep("abt", [3 * NV * C])
            abv = abt[:].rearrange("p (e v c) -> p e v c", e=3, v=NV)
            s0u0b = s0v[:, :, 4].unsqueeze(2).to_broadcast((P, NV, C))
            s0u1b = s0v[:, :, 5].unsqueeze(2).to_broadcast((P, NV, C))
            tqa = tmpp.tile([P, NV * C], dt, tag="c32", name="tqa")
            tqav = tqa[:].rearrange("p (v c) -> p v c", v=NV)
            tqb = tmpp.tile([P, NV * C], dt, tag="c32", name="tqb")
            tqbv = tqb[:].rearrange("p (v c) -> p v c", v=NV)
            for (de, a1, a2, au) in ((0, 0, 1, 4), (1, 2, 3, 5)):
                nc.vector.tensor_tensor(tqav, PmEv[:, a1], s0u0b, op=A.mult)
                nc.vector.tensor_tensor(tqbv, PmEv[:, a2], s0u1b, op=A.mult)
                nc.vector.tensor_tensor(tqav, tqav, tqbv, op=A.add)
                nc.vector.tensor_tensor(abv[:, de], tqav, PmEv[:, au],
                                        op=A.add)
            # gamma = 1 - a - b
            nc.vector.tensor_tensor(tqav, abv[:, 0], abv[:, 1], op=A.add)
            nc.vector.tensor_scalar(abv[:, 2], tqav, -1.0, 1.0,
                                    op0=A.mult, op1=A.add)

            # ---------- correction + wet ----------
            yout = big()
            youtv = yout[:].rearrange("p (v c i) -> p v c i", v=NV, c=C)
            a_b = abv[:, 0].unsqueeze(3).to_broadcast((P, NV, C, L))
            b_b = abv[:, 1].unsqueeze(3).to_broadcast((P, NV, C, L))
            g_b = abv[:, 2].unsqueeze(3).to_broadcast((P, NV, C, L))
            ct1 = big()
            ct1v = ct1[:].rearrange("p (v c i) -> p v c i", v=NV, c=C)
            ct2 = big()
            ct2v = ct2[:].rearrange("p (v c i) -> p v c i", v=NV, c=C)
            nc.vector.tensor_tensor(ct1v, Yv[:, :, :, 1, 2:L + 2], a_b,
                                    op=A.mult)
            nc.vector.tensor_tensor(ct2v, Yv[:, :, :, 2, 2:L + 2], b_b,
                                    op=A.mult)
            nc.vector.tensor_tensor(ct1v, ct1v, ct2v, op=A.add)
            nc.vector.tensor_tensor(ct2v, Yv[:, :, :, 0, 2:L + 2], g_b,
                                    op=A.mult)
            nc.vector.tensor_tensor(youtv, ct1v, ct2v, op=A.add)
            wet = big()
            for v in range(NV):
                nc.scalar.activation(vsl(wet, v), vsl(yout, v), F.Tanh,
                                     scale=vc1(v, K_DIST))
            nc.sync.dma_start(dview(wet_d),
                              wet[:].rearrange("p (v j) -> p v j", v=NV))

    nc.compile()
    return nc


def _host_prep(inputs):
    """Per-core input maps. Heavy phase cumsum matches eager CPU-XLA jax."""
    import jax
    import jax.numpy as jnp

    cpu = jax.devices("cpu")[0]
    f0 = np.asarray(inputs["f0_hz"], f32)
    s = np.asarray(inputs["osc_shape"], f32)
    note_on = np.asarray(inputs["note_on_duration"], f32)
    fcm = np.asarray(inputs["fc_mod_sig"], f32)
    qmm = np.asarray(inputs["q_mod_sig"], f32)
    dist = np.asarray(inputs["dist_gain"], f32)
    att = np.asarray(inputs["attack"], f32)
    dec = np.asarray(inputs["decay"], f32)
    sus = np.asarray(inputs["sustain"], f32)
    rel = np.asarray(inputs["release"], f32)
    alpha = np.asarray(inputs["alpha"], f32)
    Bn = f0.shape[0]

    inc = (f0 / f32(SR)).astype(f32)
    incb = jax.device_put(
        np.broadcast_to(inc[:, None], (Bn, T)).astype(f32), cpu)
    cx = np.asarray(jax.jit(lambda a: jnp.cumsum(a, axis=1))(incb))
    p32 = (f32(2 * np.pi) * cx).astype(f32)
    r64 = np.mod(p32.astype(np.float64), 2 * np.pi)
    r64 = np.where(r64 > np.pi, r64 - 2 * np.pi, r64)
    rph = r64.astype(f32)

    partials = (f32(12000.0) / (f0 * np.log10(f0))).astype(f32)
    vc_cols = np.zeros((Bn, NK), f32)
    vc_cols[:, K_TANH] = f32(np.pi) * partials / f32(2.0)
    vc_cols[:, K_S] = s
    vc_cols[:, K_DRYSC] = f32(0.5) * (f32(1.0) - s / f32(2.0))
    vc_cols[:, K_IATT] = f32(1.0) / att
    vc_cols[:, K_ALPHA] = alpha
    vc_cols[:, K_IDEC] = f32(1.0) / dec
    vc_cols[:, K_BDEC] = -(att / dec)
    vc_cols[:, K_N1MS] = -(f32(1.0) - sus)
    vc_cols[:, K_IREL] = f32(1.0) / rel
    vc_cols[:, K_BREL] = -(note_on / rel)
    vc_cols[:, K_DIST] = dist

    tsec = (np.arange(T, dtype=f32) / f32(SR)).astype(f32).reshape(P, 1024)

    shm = np.zeros((P, 8 * P), f32)
    for si, sh in enumerate((1, 2, 4, 8, 16, 32, 64)):
        m = np.zeros((P, P), f32)
        m[np.arange(P - sh), np.arange(sh, P)] = 1.0
        shm[:, si * P:(si + 1) * P] = m

    in_maps = []
    for core in range(NCORES):
        vs = slice(core * NV, (core + 1) * NV)
        vc = np.ascontiguousarray(
            np.broadcast_to(vc_cols[vs].reshape(1, NV * NK), (P, NV * NK))
        ).astype(f32)
        in_maps.append(dict(
            rph=np.ascontiguousarray(rph[vs]),
            fcm=np.ascontiguousarray(fcm[vs]),
            qmm=np.ascontiguousarray(qmm[vs]),
            tsec=tsec, vc=vc, shm=shm,
        ))
    return in_maps


def run(inputs, trace=False):
    from concourse.bass_utils import run_bass_kernel_spmd

    if "nc" not in _CACHE:
        _CACHE["nc"] = _build_module()
    nc = _CACHE["nc"]
    in_maps = _host_prep(inputs)
    res = run_bass_kernel_spmd(nc, in_maps, core_ids=list(range(NCORES)),
                               trace=trace)
    dry = np.concatenate([r["dry"] for r in res.results], axis=0)
    wet = np.concatenate([r["wet"] for r in res.results], axis=0)
    env = np.concatenate([r["env"] for r in res.results], axis=0)
    return (dry, wet, env), res


def kernel(**inputs):
    out, _ = run(inputs, trace=False)
    return out


# revision 25
# speedup vs baseline: 1.0572x; 1.0106x over previous
"""AcidSynth Trainium2 kernel: 32 voices sharded 4-per-core across 8 NeuronCores.

Pipeline per core (4 voices, T=131072):
  - VCO sin/cos from host-reduced phase (ACT Sin), tanh square-saw morph
  - ADSR envelope (ACT Ln/Exp power curves)
  - RBJ time-varying lowpass biquad coefficients (ACT Exp/Sin + DVE)
  - IIR scan: block-parallel 3-channel method (particular + 2 homogeneous
    runs per block; linear correction), in-block steps vectorized over
    128 partitions x (4 voices x 8 chunks); carry chain composed
    intra-partition, then across partitions via PE shift-matmul
    Hillis-Steele scan
  - tanh distortion
"""
import numpy as np

f32 = np.float32
SR = 48000.0
T = 131072
P = 128
NV = 4            # voices per core
C = 8             # chunks per partition per voice
L = 1024 // C     # in-block scan length (128)
NCORES = 8

MIN_W = 2.0 * np.pi * 20.0 / SR
MAX_W = 2.0 * np.pi * 8000.0 / SR
MIN_Q = 0.7071
MAX_Q = 8.0

# voice-const column indices (vc tensor [128, NV*NK], col = v*NK + k)
K_TANH, K_S, K_DRYSC, K_IATT, K_ALPHA, K_IDEC, K_BDEC, K_N1MS, K_IREL, \
    K_BREL, K_DIST = range(11)
NK = 11

_CACHE = {}


def _build_module():
    import concourse.bacc as bacc
    import concourse.mybir as mybir
    from concourse import tile

    dt = mybir.dt.float32
    A = mybir.AluOpType
    F = mybir.ActivationFunctionType

    nc = bacc.Bacc()
    rph_d = nc.dram_tensor("rph", [NV, T], dt, kind="ExternalInput")
    fc_d = nc.dram_tensor("fcm", [NV, T], dt, kind="ExternalInput")
    qm_d = nc.dram_tensor("qmm", [NV, T], dt, kind="ExternalInput")
    ts_d = nc.dram_tensor("tsec", [P, 1024], dt, kind="ExternalInput")
    vc_d = nc.dram_tensor("vc", [P, NV * NK], dt, kind="ExternalInput")
    sh_d = nc.dram_tensor("shm", [P, 8 * P], dt, kind="ExternalInput")
    import os as _os
    DBG = bool(int(_os.environ.get("KDEBUG", "0")))
    dry_d = nc.dram_tensor("dry", [NV, T], dt, kind="ExternalOutput")
    wet_d = nc.dram_tensor("wet", [NV, T], dt, kind="ExternalOutput")
    env_d = nc.dram_tensor("env", [NV, T], dt, kind="ExternalOutput")
    if DBG:
        dbgw_d = nc.dram_tensor("dbgw", [NV, T], dt, kind="ExternalOutput")
        dbga_d = nc.dram_tensor("dbga", [NV, T], dt, kind="ExternalOutput")
        dbgy_d = nc.dram_tensor("dbgy", [P, NV * C * 3 * (L + 2)], dt,
                                kind="ExternalOutput")
        dbgab_d = nc.dram_tensor("dbgab", [P, 3 * NV * C], dt,
                                 kind="ExternalOutput")

    def dview(d):  # DRAM [NV, T] as [128, NV, 1024]
        return d[:, :].rearrange("v (p j) -> p v j", p=P)

    K1 = float(np.log(MAX_W / MIN_W))
    K0 = float(np.log(MIN_W))
    Kq1 = float(np.log(MAX_Q / MIN_Q))
    Kq0 = float(np.log(MIN_Q))
    PI = float(np.pi)
    BQ = -(Kq0 + float(np.log(2.0)))

    # ACT float biases need pre-registered const APs
    for val in (PI / 2, K0, BQ):
        t = nc.alloc_sbuf_tensor(f"constap-{val}", [128, 1], dt)
        nc.gpsimd.memset(t.ap(), val)
        nc.const_aps.aps[(dt, float(val))] = t.ap()
    nc.all_engine_barrier()

    with tile.TileContext(nc) as tc:
        with (
            tc.tile_pool(name="big", bufs=8) as bigp,
            tc.tile_pool(name="ybuf", bufs=1) as ypool,
            tc.tile_pool(name="keep", bufs=1) as keepp,
            tc.tile_pool(name="tmps", bufs=16) as tmpp,
            tc.tile_pool(name="psum", bufs=2, space="PSUM") as psp,
        ):
            def big():
                return bigp.tile([P, NV * 1024], dt, tag="big", name="bigt")

            def keep(name, shape):
                return keepp.tile([P, *shape] if shape else [P, 1], dt,
                                  tag=name, name=name)

            def vsl(t, v):
                return t[:, v * 1024:(v + 1) * 1024]

            def vc1(v, k):
                return vc_t[:, v * NK + k: v * NK + k + 1]

            # ---------- small persistent loads ----------
            ts_t = keepp.tile([P, 1024], dt, tag="tsec")
            nc.sync.dma_start(ts_t[:], ts_d[:, :])
            vc_t = keepp.tile([P, NV * NK], dt, tag="vc")
            nc.sync.dma_start(vc_t[:], vc_d[:, :])
            sh_t = keepp.tile([P, 8 * P], dt, tag="shm")
            nc.sync.dma_start(sh_t[:], sh_d[:, :])

            # ---------- phase -> sin, cos ----------
            rph = big()
            nc.sync.dma_start(rph[:].rearrange("p (v j) -> p v j", v=NV),
                              dview(rph_d))
            sn = big()
            nc.scalar.activation(sn[:], rph[:], F.Sin)
            ab = big()
            nc.scalar.activation(ab[:], rph[:], F.Abs)
            cs = big()
            nc.scalar.activation(cs[:], ab[:], F.Sin, bias=PI / 2, scale=-1.0)

            # ---------- ADSR envelope ----------
            env = big()
            tA = ab  # reuse slot logically: Tile tracks deps; use fresh tiles
            tA = big()
            for v in range(NV):
                # att -> env_v
                nc.vector.tensor_scalar(vsl(env, v)[:, 0:1024], ts_t[:],
                                        vc1(v, K_IATT), 1.0, op0=A.mult,
                                        op1=A.min)
                nc.scalar.activation(vsl(env, v), vsl(env, v), F.Ln)
                nc.scalar.activation(vsl(env, v), vsl(env, v), F.Exp,
                                     scale=vc1(v, K_ALPHA))
                # dec -> tA_v
                nc.scalar.activation(vsl(tA, v), ts_t[:], F.Relu,
                                     bias=vc1(v, K_BDEC),
                                     scale=vc1(v, K_IDEC))
                nc.vector.tensor_scalar(vsl(tA, v), vsl(tA, v), 1.0, None,
                                        op0=A.min)
                nc.scalar.activation(vsl(tA, v), vsl(tA, v), F.Ln)
                nc.scalar.activation(vsl(tA, v), vsl(tA, v), F.Exp,
                                     scale=vc1(v, K_ALPHA))
                nc.scalar.activation(vsl(tA, v), vsl(tA, v), F.Identity,
                                     bias=1.0, scale=vc1(v, K_N1MS))
            nc.vector.tensor_tensor(env[:], env[:], tA[:], op=A.mult)
            for v in range(NV):
                # rel -> tA_v
                nc.scalar.activation(vsl(tA, v), ts_t[:], F.Relu,
                                     bias=vc1(v, K_BREL),
                                     scale=vc1(v, K_IREL))
                nc.vector.tensor_scalar(vsl(tA, v), vsl(tA, v), 1.0, None,
                                        op0=A.min)
                nc.scalar.activation(vsl(tA, v), vsl(tA, v), F.Ln)
                nc.scalar.activation(vsl(tA, v), vsl(tA, v), F.Exp,
                                     scale=vc1(v, K_ALPHA))
                nc.scalar.activation(vsl(tA, v), vsl(tA, v), F.Identity,
                                     bias=1.0, scale=-1.0)
            nc.vector.tensor_tensor(env[:], env[:], tA[:], op=A.mult)
            nc.sync.dma_start(dview(env_d),
                              env[:].rearrange("p (v j) -> p v j", v=NV))

            # ---------- VCO mix -> dry ----------
            sq = big()
            mx1 = big()
            for v in range(NV):
                nc.scalar.activation(vsl(sq, v), vsl(sn, v), F.Tanh,
                                     scale=vc1(v, K_TANH))
                nc.scalar.activation(vsl(mx1, v), vsl(cs, v), F.Identity,
                                     bias=1.0, scale=vc1(v, K_S))
            m2 = big()
            nc.gpsimd.tensor_tensor(m2[:], sq[:], mx1[:], op=A.mult)
            dry = big()
            for v in range(NV):
                nc.vector.scalar_tensor_tensor(
                    vsl(dry, v), vsl(m2, v), vc1(v, K_DRYSC), vsl(env, v),
                    op0=A.mult, op1=A.mult)
            nc.sync.dma_start(dview(dry_d),
                              dry[:].rearrange("p (v j) -> p v j", v=NV))

            # ---------- biquad coefficients ----------
            fc = big()
            nc.sync.dma_start(fc[:].rearrange("p (v j) -> p v j", v=NV),
                              dview(fc_d))
            qm = big()
            nc.sync.dma_start(qm[:].rearrange("p (v j) -> p v j", v=NV),
                              dview(qm_d))
            w0 = big()
            nc.scalar.activation(w0[:], fc[:], F.Exp, bias=K0, scale=K1)
            ee = big()
            nc.scalar.activation(ee[:], qm[:], F.Exp, bias=BQ, scale=-Kq1)
            sn0 = big()
            nc.scalar.activation(sn0[:], w0[:], F.Sin)
            cw = big()
            nc.scalar.activation(cw[:], w0[:], F.Sin, bias=PI / 2, scale=1.0)
            al = big()
            nc.gpsimd.tensor_tensor(al[:], sn0[:], ee[:], op=A.mult)
            a0 = big()
            nc.scalar.activation(a0[:], al[:], F.Identity, bias=1.0,
                                 scale=1.0)
            ra = big()
            rs = big()
            nc.vector.reciprocal_approx_accurate(ra[:], a0[:], rs[:])
            nb1 = big()
            nc.vector.scalar_tensor_tensor(nb1[:], cw[:], 1.0, ra[:],
                                           op0=A.subtract, op1=A.mult)
            na1 = big()
            nc.vector.scalar_tensor_tensor(na1[:], cw[:], 2.0, ra[:],
                                           op0=A.mult, op1=A.mult)
            na2 = big()
            nc.vector.scalar_tensor_tensor(na2[:], al[:], 1.0, ra[:],
                                           op0=A.subtract, op1=A.mult)

            # ---------- FIR: w = b1*(0.5*(x + x2) + x1) = nb1*(-0.5*(x+x2)-x1)
            x1 = big()
            x2 = big()
            for v in range(NV):
                nc.gpsimd.tensor_copy(vsl(x1, v)[:, 1:1024],
                                      vsl(dry, v)[:, 0:1023])
                nc.vector.memset(x1[0:1, v * 1024:v * 1024 + 1], 0.0)
                nc.sync.dma_start(x1[1:P, v * 1024:v * 1024 + 1],
                                  dry[0:P - 1, (v + 1) * 1024 - 1:
                                      (v + 1) * 1024])
            for v in range(NV):
                nc.gpsimd.tensor_copy(vsl(x2, v)[:, 1:1024],
                                      vsl(x1, v)[:, 0:1023])
                nc.vector.memset(x2[0:1, v * 1024:v * 1024 + 1], 0.0)
                nc.sync.dma_start(x2[1:P, v * 1024:v * 1024 + 1],
                                  x1[0:P - 1, (v + 1) * 1024 - 1:
                                     (v + 1) * 1024])
            ssum = big()
            nc.gpsimd.tensor_tensor(ssum[:], dry[:], x2[:], op=A.add)
            uu = big()
            nc.vector.scalar_tensor_tensor(uu[:], ssum[:], -0.5, x1[:],
                                           op0=A.mult, op1=A.subtract)
            wf = big()
            nc.vector.tensor_tensor(wf[:], uu[:], nb1[:], op=A.mult)

            if DBG:
                nc.sync.dma_start(dview(dbgw_d),
                                  wf[:].rearrange("p (v j) -> p v j", v=NV))
                nc.sync.dma_start(dview(dbga_d),
                                  na1[:].rearrange("p (v j) -> p v j", v=NV))
            # ---------- blocked IIR scan, level 1 ----------
            Y = ypool.tile([P, NV * C * 3 * (L + 2)], dt, tag="Y")
            Yv = Y[:].rearrange("p (v c ch i) -> p v c ch i", v=NV, c=C,
                                ch=3)
            nc.vector.memset(Yv[:, :, :, 0, 0:2], 0.0)
            nc.vector.memset(Yv[:, :, :, 1, 0:1], 0.0)
            nc.vector.memset(Yv[:, :, :, 1, 1:2], 1.0)
            nc.vector.memset(Yv[:, :, :, 2, 0:1], 1.0)
            nc.vector.memset(Yv[:, :, :, 2, 1:2], 0.0)

            # pair-step coefficients: y[2k+1] = PA*y[2k-1] + PB*y[2k-2] + PR
            #   PA = na1'*na1 + na2',  PB = na1'*na2,  PR = na1'*w + w'
            # packed as [p, g(32), k(64), r(2)] with r = (even formula, pair)
            na1p = na1[:].rearrange("p (g k r) -> p g k r", g=NV * C, k=L // 2)
            na2p = na2[:].rearrange("p (g k r) -> p g k r", g=NV * C, k=L // 2)
            wfp = wf[:].rearrange("p (g k r) -> p g k r", g=NV * C, k=L // 2)
            pk1 = big()
            pk1v = pk1[:].rearrange("p (g k r) -> p g k r", g=NV * C,
                                    k=L // 2)
            nc.gpsimd.tensor_copy(pk1v[:, :, :, 0], na1p[:, :, :, 0])
            nc.vector.tensor_tensor(pk1v[:, :, :, 1], na1p[:, :, :, 1],
                                    na1p[:, :, :, 0], op=A.mult)
            nc.vector.tensor_tensor(pk1v[:, :, :, 1], pk1v[:, :, :, 1],
                                    na2p[:, :, :, 1], op=A.add)
            pk2 = big()
            pk2v = pk2[:].rearrange("p (g k r) -> p g k r", g=NV * C,
                                    k=L // 2)
            nc.gpsimd.tensor_copy(pk2v[:, :, :, 0], na2p[:, :, :, 0])
            nc.vector.tensor_tensor(pk2v[:, :, :, 1], na1p[:, :, :, 1],
                                    na2p[:, :, :, 0], op=A.mult)
            pkw = big()
            pkwv = pkw[:].rearrange("p (g k r) -> p g k r", g=NV * C,
                                    k=L // 2)
            nc.gpsimd.tensor_copy(pkwv[:, :, :, 0], wfp[:, :, :, 0])
            nc.vector.tensor_tensor(pkwv[:, :, :, 1], na1p[:, :, :, 1],
                                    wfp[:, :, :, 0], op=A.mult)
            nc.vector.tensor_tensor(pkwv[:, :, :, 1], pkwv[:, :, :, 1],
                                    wfp[:, :, :, 1], op=A.add)

            Yg = Y[:].rearrange("p (g ch i) -> p g ch i", g=NV * C, ch=3)
            G = NV * C

            def bcy(col):
                return Yg[:, :, :, col].unsqueeze(3).to_broadcast(
                    (P, G, 3, 2))

            def bck(pkv, k):
                return pkv[:, :, k, :].unsqueeze(2).to_broadcast(
                    (P, G, 3, 2))

            import os as _os2
            _SKIP = bool(int(_os2.environ.get("KSKIP_SCAN", "0")))
            for k in range(0 if _SKIP else L // 2):
                c0 = 2 * k
                t1 = tmpp.tile([P, G * 3 * 2], dt, tag="sc", name="t1")
                t1v = t1[:].rearrange("p (g ch r) -> p g ch r", g=G, ch=3)
                t2 = tmpp.tile([P, G * 3 * 2], dt, tag="sc", name="t2")
                t2v = t2[:].rearrange("p (g ch r) -> p g ch r", g=G, ch=3)
                nc.vector.tensor_tensor(t1v, bcy(c0 + 1), bck(pk1v, k),
                                        op=A.mult)
                nc.gpsimd.tensor_tensor(t2v, bcy(c0), bck(pk2v, k),
                                        op=A.mult)
                nc.vector.tensor_tensor(t1v, t1v, t2v, op=A.add)
                nc.vector.tensor_tensor(Yg[:, :, :, c0 + 2:c0 + 4], t1v,
                                        bck(pkwv, k), op=A.add)

            # ---------- block transfers ----------
            # T entries [128, NV, C]: h1e h2e h1e2 h2e2 ue ue2
            TmE = keep("TmE", [NV * C * 6])
            TmEv = TmE[:].rearrange("p (e v c) -> p e v c", e=6, v=NV)
            for (e, ch, col) in ((0, 1, L + 1), (1, 2, L + 1), (2, 1, L),
                                 (3, 2, L)):
                nc.vector.tensor_tensor(TmEv[:, e], Yv[:, :, :, ch, col],
                                        Yv[:, :, :, 0, col], op=A.subtract)
            nc.vector.tensor_copy(TmEv[:, 4], Yv[:, :, :, 0, L + 1])
            nc.vector.tensor_copy(TmEv[:, 5], Yv[:, :, :, 0, L])

            # ---------- intra-partition exclusive prefixes over c ----------
            # Pm00 Pm01 Pm10 Pm11 Pu0 Pu1: [128, 6, NV, C]
            PmE = keep("PmE", [6 * NV * C])
            PmEv = PmE[:].rearrange("p (e v c) -> p e v c", e=6, v=NV)
            for e in range(6):
                init = 1.0 if e in (0, 3) else 0.0
                nc.vector.memset(PmEv[:, e, :, 0:1], init)

            def compose_step(dst, dcc, S, scc, Tt, tcc):
                """dst[:,:,dcc] = T[:,:,tcc] o S[:,:,scc] (T applied second)"""
                tq = tmpp.tile([P, 2 * NV], dt, tag="c8", name="tq")
                tqv = tq[:].rearrange("p (e v) -> p e v", e=2)
                tq2 = tmpp.tile([P, 2 * NV], dt, tag="c8", name="tq2")
                tq2v = tq2[:].rearrange("p (e v) -> p e v", e=2)
                # m rows batched: (n00,n01) = T00*(S00,S01) + T01*(S10,S11)
                #                 (n10,n11) = T10*(S00,S01) + T11*(S10,S11)
                for (dr, a1, a2) in ((0, 0, 1), (2, 2, 3)):
                    t00b = Tt[:, a1, :, tcc].unsqueeze(1).to_broadcast(
                        (P, 2, NV))
                    t01b = Tt[:, a2, :, tcc].unsqueeze(1).to_broadcast(
                        (P, 2, NV))
                    nc.vector.tensor_tensor(tqv, t00b, S[:, 0:2, :, scc],
                                            op=A.mult)
                    nc.vector.tensor_tensor(tq2v, t01b, S[:, 2:4, :, scc],
                                            op=A.mult)
                    nc.vector.tensor_tensor(dst[:, dr:dr + 2, :, dcc],
                                            tqv, tq2v, op=A.add)
                # u: newu0 = T00*Su0 + T01*Su1 + Tu0 ; newu1 = T10.. + Tu1
                tq3 = tmpp.tile([P, NV], dt, tag="c4", name="tq3")
                tq4 = tmpp.tile([P, NV], dt, tag="c4", name="tq4")
                for (de, a1, a2, au) in ((4, 0, 1, 4), (5, 2, 3, 5)):
                    nc.gpsimd.tensor_tensor(tq3[:], Tt[:, a1, :, tcc],
                                            S[:, 4, :, scc], op=A.mult)
                    nc.gpsimd.tensor_tensor(tq4[:], Tt[:, a2, :, tcc],
                                            S[:, 5, :, scc], op=A.mult)
                    nc.gpsimd.tensor_tensor(tq3[:], tq3[:], tq4[:], op=A.add)
                    nc.gpsimd.tensor_tensor(dst[:, de, :, dcc], tq3[:],
                                            Tt[:, au, :, tcc], op=A.add)

            for cc in range(1, C):
                compose_step(PmEv, cc, PmEv, cc - 1, TmEv, cc - 1)

            # full-partition transfer F = T_{C-1} o P_{C-1}, packed [128,NV,6]
            Fp = keep("Fp", [NV * 6])
            FpE = Fp[:].rearrange("p (e v c) -> p e v c", e=6, v=NV, c=1)
            compose_step(FpE, 0, PmEv, C - 1, TmEv, C - 1)

            # ---------- Hillis-Steele across partitions via PE shifts -------
            # pack to [128, NV*6] with (v, e) layout for matmul rhs
            Xp = keep("Xp", [NV * 6])
            Xv = Xp[:].rearrange("p (v e) -> p v e", v=NV)
            for e in range(6):
                nc.vector.tensor_copy(Xv[:, :, e], FpE[:, e, :, 0])

            cur = Xp
            for si, s in enumerate((1, 2, 4, 8, 16, 32, 64)):
                sh = tmpp.tile([P, NV * 6], dt, tag="hs", name="sh")
                nc.vector.memset(sh[0:s, :], 0.0)
                nc.sync.dma_start(sh[s:P, :], cur[0:P - s, :])
                shv = sh[:].rearrange("p (v e) -> p v e", v=NV)
                nc.vector.memset(shv[0:s, :, 0:1], 1.0)
                nc.vector.memset(shv[0:s, :, 3:4], 1.0)
                nxt = tmpp.tile([P, NV * 6], dt, tag="hs", name="nxt")
                nxtv = nxt[:].rearrange("p (v e) -> p v e", v=NV)
                curv = cur[:].rearrange("p (v e) -> p v e", v=NV)
                tq = tmpp.tile([P, NV * 2], dt, tag="c8", name="tq")
                tqv = tq[:].rearrange("p (v e) -> p v e", v=NV)
                tq2 = tmpp.tile([P, NV * 2], dt, tag="c8", name="tq2")
                tq2v = tq2[:].rearrange("p (v e) -> p v e", v=NV)
                for (dr, a1, a2) in ((0, 0, 1), (2, 2, 3)):
                    c0b = curv[:, :, a1].unsqueeze(2).to_broadcast((P, NV, 2))
                    c1b = curv[:, :, a2].unsqueeze(2).to_broadcast((P, NV, 2))
                    nc.vector.tensor_tensor(tqv, c0b, shv[:, :, 0:2],
                                            op=A.mult)
                    nc.vector.tensor_tensor(tq2v, c1b, shv[:, :, 2:4],
                                            op=A.mult)
                    nc.vector.tensor_tensor(nxtv[:, :, dr:dr + 2], tqv, tq2v,
                                            op=A.add)
                tq3 = tmpp.tile([P, NV], dt, tag="c4", name="tq3")
                tq4 = tmpp.tile([P, NV], dt, tag="c4", name="tq4")
                for (de, a1, a2) in ((4, 0, 1), (5, 2, 3)):
                    nc.gpsimd.tensor_tensor(tq3[:], curv[:, :, a1],
                                            shv[:, :, 4], op=A.mult)
                    nc.gpsimd.tensor_tensor(tq4[:], curv[:, :, a2],
                                            shv[:, :, 5], op=A.mult)
                    nc.gpsimd.tensor_tensor(tq3[:], tq3[:], tq4[:], op=A.add)
                    nc.gpsimd.tensor_tensor(nxtv[:, :, de], tq3[:],
                                            curv[:, :, de], op=A.add)
                cur = nxt

            # exclusive shift by 1: s0 = shift1(cur); rows<1 -> zeros
            s0 = keep("s0", [NV * 6])
            nc.vector.memset(s0[0:1, :], 0.0)
            nc.sync.dma_start(s0[1:P, :], cur[0:P - 1, :])
            s0v = s0[:].rearrange("p (v e) -> p v e", v=NV)

            # ---------- per-block incoming states: ab = Pm@s0u + Pu --------
            abt = keep("abt", [3 * NV * C])
            abv = abt[:].rearrange("p (e v c) -> p e v c", e=3, v=NV)
            s0u0b = s0v[:, :, 4].unsqueeze(2).to_broadcast((P, NV, C))
            s0u1b = s0v[:, :, 5].unsqueeze(2).to_broadcast((P, NV, C))
            tqa = tmpp.tile([P, NV * C], dt, tag="c32", name="tqa")
            tqav = tqa[:].rearrange("p (v c) -> p v c", v=NV)
            tqb = tmpp.tile([P, NV * C], dt, tag="c32", name="tqb")
            tqbv = tqb[:].rearrange("p (v c) -> p v c", v=NV)
            for (de, a1, a2, au) in ((0, 0, 1, 4), (1, 2, 3, 5)):
                nc.vector.tensor_tensor(tqav, PmEv[:, a1], s0u0b, op=A.mult)
                nc.vector.tensor_tensor(tqbv, PmEv[:, a2], s0u1b, op=A.mult)
                nc.vector.tensor_tensor(tqav, tqav, tqbv, op=A.add)
                nc.vector.tensor_tensor(abv[:, de], tqav, PmEv[:, au],
                                        op=A.add)
            # gamma = 1 - a - b
            nc.vector.tensor_tensor(tqav, abv[:, 0], abv[:, 1], op=A.add)
            nc.vector.tensor_scalar(abv[:, 2], tqav, -1.0, 1.0,
                                    op0=A.mult, op1=A.add)

            # ---------- correction + wet ----------
            yout = big()
            youtv = yout[:].rearrange("p (v c i) -> p v c i", v=NV, c=C)
            a_b = abv[:, 0].unsqueeze(3).to_broadcast((P, NV, C, L))
            b_b = abv[:, 1].unsqueeze(3).to_broadcast((P, NV, C, L))
            g_b = abv[:, 2].unsqueeze(3).to_broadcast((P, NV, C, L))
            ct1 = big()
            ct1v = ct1[:].rearrange("p (v c i) -> p v c i", v=NV, c=C)
            ct2 = big()
            ct2v = ct2[:].rearrange("p (v c i) -> p v c i", v=NV, c=C)
            nc.vector.tensor_tensor(ct1v, Yv[:, :, :, 1, 2:L + 2], a_b,
                                    op=A.mult)
            nc.vector.tensor_tensor(ct2v, Yv[:, :, :, 2, 2:L + 2], b_b,
                                    op=A.mult)
            nc.vector.tensor_tensor(ct1v, ct1v, ct2v, op=A.add)
            nc.vector.tensor_tensor(ct2v, Yv[:, :, :, 0, 2:L + 2], g_b,
                                    op=A.mult)
            nc.vector.tensor_tensor(youtv, ct1v, ct2v, op=A.add)
            wet = big()
            for v in range(NV):
                nc.scalar.activation(vsl(wet, v), vsl(yout, v), F.Tanh,
                                     scale=vc1(v, K_DIST))
            nc.sync.dma_start(dview(wet_d),
                              wet[:].rearrange("p (v j) -> p v j", v=NV))

    nc.compile()
    return nc


def _host_prep(inputs):
    """Per-core input maps. Heavy phase cumsum matches eager CPU-XLA jax."""
    import jax
    import jax.numpy as jnp

    cpu = jax.devices("cpu")[0]
    f0 = np.asarray(inputs["f0_hz"], f32)
    s = np.asarray(inputs["osc_shape"], f32)
    note_on = np.asarray(inputs["note_on_duration"], f32)
    fcm = np.asarray(inputs["fc_mod_sig"], f32)
    qmm = np.asarray(inputs["q_mod_sig"], f32)
    dist = np.asarray(inputs["dist_gain"], f32)
    att = np.asarray(inputs["attack"], f32)
    dec = np.asarray(inputs["decay"], f32)
    sus = np.asarray(inputs["sustain"], f32)
    rel = np.asarray(inputs["release"], f32)
    alpha = np.asarray(inputs["alpha"], f32)
    Bn = f0.shape[0]

    inc = (f0 / f32(SR)).astype(f32)
    incb = jax.device_put(
        np.broadcast_to(inc[:, None], (Bn, T)).astype(f32), cpu)
    cx = np.asarray(jax.jit(lambda a: jnp.cumsum(a, axis=1))(incb))
    p32 = (f32(2 * np.pi) * cx).astype(f32)
    r64 = np.mod(p32.astype(np.float64), 2 * np.pi)
    r64 = np.where(r64 > np.pi, r64 - 2 * np.pi, r64)
    rph = r64.astype(f32)

    partials = (f32(12000.0) / (f0 * np.log10(f0))).astype(f32)
    vc_cols = np.zeros((Bn, NK), f32)
    vc_cols[:, K_TANH] = f32(np.pi) * partials / f32(2.0)
    vc_cols[:, K_S] = s
    vc_cols[:, K_DRYSC] = f32(0.5) * (f32(1.0) - s / f32(2.0))
    vc_cols[:, K_IATT] = f32(1.0) / att
    vc_cols[:, K_ALPHA] = alpha
    vc_cols[:, K_IDEC] = f32(1.0) / dec
    vc_cols[:, K_BDEC] = -(att / dec)
    vc_cols[:, K_N1MS] = -(f32(1.0) - sus)
    vc_cols[:, K_IREL] = f32(1.0) / rel
    vc_cols[:, K_BREL] = -(note_on / rel)
    vc_cols[:, K_DIST] = dist

    tsec = (np.arange(T, dtype=f32) / f32(SR)).astype(f32).reshape(P, 1024)

    shm = np.zeros((P, 8 * P), f32)
    for si, sh in enumerate((1, 2, 4, 8, 16, 32, 64)):
        m = np.zeros((P, P), f32)
        m[np.arange(P - sh), np.arange(sh, P)] = 1.0
        shm[:, si * P:(si + 1) * P] = m

    in_maps = []
    for core in range(NCORES):
        vs = slice(core * NV, (core + 1) * NV)
        vc = np.ascontiguousarray(
            np.broadcast_to(vc_cols[vs].reshape(1, NV * NK), (P, NV * NK))
        ).astype(f32)
        in_maps.append(dict(
            rph=np.ascontiguousarray(rph[vs]),
            fcm=np.ascontiguousarray(fcm[vs]),
            qmm=np.ascontiguousarray(qmm[vs]),
            tsec=tsec, vc=vc, shm=shm,
        ))
    return in_maps


def run(inputs, trace=False):
    from concourse.bass_utils import run_bass_kernel_spmd

    if "nc" not in _CACHE:
        _CACHE["nc"] = _build_module()
    nc = _CACHE["nc"]
    in_maps = _host_prep(inputs)
    res = run_bass_kernel_spmd(nc, in_maps, core_ids=list(range(NCORES)),
                               trace=trace)
    dry = np.concatenate([r["dry"] for r in res.results], axis=0)
    wet = np.concatenate([r["wet"] for r in res.results], axis=0)
    env = np.concatenate([r["env"] for r in res.results], axis=0)
    return (dry, wet, env), res


def kernel(**inputs):
    out, _ = run(inputs, trace=False)
    return out


# revision 30
# speedup vs baseline: 1.0619x; 1.0045x over previous
"""AcidSynth Trainium2 kernel: 32 voices sharded 4-per-core across 8 NeuronCores.

Pipeline per core (4 voices, T=131072):
  - VCO sin/cos from host-reduced phase (ACT Sin), tanh square-saw morph
  - ADSR envelope (ACT Ln/Exp power curves)
  - RBJ time-varying lowpass biquad coefficients (ACT Exp/Sin + DVE)
  - IIR scan: block-parallel 3-channel method (particular + 2 homogeneous
    runs per block; linear correction), in-block steps vectorized over
    128 partitions x (4 voices x 8 chunks); carry chain composed
    intra-partition, then across partitions via PE shift-matmul
    Hillis-Steele scan
  - tanh distortion
"""
import numpy as np

f32 = np.float32
SR = 48000.0
T = 131072
P = 128
NV = 4            # voices per core
C = 8             # chunks per partition per voice
L = 1024 // C     # in-block scan length (128)
NCORES = 8

MIN_W = 2.0 * np.pi * 20.0 / SR
MAX_W = 2.0 * np.pi * 8000.0 / SR
MIN_Q = 0.7071
MAX_Q = 8.0

# voice-const column indices (vc tensor [128, NV*NK], col = v*NK + k)
K_TANH, K_S, K_DRYSC, K_IATT, K_ALPHA, K_IDEC, K_BDEC, K_N1MS, K_IREL, \
    K_BREL, K_DIST = range(11)
NK = 11

_CACHE = {}


def _build_module():
    import concourse.bacc as bacc
    import concourse.mybir as mybir
    from concourse import tile

    dt = mybir.dt.float32
    A = mybir.AluOpType
    F = mybir.ActivationFunctionType

    nc = bacc.Bacc()
    rph_d = nc.dram_tensor("rph", [NV, T], dt, kind="ExternalInput")
    fc_d = nc.dram_tensor("fcm", [NV, T], dt, kind="ExternalInput")
    qm_d = nc.dram_tensor("qmm", [NV, T], dt, kind="ExternalInput")
    ts_d = nc.dram_tensor("tsec", [P, 1024], dt, kind="ExternalInput")
    vc_d = nc.dram_tensor("vc", [P, NV * NK], dt, kind="ExternalInput")
    sh_d = nc.dram_tensor("shm", [P, 8 * P], dt, kind="ExternalInput")
    import os as _os
    DBG = bool(int(_os.environ.get("KDEBUG", "0")))
    dry_d = nc.dram_tensor("dry", [NV, T], dt, kind="ExternalOutput")
    wet_d = nc.dram_tensor("wet", [NV, T], dt, kind="ExternalOutput")
    env_d = nc.dram_tensor("env", [NV, T], dt, kind="ExternalOutput")
    if DBG:
        dbgw_d = nc.dram_tensor("dbgw", [NV, T], dt, kind="ExternalOutput")
        dbga_d = nc.dram_tensor("dbga", [NV, T], dt, kind="ExternalOutput")
        dbgy_d = nc.dram_tensor("dbgy", [P, NV * C * 3 * (L + 2)], dt,
                                kind="ExternalOutput")
        dbgab_d = nc.dram_tensor("dbgab", [P, 3 * NV * C], dt,
                                 kind="ExternalOutput")

    def dview(d):  # DRAM [NV, T] as [128, NV, 1024]
        return d[:, :].rearrange("v (p j) -> p v j", p=P)

    K1 = float(np.log(MAX_W / MIN_W))
    K0 = float(np.log(MIN_W))
    Kq1 = float(np.log(MAX_Q / MIN_Q))
    Kq0 = float(np.log(MIN_Q))
    PI = float(np.pi)
    BQ = -(Kq0 + float(np.log(2.0)))

    # ACT float biases need pre-registered const APs
    for val in (PI / 2, K0, BQ):
        t = nc.alloc_sbuf_tensor(f"constap-{val}", [128, 1], dt)
        nc.gpsimd.memset(t.ap(), val)
        nc.const_aps.aps[(dt, float(val))] = t.ap()
    nc.all_engine_barrier()

    with tile.TileContext(nc) as tc:
        with (
            tc.tile_pool(name="big", bufs=8) as bigp,
            tc.tile_pool(name="ybuf", bufs=1) as ypool,
            tc.tile_pool(name="keep", bufs=1) as keepp,
            tc.tile_pool(name="tmps", bufs=16) as tmpp,
            tc.tile_pool(name="psum", bufs=2, space="PSUM") as psp,
        ):
            def big():
                return bigp.tile([P, NV * 1024], dt, tag="big", name="bigt")

            def keep(name, shape):
                return keepp.tile([P, *shape] if shape else [P, 1], dt,
                                  tag=name, name=name)

            def vsl(t, v):
                return t[:, v * 1024:(v + 1) * 1024]

            def vc1(v, k):
                return vc_t[:, v * NK + k: v * NK + k + 1]

            # ---------- small persistent loads ----------
            ts_t = keepp.tile([P, 1024], dt, tag="tsec")
            nc.sync.dma_start(ts_t[:], ts_d[:, :])
            vc_t = keepp.tile([P, NV * NK], dt, tag="vc")
            nc.sync.dma_start(vc_t[:], vc_d[:, :])
            sh_t = keepp.tile([P, 8 * P], dt, tag="shm")
            nc.sync.dma_start(sh_t[:], sh_d[:, :])

            # ---------- phase -> sin, cos ----------
            rph = big()
            for v in range(NV):
                nc.sync.dma_start(
                    vsl(rph, v)[:].rearrange("p (o j) -> p o j", o=1),
                    rph_d[v:v + 1, :].rearrange("o (p j) -> p o j", p=P))
            sn = big()
            nc.scalar.activation(sn[:], rph[:], F.Sin)
            ab = big()
            nc.scalar.activation(ab[:], rph[:], F.Abs)
            cs = big()
            nc.scalar.activation(cs[:], ab[:], F.Sin, bias=PI / 2, scale=-1.0)

            # ---------- ADSR envelope ----------
            env = big()
            tA = ab  # reuse slot logically: Tile tracks deps; use fresh tiles
            tA = big()
            for v in range(NV):
                # att -> env_v
                nc.vector.tensor_scalar(vsl(env, v)[:, 0:1024], ts_t[:],
                                        vc1(v, K_IATT), 1.0, op0=A.mult,
                                        op1=A.min)
                nc.scalar.activation(vsl(env, v), vsl(env, v), F.Ln)
                nc.scalar.activation(vsl(env, v), vsl(env, v), F.Exp,
                                     scale=vc1(v, K_ALPHA))
                # dec -> tA_v
                nc.scalar.activation(vsl(tA, v), ts_t[:], F.Relu,
                                     bias=vc1(v, K_BDEC),
                                     scale=vc1(v, K_IDEC))
                nc.vector.tensor_scalar(vsl(tA, v), vsl(tA, v), 1.0, None,
                                        op0=A.min)
                nc.scalar.activation(vsl(tA, v), vsl(tA, v), F.Ln)
                nc.scalar.activation(vsl(tA, v), vsl(tA, v), F.Exp,
                                     scale=vc1(v, K_ALPHA))
                nc.scalar.activation(vsl(tA, v), vsl(tA, v), F.Identity,
                                     bias=1.0, scale=vc1(v, K_N1MS))
            nc.vector.tensor_tensor(env[:], env[:], tA[:], op=A.mult)
            for v in range(NV):
                # rel -> tA_v
                nc.scalar.activation(vsl(tA, v), ts_t[:], F.Relu,
                                     bias=vc1(v, K_BREL),
                                     scale=vc1(v, K_IREL))
                nc.vector.tensor_scalar(vsl(tA, v), vsl(tA, v), 1.0, None,
                                        op0=A.min)
                nc.scalar.activation(vsl(tA, v), vsl(tA, v), F.Ln)
                nc.scalar.activation(vsl(tA, v), vsl(tA, v), F.Exp,
                                     scale=vc1(v, K_ALPHA))
                nc.scalar.activation(vsl(tA, v), vsl(tA, v), F.Identity,
                                     bias=1.0, scale=-1.0)
            nc.vector.tensor_tensor(env[:], env[:], tA[:], op=A.mult)
            for v in range(NV):
                nc.sync.dma_start(
                    env_d[v:v + 1, :].rearrange("o (p j) -> p o j", p=P),
                    vsl(env, v)[:].rearrange("p (o j) -> p o j", o=1))

            # ---------- VCO mix -> dry ----------
            sq = big()
            mx1 = big()
            for v in range(NV):
                nc.scalar.activation(vsl(sq, v), vsl(sn, v), F.Tanh,
                                     scale=vc1(v, K_TANH))
                nc.scalar.activation(vsl(mx1, v), vsl(cs, v), F.Identity,
                                     bias=1.0, scale=vc1(v, K_S))
            m2 = big()
            nc.gpsimd.tensor_tensor(m2[:], sq[:], mx1[:], op=A.mult)
            dry = big()
            for v in range(NV):
                nc.vector.scalar_tensor_tensor(
                    vsl(dry, v), vsl(m2, v), vc1(v, K_DRYSC), vsl(env, v),
                    op0=A.mult, op1=A.mult)
            for v in range(NV):
                nc.sync.dma_start(
                    dry_d[v:v + 1, :].rearrange("o (p j) -> p o j", p=P),
                    vsl(dry, v)[:].rearrange("p (o j) -> p o j", o=1))

            # ---------- biquad coefficients ----------
            fc = big()
            for v in range(NV):
                nc.sync.dma_start(
                    vsl(fc, v)[:].rearrange("p (o j) -> p o j", o=1),
                    fc_d[v:v + 1, :].rearrange("o (p j) -> p o j", p=P))
            qm = big()
            for v in range(NV):
                nc.sync.dma_start(
                    vsl(qm, v)[:].rearrange("p (o j) -> p o j", o=1),
                    qm_d[v:v + 1, :].rearrange("o (p j) -> p o j", p=P))
            w0 = big()
            nc.scalar.activation(w0[:], fc[:], F.Exp, bias=K0, scale=K1)
            ee = big()
            nc.scalar.activation(ee[:], qm[:], F.Exp, bias=BQ, scale=-Kq1)
            sn0 = big()
            nc.scalar.activation(sn0[:], w0[:], F.Sin)
            cw = big()
            nc.scalar.activation(cw[:], w0[:], F.Sin, bias=PI / 2, scale=1.0)
            al = big()
            nc.gpsimd.tensor_tensor(al[:], sn0[:], ee[:], op=A.mult)
            a0 = big()
            nc.scalar.activation(a0[:], al[:], F.Identity, bias=1.0,
                                 scale=1.0)
            ra = big()
            rs = big()
            nc.vector.reciprocal_approx_accurate(ra[:], a0[:], rs[:])
            nb1 = big()
            nc.vector.scalar_tensor_tensor(nb1[:], cw[:], 1.0, ra[:],
                                           op0=A.subtract, op1=A.mult)
            na1 = big()
            nc.vector.scalar_tensor_tensor(na1[:], cw[:], 2.0, ra[:],
                                           op0=A.mult, op1=A.mult)
            na2 = big()
            nc.vector.scalar_tensor_tensor(na2[:], al[:], 1.0, ra[:],
                                           op0=A.subtract, op1=A.mult)

            # ---------- FIR: w = b1*(0.5*(x + x2) + x1) = nb1*(-0.5*(x+x2)-x1)
            x1 = big()
            x2 = big()
            for v in range(NV):
                nc.gpsimd.tensor_copy(vsl(x1, v)[:, 1:1024],
                                      vsl(dry, v)[:, 0:1023])
                nc.vector.memset(x1[0:1, v * 1024:v * 1024 + 1], 0.0)
                nc.sync.dma_start(x1[1:P, v * 1024:v * 1024 + 1],
                                  dry[0:P - 1, (v + 1) * 1024 - 1:
                                      (v + 1) * 1024])
            for v in range(NV):
                nc.gpsimd.tensor_copy(vsl(x2, v)[:, 1:1024],
                                      vsl(x1, v)[:, 0:1023])
                nc.vector.memset(x2[0:1, v * 1024:v * 1024 + 1], 0.0)
                nc.sync.dma_start(x2[1:P, v * 1024:v * 1024 + 1],
                                  x1[0:P - 1, (v + 1) * 1024 - 1:
                                     (v + 1) * 1024])
            ssum = big()
            nc.gpsimd.tensor_tensor(ssum[:], dry[:], x2[:], op=A.add)
            uu = big()
            nc.vector.scalar_tensor_tensor(uu[:], ssum[:], -0.5, x1[:],
                                           op0=A.mult, op1=A.subtract)
            wf = big()
            nc.vector.tensor_tensor(wf[:], uu[:], nb1[:], op=A.mult)

            if DBG:
                nc.sync.dma_start(dview(dbgw_d),
                                  wf[:].rearrange("p (v j) -> p v j", v=NV))
                nc.sync.dma_start(dview(dbga_d),
                                  na1[:].rearrange("p (v j) -> p v j", v=NV))
            # ---------- blocked IIR scan, level 1 ----------
            Y = ypool.tile([P, NV * C * 3 * (L + 2)], dt, tag="Y")
            Yv = Y[:].rearrange("p (v c ch i) -> p v c ch i", v=NV, c=C,
                                ch=3)
            nc.vector.memset(Yv[:, :, :, 0, 0:2], 0.0)
            nc.vector.memset(Yv[:, :, :, 1, 0:1], 0.0)
            nc.vector.memset(Yv[:, :, :, 1, 1:2], 1.0)
            nc.vector.memset(Yv[:, :, :, 2, 0:1], 1.0)
            nc.vector.memset(Yv[:, :, :, 2, 1:2], 0.0)

            # pair-step coefficients: y[2k+1] = PA*y[2k-1] + PB*y[2k-2] + PR
            #   PA = na1'*na1 + na2',  PB = na1'*na2,  PR = na1'*w + w'
            # packed as [p, g(32), k(64), r(2)] with r = (even formula, pair)
            na1p = na1[:].rearrange("p (g k r) -> p g k r", g=NV * C, k=L // 2)
            na2p = na2[:].rearrange("p (g k r) -> p g k r", g=NV * C, k=L // 2)
            wfp = wf[:].rearrange("p (g k r) -> p g k r", g=NV * C, k=L // 2)
            pk1 = big()
            pk1v = pk1[:].rearrange("p (g k r) -> p g k r", g=NV * C,
                                    k=L // 2)
            nc.gpsimd.tensor_copy(pk1v[:, :, :, 0], na1p[:, :, :, 0])
            nc.vector.tensor_tensor(pk1v[:, :, :, 1], na1p[:, :, :, 1],
                                    na1p[:, :, :, 0], op=A.mult)
            nc.vector.tensor_tensor(pk1v[:, :, :, 1], pk1v[:, :, :, 1],
                                    na2p[:, :, :, 1], op=A.add)
            pk2 = big()
            pk2v = pk2[:].rearrange("p (g k r) -> p g k r", g=NV * C,
                                    k=L // 2)
            nc.gpsimd.tensor_copy(pk2v[:, :, :, 0], na2p[:, :, :, 0])
            nc.vector.tensor_tensor(pk2v[:, :, :, 1], na1p[:, :, :, 1],
                                    na2p[:, :, :, 0], op=A.mult)
            pkw = big()
            pkwv = pkw[:].rearrange("p (g k r) -> p g k r", g=NV * C,
                                    k=L // 2)
            nc.gpsimd.tensor_copy(pkwv[:, :, :, 0], wfp[:, :, :, 0])
            nc.vector.tensor_tensor(pkwv[:, :, :, 1], na1p[:, :, :, 1],
                                    wfp[:, :, :, 0], op=A.mult)
            nc.vector.tensor_tensor(pkwv[:, :, :, 1], pkwv[:, :, :, 1],
                                    wfp[:, :, :, 1], op=A.add)

            Yg = Y[:].rearrange("p (g ch i) -> p g ch i", g=NV * C, ch=3)
            G = NV * C

            def bcy(col):
                return Yg[:, :, :, col].unsqueeze(3).to_broadcast(
                    (P, G, 3, 2))

            def bck(pkv, k):
                return pkv[:, :, k, :].unsqueeze(2).to_broadcast(
                    (P, G, 3, 2))

            import os as _os2
            _SKIP = bool(int(_os2.environ.get("KSKIP_SCAN", "0")))
            for k in range(0 if _SKIP else L // 2):
                c0 = 2 * k
                t1 = tmpp.tile([P, G * 3 * 2], dt, tag="sc", name="t1")
                t1v = t1[:].rearrange("p (g ch r) -> p g ch r", g=G, ch=3)
                t2 = tmpp.tile([P, G * 3 * 2], dt, tag="sc", name="t2")
                t2v = t2[:].rearrange("p (g ch r) -> p g ch r", g=G, ch=3)
                nc.vector.tensor_tensor(t1v, bcy(c0 + 1), bck(pk1v, k),
                                        op=A.mult)
                nc.gpsimd.tensor_tensor(t2v, bcy(c0), bck(pk2v, k),
                                        op=A.mult)
                nc.vector.tensor_tensor(t1v, t1v, t2v, op=A.add)
                nc.vector.tensor_tensor(Yg[:, :, :, c0 + 2:c0 + 4], t1v,
                                        bck(pkwv, k), op=A.add)

            # ---------- block transfers ----------
            # T entries [128, NV, C]: h1e h2e h1e2 h2e2 ue ue2
            TmE = keep("TmE", [NV * C * 6])
            TmEv = TmE[:].rearrange("p (e v c) -> p e v c", e=6, v=NV)
            for (e, ch, col) in ((0, 1, L + 1), (1, 2, L + 1), (2, 1, L),
                                 (3, 2, L)):
                nc.vector.tensor_tensor(TmEv[:, e], Yv[:, :, :, ch, col],
                                        Yv[:, :, :, 0, col], op=A.subtract)
            nc.vector.tensor_copy(TmEv[:, 4], Yv[:, :, :, 0, L + 1])
            nc.vector.tensor_copy(TmEv[:, 5], Yv[:, :, :, 0, L])

            # ---------- intra-partition exclusive prefixes over c ----------
            # Pm00 Pm01 Pm10 Pm11 Pu0 Pu1: [128, 6, NV, C]
            PmE = keep("PmE", [6 * NV * C])
            PmEv = PmE[:].rearrange("p (e v c) -> p e v c", e=6, v=NV)
            for e in range(6):
                init = 1.0 if e in (0, 3) else 0.0
                nc.vector.memset(PmEv[:, e, :, 0:1], init)

            def compose_step(dst, dcc, S, scc, Tt, tcc):
                """dst[:,:,dcc] = T[:,:,tcc] o S[:,:,scc] (T applied second)"""
                tq = tmpp.tile([P, 2 * NV], dt, tag="c8", name="tq")
                tqv = tq[:].rearrange("p (e v) -> p e v", e=2)
                tq2 = tmpp.tile([P, 2 * NV], dt, tag="c8", name="tq2")
                tq2v = tq2[:].rearrange("p (e v) -> p e v", e=2)
                # m rows batched: (n00,n01) = T00*(S00,S01) + T01*(S10,S11)
                #                 (n10,n11) = T10*(S00,S01) + T11*(S10,S11)
                for (dr, a1, a2) in ((0, 0, 1), (2, 2, 3)):
                    t00b = Tt[:, a1, :, tcc].unsqueeze(1).to_broadcast(
                        (P, 2, NV))
                    t01b = Tt[:, a2, :, tcc].unsqueeze(1).to_broadcast(
                        (P, 2, NV))
                    nc.vector.tensor_tensor(tqv, t00b, S[:, 0:2, :, scc],
                                            op=A.mult)
                    nc.vector.tensor_tensor(tq2v, t01b, S[:, 2:4, :, scc],
                                            op=A.mult)
                    nc.vector.tensor_tensor(dst[:, dr:dr + 2, :, dcc],
                                            tqv, tq2v, op=A.add)
                # u: newu0 = T00*Su0 + T01*Su1 + Tu0 ; newu1 = T10.. + Tu1
                tq3 = tmpp.tile([P, NV], dt, tag="c4", name="tq3")
                tq4 = tmpp.tile([P, NV], dt, tag="c4", name="tq4")
                for (de, a1, a2, au) in ((4, 0, 1, 4), (5, 2, 3, 5)):
                    nc.gpsimd.tensor_tensor(tq3[:], Tt[:, a1, :, tcc],
                                            S[:, 4, :, scc], op=A.mult)
                    nc.gpsimd.tensor_tensor(tq4[:], Tt[:, a2, :, tcc],
                                            S[:, 5, :, scc], op=A.mult)
                    nc.gpsimd.tensor_tensor(tq3[:], tq3[:], tq4[:], op=A.add)
                    nc.gpsimd.tensor_tensor(dst[:, de, :, dcc], tq3[:],
                                            Tt[:, au, :, tcc], op=A.add)

            for cc in range(1, C):
                compose_step(PmEv, cc, PmEv, cc - 1, TmEv, cc - 1)

            # full-partition transfer F = T_{C-1} o P_{C-1}, packed [128,NV,6]
            Fp = keep("Fp", [NV * 6])
            FpE = Fp[:].rearrange("p (e v c) -> p e v c", e=6, v=NV, c=1)
            compose_step(FpE, 0, PmEv, C - 1, TmEv, C - 1)

            # ---------- Hillis-Steele across partitions via PE shifts -------
            # pack to [128, NV*6] with (v, e) layout for matmul rhs
            Xp = keep("Xp", [NV * 6])
            Xv = Xp[:].rearrange("p (v e) -> p v e", v=NV)
            for e in range(6):
                nc.vector.tensor_copy(Xv[:, :, e], FpE[:, e, :, 0])

            cur = Xp
            for si, s in enumerate((1, 2, 4, 8, 16, 32, 64)):
                sh = tmpp.tile([P, NV * 6], dt, tag="hs", name="sh")
                nc.vector.memset(sh[0:s, :], 0.0)
                nc.sync.dma_start(sh[s:P, :], cur[0:P - s, :])
                shv = sh[:].rearrange("p (v e) -> p v e", v=NV)
                nc.vector.memset(shv[0:s, :, 0:1], 1.0)
                nc.vector.memset(shv[0:s, :, 3:4], 1.0)
                nxt = tmpp.tile([P, NV * 6], dt, tag="hs", name="nxt")
                nxtv = nxt[:].rearrange("p (v e) -> p v e", v=NV)
                curv = cur[:].rearrange("p (v e) -> p v e", v=NV)
                tq = tmpp.tile([P, NV * 2], dt, tag="c8", name="tq")
                tqv = tq[:].rearrange("p (v e) -> p v e", v=NV)
                tq2 = tmpp.tile([P, NV * 2], dt, tag="c8", name="tq2")
                tq2v = tq2[:].rearrange("p (v e) -> p v e", v=NV)
                for (dr, a1, a2) in ((0, 0, 1), (2, 2, 3)):
                    c0b = curv[:, :, a1].unsqueeze(2).to_broadcast((P, NV, 2))
                    c1b = curv[:, :, a2].unsqueeze(2).to_broadcast((P, NV, 2))
                    nc.vector.tensor_tensor(tqv, c0b, shv[:, :, 0:2],
                                            op=A.mult)
                    nc.vector.tensor_tensor(tq2v, c1b, shv[:, :, 2:4],
                                            op=A.mult)
                    nc.vector.tensor_tensor(nxtv[:, :, dr:dr + 2], tqv, tq2v,
                                            op=A.add)
                tq3 = tmpp.tile([P, NV], dt, tag="c4", name="tq3")
                tq4 = tmpp.tile([P, NV], dt, tag="c4", name="tq4")
                for (de, a1, a2) in ((4, 0, 1), (5, 2, 3)):
                    nc.gpsimd.tensor_tensor(tq3[:], curv[:, :, a1],
                                            shv[:, :, 4], op=A.mult)
                    nc.gpsimd.tensor_tensor(tq4[:], curv[:, :, a2],
                                            shv[:, :, 5], op=A.mult)
                    nc.gpsimd.tensor_tensor(tq3[:], tq3[:], tq4[:], op=A.add)
                    nc.gpsimd.tensor_tensor(nxtv[:, :, de], tq3[:],
                                            curv[:, :, de], op=A.add)
                cur = nxt

            # exclusive shift by 1: s0 = shift1(cur); rows<1 -> zeros
            s0 = keep("s0", [NV * 6])
            nc.vector.memset(s0[0:1, :], 0.0)
            nc.sync.dma_start(s0[1:P, :], cur[0:P - 1, :])
            s0v = s0[:].rearrange("p (v e) -> p v e", v=NV)

            # ---------- per-block incoming states: ab = Pm@s0u + Pu --------
            abt = keep("abt", [3 * NV * C])
            abv = abt[:].rearrange("p (e v c) -> p e v c", e=3, v=NV)
            s0u0b = s0v[:, :, 4].unsqueeze(2).to_broadcast((P, NV, C))
            s0u1b = s0v[:, :, 5].unsqueeze(2).to_broadcast((P, NV, C))
            tqa = tmpp.tile([P, NV * C], dt, tag="c32", name="tqa")
            tqav = tqa[:].rearrange("p (v c) -> p v c", v=NV)
            tqb = tmpp.tile([P, NV * C], dt, tag="c32", name="tqb")
            tqbv = tqb[:].rearrange("p (v c) -> p v c", v=NV)
            for (de, a1, a2, au) in ((0, 0, 1, 4), (1, 2, 3, 5)):
                nc.vector.tensor_tensor(tqav, PmEv[:, a1], s0u0b, op=A.mult)
                nc.vector.tensor_tensor(tqbv, PmEv[:, a2], s0u1b, op=A.mult)
                nc.vector.tensor_tensor(tqav, tqav, tqbv, op=A.add)
                nc.vector.tensor_tensor(abv[:, de], tqav, PmEv[:, au],
                                        op=A.add)
            # gamma = 1 - a - b
            nc.vector.tensor_tensor(tqav, abv[:, 0], abv[:, 1], op=A.add)
            nc.vector.tensor_scalar(abv[:, 2], tqav, -1.0, 1.0,
                                    op0=A.mult, op1=A.add)

            # ---------- correction + wet ----------
            yout = big()
            youtv = yout[:].rearrange("p (v c i) -> p v c i", v=NV, c=C)
            a_b = abv[:, 0].unsqueeze(3).to_broadcast((P, NV, C, L))
            b_b = abv[:, 1].unsqueeze(3).to_broadcast((P, NV, C, L))
            g_b = abv[:, 2].unsqueeze(3).to_broadcast((P, NV, C, L))
            ct1 = big()
            ct1v = ct1[:].rearrange("p (v c i) -> p v c i", v=NV, c=C)
            ct2 = big()
            ct2v = ct2[:].rearrange("p (v c i) -> p v c i", v=NV, c=C)
            nc.vector.tensor_tensor(ct1v, Yv[:, :, :, 1, 2:L + 2], a_b,
                                    op=A.mult)
            nc.vector.tensor_tensor(ct2v, Yv[:, :, :, 2, 2:L + 2], b_b,
                                    op=A.mult)
            nc.vector.tensor_tensor(ct1v, ct1v, ct2v, op=A.add)
            nc.vector.tensor_tensor(ct2v, Yv[:, :, :, 0, 2:L + 2], g_b,
                                    op=A.mult)
            nc.vector.tensor_tensor(youtv, ct1v, ct2v, op=A.add)
            wet = big()
            for v in range(NV):
                nc.scalar.activation(vsl(wet, v), vsl(yout, v), F.Tanh,
                                     scale=vc1(v, K_DIST))
            for v in range(NV):
                nc.sync.dma_start(
                    wet_d[v:v + 1, :].rearrange("o (p j) -> p o j", p=P),
                    vsl(wet, v)[:].rearrange("p (o j) -> p o j", o=1))

    nc.compile()
    return nc


def _host_prep(inputs):
    """Per-core input maps. Heavy phase cumsum matches eager CPU-XLA jax."""
    import jax
    import jax.numpy as jnp

    cpu = jax.devices("cpu")[0]
    f0 = np.asarray(inputs["f0_hz"], f32)
    s = np.asarray(inputs["osc_shape"], f32)
    note_on = np.asarray(inputs["note_on_duration"], f32)
    fcm = np.asarray(inputs["fc_mod_sig"], f32)
    qmm = np.asarray(inputs["q_mod_sig"], f32)
    dist = np.asarray(inputs["dist_gain"], f32)
    att = np.asarray(inputs["attack"], f32)
    dec = np.asarray(inputs["decay"], f32)
    sus = np.asarray(inputs["sustain"], f32)
    rel = np.asarray(inputs["release"], f32)
    alpha = np.asarray(inputs["alpha"], f32)
    Bn = f0.shape[0]

    inc = (f0 / f32(SR)).astype(f32)
    incb = jax.device_put(
        np.broadcast_to(inc[:, None], (Bn, T)).astype(f32), cpu)
    cx = np.asarray(jax.jit(lambda a: jnp.cumsum(a, axis=1))(incb))
    p32 = (f32(2 * np.pi) * cx).astype(f32)
    r64 = np.mod(p32.astype(np.float64), 2 * np.pi)
    r64 = np.where(r64 > np.pi, r64 - 2 * np.pi, r64)
    rph = r64.astype(f32)

    partials = (f32(12000.0) / (f0 * np.log10(f0))).astype(f32)
    vc_cols = np.zeros((Bn, NK), f32)
    vc_cols[:, K_TANH] = f32(np.pi) * partials / f32(2.0)
    vc_cols[:, K_S] = s
    vc_cols[:, K_DRYSC] = f32(0.5) * (f32(1.0) - s / f32(2.0))
    vc_cols[:, K_IATT] = f32(1.0) / att
    vc_cols[:, K_ALPHA] = alpha
    vc_cols[:, K_IDEC] = f32(1.0) / dec
    vc_cols[:, K_BDEC] = -(att / dec)
    vc_cols[:, K_N1MS] = -(f32(1.0) - sus)
    vc_cols[:, K_IREL] = f32(1.0) / rel
    vc_cols[:, K_BREL] = -(note_on / rel)
    vc_cols[:, K_DIST] = dist

    tsec = (np.arange(T, dtype=f32) / f32(SR)).astype(f32).reshape(P, 1024)

    shm = np.zeros((P, 8 * P), f32)
    for si, sh in enumerate((1, 2, 4, 8, 16, 32, 64)):
        m = np.zeros((P, P), f32)
        m[np.arange(P - sh), np.arange(sh, P)] = 1.0
        shm[:, si * P:(si + 1) * P] = m

    in_maps = []
    for core in range(NCORES):
        vs = slice(core * NV, (core + 1) * NV)
        vc = np.ascontiguousarray(
            np.broadcast_to(vc_cols[vs].reshape(1, NV * NK), (P, NV * NK))
        ).astype(f32)
        in_maps.append(dict(
            rph=np.ascontiguousarray(rph[vs]),
            fcm=np.ascontiguousarray(fcm[vs]),
            qmm=np.ascontiguousarray(qmm[vs]),
            tsec=tsec, vc=vc, shm=shm,
        ))
    return in_maps


def run(inputs, trace=False):
    from concourse.bass_utils import run_bass_kernel_spmd

    if "nc" not in _CACHE:
        _CACHE["nc"] = _build_module()
    nc = _CACHE["nc"]
    in_maps = _host_prep(inputs)
    res = run_bass_kernel_spmd(nc, in_maps, core_ids=list(range(NCORES)),
                               trace=trace)
    dry = np.concatenate([r["dry"] for r in res.results], axis=0)
    wet = np.concatenate([r["wet"] for r in res.results], axis=0)
    env = np.concatenate([r["env"] for r in res.results], axis=0)
    return (dry, wet, env), res


def kernel(**inputs):
    out, _ = run(inputs, trace=False)
    return out


# revision 32
# speedup vs baseline: 1.0664x; 1.0043x over previous
"""AcidSynth Trainium2 kernel: 32 voices sharded 4-per-core across 8 NeuronCores.

Pipeline per core (4 voices, T=131072):
  - VCO sin/cos from host-reduced phase (ACT Sin), tanh square-saw morph
  - ADSR envelope (ACT Ln/Exp power curves)
  - RBJ time-varying lowpass biquad coefficients (ACT Exp/Sin + DVE)
  - IIR scan: block-parallel 3-channel method (particular + 2 homogeneous
    runs per block; linear correction), in-block steps vectorized over
    128 partitions x (4 voices x 8 chunks); carry chain composed
    intra-partition, then across partitions via PE shift-matmul
    Hillis-Steele scan
  - tanh distortion
"""
import numpy as np

f32 = np.float32
SR = 48000.0
T = 131072
P = 128
NV = 4            # voices per core
C = 8             # chunks per partition per voice
L = 1024 // C     # in-block scan length (128)
NCORES = 8

MIN_W = 2.0 * np.pi * 20.0 / SR
MAX_W = 2.0 * np.pi * 8000.0 / SR
MIN_Q = 0.7071
MAX_Q = 8.0

# voice-const column indices (vc tensor [128, NV*NK], col = v*NK + k)
K_TANH, K_S, K_DRYSC, K_IATT, K_ALPHA, K_IDEC, K_BDEC, K_N1MS, K_IREL, \
    K_BREL, K_DIST = range(11)
NK = 11

_CACHE = {}


def _build_module():
    import concourse.bacc as bacc
    import concourse.mybir as mybir
    from concourse import tile

    dt = mybir.dt.float32
    A = mybir.AluOpType
    F = mybir.ActivationFunctionType

    nc = bacc.Bacc()
    rph_d = nc.dram_tensor("rph", [NV, T], dt, kind="ExternalInput")
    fc_d = nc.dram_tensor("fcm", [NV, T], dt, kind="ExternalInput")
    qm_d = nc.dram_tensor("qmm", [NV, T], dt, kind="ExternalInput")
    ts_d = nc.dram_tensor("tsec", [P, 1024], dt, kind="ExternalInput")
    vc_d = nc.dram_tensor("vc", [P, NV * NK], dt, kind="ExternalInput")
    import os as _os
    DBG = bool(int(_os.environ.get("KDEBUG", "0")))
    dry_d = nc.dram_tensor("dry", [NV, T], dt, kind="ExternalOutput")
    wet_d = nc.dram_tensor("wet", [NV, T], dt, kind="ExternalOutput")
    env_d = nc.dram_tensor("env", [NV, T], dt, kind="ExternalOutput")
    if DBG:
        dbgw_d = nc.dram_tensor("dbgw", [NV, T], dt, kind="ExternalOutput")
        dbga_d = nc.dram_tensor("dbga", [NV, T], dt, kind="ExternalOutput")
        dbgy_d = nc.dram_tensor("dbgy", [P, NV * C * 3 * (L + 2)], dt,
                                kind="ExternalOutput")
        dbgab_d = nc.dram_tensor("dbgab", [P, 3 * NV * C], dt,
                                 kind="ExternalOutput")

    def dview(d):  # DRAM [NV, T] as [128, NV, 1024]
        return d[:, :].rearrange("v (p j) -> p v j", p=P)

    K1 = float(np.log(MAX_W / MIN_W))
    K0 = float(np.log(MIN_W))
    Kq1 = float(np.log(MAX_Q / MIN_Q))
    Kq0 = float(np.log(MIN_Q))
    PI = float(np.pi)
    BQ = -(Kq0 + float(np.log(2.0)))

    # ACT float biases need pre-registered const APs
    for val in (PI / 2, K0, BQ):
        t = nc.alloc_sbuf_tensor(f"constap-{val}", [128, 1], dt)
        nc.gpsimd.memset(t.ap(), val)
        nc.const_aps.aps[(dt, float(val))] = t.ap()
    nc.all_engine_barrier()

    with tile.TileContext(nc) as tc:
        with (
            tc.tile_pool(name="big", bufs=8) as bigp,
            tc.tile_pool(name="ybuf", bufs=1) as ypool,
            tc.tile_pool(name="keep", bufs=1) as keepp,
            tc.tile_pool(name="tmps", bufs=16) as tmpp,
        ):
            def big():
                return bigp.tile([P, NV * 1024], dt, tag="big", name="bigt")

            def keep(name, shape):
                return keepp.tile([P, *shape] if shape else [P, 1], dt,
                                  tag=name, name=name)

            def vsl(t, v):
                return t[:, v * 1024:(v + 1) * 1024]

            def vc1(v, k):
                return vc_t[:, v * NK + k: v * NK + k + 1]

            # ---------- small persistent loads ----------
            ts_t = keepp.tile([P, 1024], dt, tag="tsec")
            nc.sync.dma_start(ts_t[:], ts_d[:, :])
            vc_t = keepp.tile([P, NV * NK], dt, tag="vc")
            nc.sync.dma_start(vc_t[:], vc_d[:, :])

            # ---------- phase -> sin, cos ----------
            rph = big()
            for v in range(NV):
                nc.sync.dma_start(
                    vsl(rph, v)[:].rearrange("p (o j) -> p o j", o=1),
                    rph_d[v:v + 1, :].rearrange("o (p j) -> p o j", p=P))
            sn = big()
            nc.scalar.activation(sn[:], rph[:], F.Sin)
            ab = big()
            nc.scalar.activation(ab[:], rph[:], F.Abs)
            cs = big()
            nc.scalar.activation(cs[:], ab[:], F.Sin, bias=PI / 2, scale=-1.0)

            # ---------- ADSR envelope ----------
            env = big()
            tA = ab  # reuse slot logically: Tile tracks deps; use fresh tiles
            tA = big()
            for v in range(NV):
                # att -> env_v
                nc.vector.tensor_scalar(vsl(env, v)[:, 0:1024], ts_t[:],
                                        vc1(v, K_IATT), 1.0, op0=A.mult,
                                        op1=A.min)
                nc.scalar.activation(vsl(env, v), vsl(env, v), F.Ln)
                nc.scalar.activation(vsl(env, v), vsl(env, v), F.Exp,
                                     scale=vc1(v, K_ALPHA))
                # dec -> tA_v
                nc.scalar.activation(vsl(tA, v), ts_t[:], F.Relu,
                                     bias=vc1(v, K_BDEC),
                                     scale=vc1(v, K_IDEC))
                nc.vector.tensor_scalar(vsl(tA, v), vsl(tA, v), 1.0, None,
                                        op0=A.min)
                nc.scalar.activation(vsl(tA, v), vsl(tA, v), F.Ln)
                nc.scalar.activation(vsl(tA, v), vsl(tA, v), F.Exp,
                                     scale=vc1(v, K_ALPHA))
                nc.scalar.activation(vsl(tA, v), vsl(tA, v), F.Identity,
                                     bias=1.0, scale=vc1(v, K_N1MS))
            nc.vector.tensor_tensor(env[:], env[:], tA[:], op=A.mult)
            for v in range(NV):
                # rel -> tA_v
                nc.scalar.activation(vsl(tA, v), ts_t[:], F.Relu,
                                     bias=vc1(v, K_BREL),
                                     scale=vc1(v, K_IREL))
                nc.vector.tensor_scalar(vsl(tA, v), vsl(tA, v), 1.0, None,
                                        op0=A.min)
                nc.scalar.activation(vsl(tA, v), vsl(tA, v), F.Ln)
                nc.scalar.activation(vsl(tA, v), vsl(tA, v), F.Exp,
                                     scale=vc1(v, K_ALPHA))
                nc.scalar.activation(vsl(tA, v), vsl(tA, v), F.Identity,
                                     bias=1.0, scale=-1.0)
            nc.vector.tensor_tensor(env[:], env[:], tA[:], op=A.mult)
            for v in range(NV):
                nc.sync.dma_start(
                    env_d[v:v + 1, :].rearrange("o (p j) -> p o j", p=P),
                    vsl(env, v)[:].rearrange("p (o j) -> p o j", o=1))

            # ---------- VCO mix -> dry ----------
            sq = big()
            mx1 = big()
            for v in range(NV):
                nc.scalar.activation(vsl(sq, v), vsl(sn, v), F.Tanh,
                                     scale=vc1(v, K_TANH))
                nc.scalar.activation(vsl(mx1, v), vsl(cs, v), F.Identity,
                                     bias=1.0, scale=vc1(v, K_S))
            m2 = big()
            nc.gpsimd.tensor_tensor(m2[:], sq[:], mx1[:], op=A.mult)
            dry = big()
            for v in range(NV):
                nc.vector.scalar_tensor_tensor(
                    vsl(dry, v), vsl(m2, v), vc1(v, K_DRYSC), vsl(env, v),
                    op0=A.mult, op1=A.mult)
            for v in range(NV):
                nc.sync.dma_start(
                    dry_d[v:v + 1, :].rearrange("o (p j) -> p o j", p=P),
                    vsl(dry, v)[:].rearrange("p (o j) -> p o j", o=1))

            # ---------- biquad coefficients ----------
            fc = big()
            for v in range(NV):
                nc.sync.dma_start(
                    vsl(fc, v)[:].rearrange("p (o j) -> p o j", o=1),
                    fc_d[v:v + 1, :].rearrange("o (p j) -> p o j", p=P))
            qm = big()
            for v in range(NV):
                nc.sync.dma_start(
                    vsl(qm, v)[:].rearrange("p (o j) -> p o j", o=1),
                    qm_d[v:v + 1, :].rearrange("o (p j) -> p o j", p=P))
            w0 = big()
            nc.scalar.activation(w0[:], fc[:], F.Exp, bias=K0, scale=K1)
            ee = big()
            nc.scalar.activation(ee[:], qm[:], F.Exp, bias=BQ, scale=-Kq1)
            sn0 = big()
            nc.scalar.activation(sn0[:], w0[:], F.Sin)
            cw = big()
            nc.scalar.activation(cw[:], w0[:], F.Sin, bias=PI / 2, scale=1.0)
            al = big()
            nc.gpsimd.tensor_tensor(al[:], sn0[:], ee[:], op=A.mult)
            a0 = big()
            nc.scalar.activation(a0[:], al[:], F.Identity, bias=1.0,
                                 scale=1.0)
            ra = big()
            rs = big()
            nc.vector.reciprocal_approx_accurate(ra[:], a0[:], rs[:])
            nb1 = big()
            nc.vector.scalar_tensor_tensor(nb1[:], cw[:], 1.0, ra[:],
                                           op0=A.subtract, op1=A.mult)
            na1 = big()
            nc.vector.scalar_tensor_tensor(na1[:], cw[:], 2.0, ra[:],
                                           op0=A.mult, op1=A.mult)
            na2 = big()
            nc.vector.scalar_tensor_tensor(na2[:], al[:], 1.0, ra[:],
                                           op0=A.subtract, op1=A.mult)

            # ---------- FIR: w = b1*(0.5*(x + x2) + x1) = nb1*(-0.5*(x+x2)-x1)
            x1 = big()
            x2 = big()
            for v in range(NV):
                nc.gpsimd.tensor_copy(vsl(x1, v)[:, 1:1024],
                                      vsl(dry, v)[:, 0:1023])
                nc.vector.memset(x1[0:1, v * 1024:v * 1024 + 1], 0.0)
                nc.sync.dma_start(x1[1:P, v * 1024:v * 1024 + 1],
                                  dry[0:P - 1, (v + 1) * 1024 - 1:
                                      (v + 1) * 1024])
            for v in range(NV):
                nc.gpsimd.tensor_copy(vsl(x2, v)[:, 1:1024],
                                      vsl(x1, v)[:, 0:1023])
                nc.vector.memset(x2[0:1, v * 1024:v * 1024 + 1], 0.0)
                nc.sync.dma_start(x2[1:P, v * 1024:v * 1024 + 1],
                                  x1[0:P - 1, (v + 1) * 1024 - 1:
                                     (v + 1) * 1024])
            ssum = big()
            nc.gpsimd.tensor_tensor(ssum[:], dry[:], x2[:], op=A.add)
            uu = big()
            nc.vector.scalar_tensor_tensor(uu[:], ssum[:], -0.5, x1[:],
                                           op0=A.mult, op1=A.subtract)
            wf = big()
            nc.vector.tensor_tensor(wf[:], uu[:], nb1[:], op=A.mult)

            if DBG:
                nc.sync.dma_start(dview(dbgw_d),
                                  wf[:].rearrange("p (v j) -> p v j", v=NV))
                nc.sync.dma_start(dview(dbga_d),
                                  na1[:].rearrange("p (v j) -> p v j", v=NV))
            # ---------- blocked IIR scan, level 1 ----------
            Y = ypool.tile([P, NV * C * 3 * (L + 2)], dt, tag="Y")
            Yv = Y[:].rearrange("p (v c ch i) -> p v c ch i", v=NV, c=C,
                                ch=3)
            nc.vector.memset(Yv[:, :, :, 0, 0:2], 0.0)
            nc.vector.memset(Yv[:, :, :, 1, 0:1], 0.0)
            nc.vector.memset(Yv[:, :, :, 1, 1:2], 1.0)
            nc.vector.memset(Yv[:, :, :, 2, 0:1], 1.0)
            nc.vector.memset(Yv[:, :, :, 2, 1:2], 0.0)

            # pair-step coefficients: y[2k+1] = PA*y[2k-1] + PB*y[2k-2] + PR
            #   PA = na1'*na1 + na2',  PB = na1'*na2,  PR = na1'*w + w'
            # packed as [p, g(32), k(64), r(2)] with r = (even formula, pair)
            na1p = na1[:].rearrange("p (g k r) -> p g k r", g=NV * C, k=L // 2)
            na2p = na2[:].rearrange("p (g k r) -> p g k r", g=NV * C, k=L // 2)
            wfp = wf[:].rearrange("p (g k r) -> p g k r", g=NV * C, k=L // 2)
            pk1 = big()
            pk1v = pk1[:].rearrange("p (g k r) -> p g k r", g=NV * C,
                                    k=L // 2)
            nc.gpsimd.tensor_copy(pk1v[:, :, :, 0], na1p[:, :, :, 0])
            nc.vector.tensor_tensor(pk1v[:, :, :, 1], na1p[:, :, :, 1],
                                    na1p[:, :, :, 0], op=A.mult)
            nc.vector.tensor_tensor(pk1v[:, :, :, 1], pk1v[:, :, :, 1],
                                    na2p[:, :, :, 1], op=A.add)
            pk2 = big()
            pk2v = pk2[:].rearrange("p (g k r) -> p g k r", g=NV * C,
                                    k=L // 2)
            nc.gpsimd.tensor_copy(pk2v[:, :, :, 0], na2p[:, :, :, 0])
            nc.vector.tensor_tensor(pk2v[:, :, :, 1], na1p[:, :, :, 1],
                                    na2p[:, :, :, 0], op=A.mult)
            pkw = big()
            pkwv = pkw[:].rearrange("p (g k r) -> p g k r", g=NV * C,
                                    k=L // 2)
            nc.gpsimd.tensor_copy(pkwv[:, :, :, 0], wfp[:, :, :, 0])
            nc.vector.tensor_tensor(pkwv[:, :, :, 1], na1p[:, :, :, 1],
                                    wfp[:, :, :, 0], op=A.mult)
            nc.vector.tensor_tensor(pkwv[:, :, :, 1], pkwv[:, :, :, 1],
                                    wfp[:, :, :, 1], op=A.add)

            Yg = Y[:].rearrange("p (g ch i) -> p g ch i", g=NV * C, ch=3)
            G = NV * C

            def bcy(col):
                return Yg[:, :, :, col].unsqueeze(3).to_broadcast(
                    (P, G, 3, 2))

            def bck(pkv, k):
                return pkv[:, :, k, :].unsqueeze(2).to_broadcast(
                    (P, G, 3, 2))

            import os as _os2
            _SKIP = bool(int(_os2.environ.get("KSKIP_SCAN", "0")))
            for k in range(0 if _SKIP else L // 2):
                c0 = 2 * k
                t1 = tmpp.tile([P, G * 3 * 2], dt, tag="sc", name="t1")
                t1v = t1[:].rearrange("p (g ch r) -> p g ch r", g=G, ch=3)
                t2 = tmpp.tile([P, G * 3 * 2], dt, tag="sc", name="t2")
                t2v = t2[:].rearrange("p (g ch r) -> p g ch r", g=G, ch=3)
                nc.vector.tensor_tensor(t1v, bcy(c0 + 1), bck(pk1v, k),
                                        op=A.mult)
                nc.gpsimd.tensor_tensor(t2v, bcy(c0), bck(pk2v, k),
                                        op=A.mult)
                nc.vector.tensor_tensor(t1v, t1v, t2v, op=A.add)
                nc.vector.tensor_tensor(Yg[:, :, :, c0 + 2:c0 + 4], t1v,
                                        bck(pkwv, k), op=A.add)

            # ---------- block transfers ----------
            # T entries [128, NV, C]: h1e h2e h1e2 h2e2 ue ue2
            TmE = keep("TmE", [NV * C * 6])
            TmEv = TmE[:].rearrange("p (e v c) -> p e v c", e=6, v=NV)
            for (e, ch, col) in ((0, 1, L + 1), (1, 2, L + 1), (2, 1, L),
                                 (3, 2, L)):
                nc.vector.tensor_tensor(TmEv[:, e], Yv[:, :, :, ch, col],
                                        Yv[:, :, :, 0, col], op=A.subtract)
            nc.vector.tensor_copy(TmEv[:, 4], Yv[:, :, :, 0, L + 1])
            nc.vector.tensor_copy(TmEv[:, 5], Yv[:, :, :, 0, L])

            # ---------- intra-partition exclusive prefixes over c ----------
            # Pm00 Pm01 Pm10 Pm11 Pu0 Pu1: [128, 6, NV, C]
            PmE = keep("PmE", [6 * NV * C])
            PmEv = PmE[:].rearrange("p (e v c) -> p e v c", e=6, v=NV)
            for e in range(6):
                init = 1.0 if e in (0, 3) else 0.0
                nc.vector.memset(PmEv[:, e, :, 0:1], init)

            def compose_step(dst, dcc, S, scc, Tt, tcc):
                """dst[:,:,dcc] = T[:,:,tcc] o S[:,:,scc] (T applied second)"""
                tq = tmpp.tile([P, 2 * NV], dt, tag="c8", name="tq")
                tqv = tq[:].rearrange("p (e v) -> p e v", e=2)
                tq2 = tmpp.tile([P, 2 * NV], dt, tag="c8", name="tq2")
                tq2v = tq2[:].rearrange("p (e v) -> p e v", e=2)
                # m rows batched: (n00,n01) = T00*(S00,S01) + T01*(S10,S11)
                #                 (n10,n11) = T10*(S00,S01) + T11*(S10,S11)
                for (dr, a1, a2) in ((0, 0, 1), (2, 2, 3)):
                    t00b = Tt[:, a1, :, tcc].unsqueeze(1).to_broadcast(
                        (P, 2, NV))
                    t01b = Tt[:, a2, :, tcc].unsqueeze(1).to_broadcast(
                        (P, 2, NV))
                    nc.vector.tensor_tensor(tqv, t00b, S[:, 0:2, :, scc],
                                            op=A.mult)
                    nc.vector.tensor_tensor(tq2v, t01b, S[:, 2:4, :, scc],
                                            op=A.mult)
                    nc.vector.tensor_tensor(dst[:, dr:dr + 2, :, dcc],
                                            tqv, tq2v, op=A.add)
                # u: newu0 = T00*Su0 + T01*Su1 + Tu0 ; newu1 = T10.. + Tu1
                tq3 = tmpp.tile([P, NV], dt, tag="c4", name="tq3")
                tq4 = tmpp.tile([P, NV], dt, tag="c4", name="tq4")
                for (de, a1, a2, au) in ((4, 0, 1, 4), (5, 2, 3, 5)):
                    nc.gpsimd.tensor_tensor(tq3[:], Tt[:, a1, :, tcc],
                                            S[:, 4, :, scc], op=A.mult)
                    nc.gpsimd.tensor_tensor(tq4[:], Tt[:, a2, :, tcc],
                                            S[:, 5, :, scc], op=A.mult)
                    nc.gpsimd.tensor_tensor(tq3[:], tq3[:], tq4[:], op=A.add)
                    nc.gpsimd.tensor_tensor(dst[:, de, :, dcc], tq3[:],
                                            Tt[:, au, :, tcc], op=A.add)

            for cc in range(1, C):
                compose_step(PmEv, cc, PmEv, cc - 1, TmEv, cc - 1)

            # full-partition transfer F = T_{C-1} o P_{C-1}, packed [128,NV,6]
            Fp = keep("Fp", [NV * 6])
            FpE = Fp[:].rearrange("p (e v c) -> p e v c", e=6, v=NV, c=1)
            compose_step(FpE, 0, PmEv, C - 1, TmEv, C - 1)

            # ---------- Hillis-Steele across partitions via PE shifts -------
            # pack to [128, NV*6] with (v, e) layout for matmul rhs
            Xp = keep("Xp", [NV * 6])
            Xv = Xp[:].rearrange("p (v e) -> p v e", v=NV)
            for e in range(6):
                nc.vector.tensor_copy(Xv[:, :, e], FpE[:, e, :, 0])

            cur = Xp
            for si, s in enumerate((1, 2, 4, 8, 16, 32, 64)):
                sh = tmpp.tile([P, NV * 6], dt, tag="hs", name="sh")
                nc.vector.memset(sh[0:s, :], 0.0)
                nc.sync.dma_start(sh[s:P, :], cur[0:P - s, :])
                shv = sh[:].rearrange("p (v e) -> p v e", v=NV)
                nc.vector.memset(shv[0:s, :, 0:1], 1.0)
                nc.vector.memset(shv[0:s, :, 3:4], 1.0)
                nxt = tmpp.tile([P, NV * 6], dt, tag="hs", name="nxt")
                nxtv = nxt[:].rearrange("p (v e) -> p v e", v=NV)
                curv = cur[:].rearrange("p (v e) -> p v e", v=NV)
                tq = tmpp.tile([P, NV * 2], dt, tag="c8", name="tq")
                tqv = tq[:].rearrange("p (v e) -> p v e", v=NV)
                tq2 = tmpp.tile([P, NV * 2], dt, tag="c8", name="tq2")
                tq2v = tq2[:].rearrange("p (v e) -> p v e", v=NV)
                for (dr, a1, a2) in ((0, 0, 1), (2, 2, 3)):
                    c0b = curv[:, :, a1].unsqueeze(2).to_broadcast((P, NV, 2))
                    c1b = curv[:, :, a2].unsqueeze(2).to_broadcast((P, NV, 2))
                    nc.vector.tensor_tensor(tqv, c0b, shv[:, :, 0:2],
                                            op=A.mult)
                    nc.vector.tensor_tensor(tq2v, c1b, shv[:, :, 2:4],
                                            op=A.mult)
                    nc.vector.tensor_tensor(nxtv[:, :, dr:dr + 2], tqv, tq2v,
                                            op=A.add)
                tq3 = tmpp.tile([P, NV], dt, tag="c4", name="tq3")
                tq4 = tmpp.tile([P, NV], dt, tag="c4", name="tq4")
                for (de, a1, a2) in ((4, 0, 1), (5, 2, 3)):
                    nc.gpsimd.tensor_tensor(tq3[:], curv[:, :, a1],
                                            shv[:, :, 4], op=A.mult)
                    nc.gpsimd.tensor_tensor(tq4[:], curv[:, :, a2],
                                            shv[:, :, 5], op=A.mult)
                    nc.gpsimd.tensor_tensor(tq3[:], tq3[:], tq4[:], op=A.add)
                    nc.gpsimd.tensor_tensor(nxtv[:, :, de], tq3[:],
                                            curv[:, :, de], op=A.add)
                cur = nxt

            # exclusive shift by 1: s0 = shift1(cur); rows<1 -> zeros
            s0 = keep("s0", [NV * 6])
            nc.vector.memset(s0[0:1, :], 0.0)
            nc.sync.dma_start(s0[1:P, :], cur[0:P - 1, :])
            s0v = s0[:].rearrange("p (v e) -> p v e", v=NV)

            # ---------- per-block incoming states: ab = Pm@s0u + Pu --------
            abt = keep("abt", [3 * NV * C])
            abv = abt[:].rearrange("p (e v c) -> p e v c", e=3, v=NV)
            s0u0b = s0v[:, :, 4].unsqueeze(2).to_broadcast((P, NV, C))
            s0u1b = s0v[:, :, 5].unsqueeze(2).to_broadcast((P, NV, C))
            tqa = tmpp.tile([P, NV * C], dt, tag="c32", name="tqa")
            tqav = tqa[:].rearrange("p (v c) -> p v c", v=NV)
            tqb = tmpp.tile([P, NV * C], dt, tag="c32", name="tqb")
            tqbv = tqb[:].rearrange("p (v c) -> p v c", v=NV)
            for (de, a1, a2, au) in ((0, 0, 1, 4), (1, 2, 3, 5)):
                nc.vector.tensor_tensor(tqav, PmEv[:, a1], s0u0b, op=A.mult)
                nc.vector.tensor_tensor(tqbv, PmEv[:, a2], s0u1b, op=A.mult)
                nc.vector.tensor_tensor(tqav, tqav, tqbv, op=A.add)
                nc.vector.tensor_tensor(abv[:, de], tqav, PmEv[:, au],
                                        op=A.add)
            # gamma = 1 - a - b
            nc.vector.tensor_tensor(tqav, abv[:, 0], abv[:, 1], op=A.add)
            nc.vector.tensor_scalar(abv[:, 2], tqav, -1.0, 1.0,
                                    op0=A.mult, op1=A.add)

            # ---------- correction + wet ----------
            yout = big()
            youtv = yout[:].rearrange("p (v c i) -> p v c i", v=NV, c=C)
            a_b = abv[:, 0].unsqueeze(3).to_broadcast((P, NV, C, L))
            b_b = abv[:, 1].unsqueeze(3).to_broadcast((P, NV, C, L))
            g_b = abv[:, 2].unsqueeze(3).to_broadcast((P, NV, C, L))
            ct1 = big()
            ct1v = ct1[:].rearrange("p (v c i) -> p v c i", v=NV, c=C)
            ct2 = big()
            ct2v = ct2[:].rearrange("p (v c i) -> p v c i", v=NV, c=C)
            nc.vector.tensor_tensor(ct1v, Yv[:, :, :, 1, 2:L + 2], a_b,
                                    op=A.mult)
            nc.vector.tensor_tensor(ct2v, Yv[:, :, :, 2, 2:L + 2], b_b,
                                    op=A.mult)
            nc.vector.tensor_tensor(ct1v, ct1v, ct2v, op=A.add)
            nc.vector.tensor_tensor(ct2v, Yv[:, :, :, 0, 2:L + 2], g_b,
                                    op=A.mult)
            nc.vector.tensor_tensor(youtv, ct1v, ct2v, op=A.add)
            wet = big()
            for v in range(NV):
                nc.scalar.activation(vsl(wet, v), vsl(yout, v), F.Tanh,
                                     scale=vc1(v, K_DIST))
            for v in range(NV):
                nc.sync.dma_start(
                    wet_d[v:v + 1, :].rearrange("o (p j) -> p o j", p=P),
                    vsl(wet, v)[:].rearrange("p (o j) -> p o j", o=1))

    nc.compile()
    return nc


def _host_prep(inputs):
    """Per-core input maps. Heavy phase cumsum matches eager CPU-XLA jax."""
    import jax
    import jax.numpy as jnp

    cpu = jax.devices("cpu")[0]
    f0 = np.asarray(inputs["f0_hz"], f32)
    s = np.asarray(inputs["osc_shape"], f32)
    note_on = np.asarray(inputs["note_on_duration"], f32)
    fcm = np.asarray(inputs["fc_mod_sig"], f32)
    qmm = np.asarray(inputs["q_mod_sig"], f32)
    dist = np.asarray(inputs["dist_gain"], f32)
    att = np.asarray(inputs["attack"], f32)
    dec = np.asarray(inputs["decay"], f32)
    sus = np.asarray(inputs["sustain"], f32)
    rel = np.asarray(inputs["release"], f32)
    alpha = np.asarray(inputs["alpha"], f32)
    Bn = f0.shape[0]

    inc = (f0 / f32(SR)).astype(f32)
    incb = jax.device_put(
        np.broadcast_to(inc[:, None], (Bn, T)).astype(f32), cpu)
    cx = np.asarray(jax.jit(lambda a: jnp.cumsum(a, axis=1))(incb))
    p32 = (f32(2 * np.pi) * cx).astype(f32)
    r64 = np.mod(p32.astype(np.float64), 2 * np.pi)
    r64 = np.where(r64 > np.pi, r64 - 2 * np.pi, r64)
    rph = r64.astype(f32)

    partials = (f32(12000.0) / (f0 * np.log10(f0))).astype(f32)
    vc_cols = np.zeros((Bn, NK), f32)
    vc_cols[:, K_TANH] = f32(np.pi) * partials / f32(2.0)
    vc_cols[:, K_S] = s
    vc_cols[:, K_DRYSC] = f32(0.5) * (f32(1.0) - s / f32(2.0))
    vc_cols[:, K_IATT] = f32(1.0) / att
    vc_cols[:, K_ALPHA] = alpha
    vc_cols[:, K_IDEC] = f32(1.0) / dec
    vc_cols[:, K_BDEC] = -(att / dec)
    vc_cols[:, K_N1MS] = -(f32(1.0) - sus)
    vc_cols[:, K_IREL] = f32(1.0) / rel
    vc_cols[:, K_BREL] = -(note_on / rel)
    vc_cols[:, K_DIST] = dist

    tsec = (np.arange(T, dtype=f32) / f32(SR)).astype(f32).reshape(P, 1024)

    in_maps = []
    for core in range(NCORES):
        vs = slice(core * NV, (core + 1) * NV)
        vc = np.ascontiguousarray(
            np.broadcast_to(vc_cols[vs].reshape(1, NV * NK), (P, NV * NK))
        ).astype(f32)
        in_maps.append(dict(
            rph=np.ascontiguousarray(rph[vs]),
            fcm=np.ascontiguousarray(fcm[vs]),
            qmm=np.ascontiguousarray(qmm[vs]),
            tsec=tsec, vc=vc,
        ))
    return in_maps


def run(inputs, trace=False):
    from concourse.bass_utils import run_bass_kernel_spmd

    if "nc" not in _CACHE:
        _CACHE["nc"] = _build_module()
    nc = _CACHE["nc"]
    in_maps = _host_prep(inputs)
    res = run_bass_kernel_spmd(nc, in_maps, core_ids=list(range(NCORES)),
                               trace=trace)
    dry = np.concatenate([r["dry"] for r in res.results], axis=0)
    wet = np.concatenate([r["wet"] for r in res.results], axis=0)
    env = np.concatenate([r["env"] for r in res.results], axis=0)
    return (dry, wet, env), res


def kernel(**inputs):
    out, _ = run(inputs, trace=False)
    return out


# revision 35
# speedup vs baseline: 1.0867x; 1.0190x over previous
"""AcidSynth Trainium2 kernel: 32 voices sharded 4-per-core across 8 NeuronCores.

Pipeline per core (4 voices, T=131072):
  - VCO sin/cos from host-reduced phase (ACT Sin), tanh square-saw morph
  - ADSR envelope (ACT Ln/Exp power curves)
  - RBJ time-varying lowpass biquad coefficients (ACT Exp/Sin + DVE)
  - IIR scan: block-parallel 3-channel method (particular + 2 homogeneous
    runs per block; linear correction), pair-stepped in-block scan
    vectorized over 128 partitions x (4 voices x 8 chunks); carry chain
    composed intra-partition, then across partitions via DMA
    partition-shift Hillis-Steele scan
  - tanh distortion
"""
import numpy as np

f32 = np.float32
SR = 48000.0
T = 131072
P = 128
NV = 4            # voices per core
C = 16            # chunks per partition per voice
L = 1024 // C     # in-block scan length (128)
NCORES = 8

MIN_W = 2.0 * np.pi * 20.0 / SR
MAX_W = 2.0 * np.pi * 8000.0 / SR
MIN_Q = 0.7071
MAX_Q = 8.0

# voice-const column indices (vc tensor [128, NV*NK], col = v*NK + k)
K_TANH, K_S, K_DRYSC, K_IATT, K_ALPHA, K_IDEC, K_BDEC, K_N1MS, K_IREL, \
    K_BREL, K_DIST = range(11)
NK = 11

_CACHE = {}


def _build_module():
    import concourse.bacc as bacc
    import concourse.mybir as mybir
    from concourse import tile

    dt = mybir.dt.float32
    A = mybir.AluOpType
    F = mybir.ActivationFunctionType

    nc = bacc.Bacc()
    rph_d = nc.dram_tensor("rph", [NV, T], dt, kind="ExternalInput")
    fc_d = nc.dram_tensor("fcm", [NV, T], dt, kind="ExternalInput")
    qm_d = nc.dram_tensor("qmm", [NV, T], dt, kind="ExternalInput")
    ts_d = nc.dram_tensor("tsec", [P, 1024], dt, kind="ExternalInput")
    vc_d = nc.dram_tensor("vc", [P, NV * NK], dt, kind="ExternalInput")
    import os as _os
    DBG = bool(int(_os.environ.get("KDEBUG", "0")))
    dry_d = nc.dram_tensor("dry", [NV, T], dt, kind="ExternalOutput")
    wet_d = nc.dram_tensor("wet", [NV, T], dt, kind="ExternalOutput")
    env_d = nc.dram_tensor("env", [NV, T], dt, kind="ExternalOutput")
    if DBG:
        dbgw_d = nc.dram_tensor("dbgw", [NV, T], dt, kind="ExternalOutput")
        dbga_d = nc.dram_tensor("dbga", [NV, T], dt, kind="ExternalOutput")
        dbgy_d = nc.dram_tensor("dbgy", [P, NV * C * 3 * (L + 2)], dt,
                                kind="ExternalOutput")
        dbgab_d = nc.dram_tensor("dbgab", [P, 3 * NV * C], dt,
                                 kind="ExternalOutput")

    def dview(d):  # DRAM [NV, T] as [128, NV, 1024]
        return d[:, :].rearrange("v (p j) -> p v j", p=P)

    K1 = float(np.log(MAX_W / MIN_W))
    K0 = float(np.log(MIN_W))
    Kq1 = float(np.log(MAX_Q / MIN_Q))
    Kq0 = float(np.log(MIN_Q))
    PI = float(np.pi)
    BQ = -(Kq0 + float(np.log(2.0)))

    # ACT float biases need pre-registered const APs
    for val in (PI / 2, K0, BQ):
        t = nc.alloc_sbuf_tensor(f"constap-{val}", [128, 1], dt)
        nc.gpsimd.memset(t.ap(), val)
        nc.const_aps.aps[(dt, float(val))] = t.ap()
    nc.all_engine_barrier()

    with tile.TileContext(nc) as tc:
        with (
            tc.tile_pool(name="big", bufs=8) as bigp,
            tc.tile_pool(name="ybuf", bufs=1) as ypool,
            tc.tile_pool(name="keep", bufs=1) as keepp,
            tc.tile_pool(name="tmps", bufs=8) as tmpp,
        ):
            def big():
                return bigp.tile([P, NV * 1024], dt, tag="big", name="bigt")

            def keep(name, shape):
                return keepp.tile([P, *shape] if shape else [P, 1], dt,
                                  tag=name, name=name)

            def vsl(t, v):
                return t[:, v * 1024:(v + 1) * 1024]

            def vc1(v, k):
                return vc_t[:, v * NK + k: v * NK + k + 1]

            # ---------- small persistent loads ----------
            ts_t = keepp.tile([P, 1024], dt, tag="tsec")
            nc.sync.dma_start(ts_t[:], ts_d[:, :])
            vc_t = keepp.tile([P, NV * NK], dt, tag="vc")
            nc.sync.dma_start(vc_t[:], vc_d[:, :])

            # ---------- phase -> sin, cos ----------
            rph = big()
            for v in range(NV):
                nc.sync.dma_start(
                    vsl(rph, v)[:].rearrange("p (o j) -> p o j", o=1),
                    rph_d[v:v + 1, :].rearrange("o (p j) -> p o j", p=P))
            sn = big()
            nc.scalar.activation(sn[:], rph[:], F.Sin)
            ab = big()
            nc.scalar.activation(ab[:], rph[:], F.Abs)
            cs = big()
            nc.scalar.activation(cs[:], ab[:], F.Sin, bias=PI / 2, scale=-1.0)

            # ---------- ADSR envelope ----------
            env = big()
            tA = ab  # reuse slot logically: Tile tracks deps; use fresh tiles
            tA = big()
            for v in range(NV):
                # att -> env_v
                nc.vector.tensor_scalar(vsl(env, v)[:, 0:1024], ts_t[:],
                                        vc1(v, K_IATT), 1.0, op0=A.mult,
                                        op1=A.min)
                nc.scalar.activation(vsl(env, v), vsl(env, v), F.Ln)
                nc.scalar.activation(vsl(env, v), vsl(env, v), F.Exp,
                                     scale=vc1(v, K_ALPHA))
                # dec -> tA_v
                nc.scalar.activation(vsl(tA, v), ts_t[:], F.Relu,
                                     bias=vc1(v, K_BDEC),
                                     scale=vc1(v, K_IDEC))
                nc.vector.tensor_scalar(vsl(tA, v), vsl(tA, v), 1.0, None,
                                        op0=A.min)
                nc.scalar.activation(vsl(tA, v), vsl(tA, v), F.Ln)
                nc.scalar.activation(vsl(tA, v), vsl(tA, v), F.Exp,
                                     scale=vc1(v, K_ALPHA))
                nc.scalar.activation(vsl(tA, v), vsl(tA, v), F.Identity,
                                     bias=1.0, scale=vc1(v, K_N1MS))
            nc.vector.tensor_tensor(env[:], env[:], tA[:], op=A.mult)
            for v in range(NV):
                # rel -> tA_v
                nc.scalar.activation(vsl(tA, v), ts_t[:], F.Relu,
                                     bias=vc1(v, K_BREL),
                                     scale=vc1(v, K_IREL))
                nc.vector.tensor_scalar(vsl(tA, v), vsl(tA, v), 1.0, None,
                                        op0=A.min)
                nc.scalar.activation(vsl(tA, v), vsl(tA, v), F.Ln)
                nc.scalar.activation(vsl(tA, v), vsl(tA, v), F.Exp,
                                     scale=vc1(v, K_ALPHA))
                nc.scalar.activation(vsl(tA, v), vsl(tA, v), F.Identity,
                                     bias=1.0, scale=-1.0)
            nc.vector.tensor_tensor(env[:], env[:], tA[:], op=A.mult)
            for v in range(NV):
                nc.sync.dma_start(
                    env_d[v:v + 1, :].rearrange("o (p j) -> p o j", p=P),
                    vsl(env, v)[:].rearrange("p (o j) -> p o j", o=1))

            # ---------- VCO mix -> dry ----------
            sq = big()
            mx1 = big()
            for v in range(NV):
                nc.scalar.activation(vsl(sq, v), vsl(sn, v), F.Tanh,
                                     scale=vc1(v, K_TANH))
                nc.scalar.activation(vsl(mx1, v), vsl(cs, v), F.Identity,
                                     bias=1.0, scale=vc1(v, K_S))
            m2 = big()
            nc.gpsimd.tensor_tensor(m2[:], sq[:], mx1[:], op=A.mult)
            dry = big()
            for v in range(NV):
                nc.vector.scalar_tensor_tensor(
                    vsl(dry, v), vsl(m2, v), vc1(v, K_DRYSC), vsl(env, v),
                    op0=A.mult, op1=A.mult)
            for v in range(NV):
                nc.sync.dma_start(
                    dry_d[v:v + 1, :].rearrange("o (p j) -> p o j", p=P),
                    vsl(dry, v)[:].rearrange("p (o j) -> p o j", o=1))

            # ---------- biquad coefficients ----------
            fc = big()
            for v in range(NV):
                nc.sync.dma_start(
                    vsl(fc, v)[:].rearrange("p (o j) -> p o j", o=1),
                    fc_d[v:v + 1, :].rearrange("o (p j) -> p o j", p=P))
            qm = big()
            for v in range(NV):
                nc.sync.dma_start(
                    vsl(qm, v)[:].rearrange("p (o j) -> p o j", o=1),
                    qm_d[v:v + 1, :].rearrange("o (p j) -> p o j", p=P))
            w0 = big()
            nc.scalar.activation(w0[:], fc[:], F.Exp, bias=K0, scale=K1)
            ee = big()
            nc.scalar.activation(ee[:], qm[:], F.Exp, bias=BQ, scale=-Kq1)
            sn0 = big()
            nc.scalar.activation(sn0[:], w0[:], F.Sin)
            cw = big()
            nc.scalar.activation(cw[:], w0[:], F.Sin, bias=PI / 2, scale=1.0)
            al = big()
            nc.gpsimd.tensor_tensor(al[:], sn0[:], ee[:], op=A.mult)
            a0 = big()
            nc.scalar.activation(a0[:], al[:], F.Identity, bias=1.0,
                                 scale=1.0)
            ra = big()
            rs = big()
            nc.vector.reciprocal_approx_accurate(ra[:], a0[:], rs[:])
            nb1 = big()
            nc.vector.scalar_tensor_tensor(nb1[:], cw[:], 1.0, ra[:],
                                           op0=A.subtract, op1=A.mult)
            na1 = big()
            nc.vector.scalar_tensor_tensor(na1[:], cw[:], 2.0, ra[:],
                                           op0=A.mult, op1=A.mult)
            na2 = big()
            nc.vector.scalar_tensor_tensor(na2[:], al[:], 1.0, ra[:],
                                           op0=A.subtract, op1=A.mult)

            # ---------- FIR: w = b1*(0.5*(x + x2) + x1) = nb1*(-0.5*(x+x2)-x1)
            x1 = big()
            x2 = big()
            for v in range(NV):
                nc.gpsimd.tensor_copy(vsl(x1, v)[:, 1:1024],
                                      vsl(dry, v)[:, 0:1023])
                nc.vector.memset(x1[0:1, v * 1024:v * 1024 + 1], 0.0)
                nc.sync.dma_start(x1[1:P, v * 1024:v * 1024 + 1],
                                  dry[0:P - 1, (v + 1) * 1024 - 1:
                                      (v + 1) * 1024])
            for v in range(NV):
                nc.gpsimd.tensor_copy(vsl(x2, v)[:, 1:1024],
                                      vsl(x1, v)[:, 0:1023])
                nc.vector.memset(x2[0:1, v * 1024:v * 1024 + 1], 0.0)
                nc.sync.dma_start(x2[1:P, v * 1024:v * 1024 + 1],
                                  x1[0:P - 1, (v + 1) * 1024 - 1:
                                     (v + 1) * 1024])
            ssum = big()
            nc.gpsimd.tensor_tensor(ssum[:], dry[:], x2[:], op=A.add)
            uu = big()
            nc.vector.scalar_tensor_tensor(uu[:], ssum[:], -0.5, x1[:],
                                           op0=A.mult, op1=A.subtract)
            wf = big()
            nc.vector.tensor_tensor(wf[:], uu[:], nb1[:], op=A.mult)

            if DBG:
                nc.sync.dma_start(dview(dbgw_d),
                                  wf[:].rearrange("p (v j) -> p v j", v=NV))
                nc.sync.dma_start(dview(dbga_d),
                                  na1[:].rearrange("p (v j) -> p v j", v=NV))
            # ---------- blocked IIR scan, level 1 ----------
            Y = ypool.tile([P, NV * C * 3 * (L + 2)], dt, tag="Y")
            Yv = Y[:].rearrange("p (v c ch i) -> p v c ch i", v=NV, c=C,
                                ch=3)
            nc.vector.memset(Yv[:, :, :, 0, 0:2], 0.0)
            nc.vector.memset(Yv[:, :, :, 1, 0:1], 0.0)
            nc.vector.memset(Yv[:, :, :, 1, 1:2], 1.0)
            nc.vector.memset(Yv[:, :, :, 2, 0:1], 1.0)
            nc.vector.memset(Yv[:, :, :, 2, 1:2], 0.0)

            # pair-step coefficients: y[2k+1] = PA*y[2k-1] + PB*y[2k-2] + PR
            #   PA = na1'*na1 + na2',  PB = na1'*na2,  PR = na1'*w + w'
            # packed as [p, g(32), k(64), r(2)] with r = (even formula, pair)
            na1p = na1[:].rearrange("p (g k r) -> p g k r", g=NV * C, k=L // 2)
            na2p = na2[:].rearrange("p (g k r) -> p g k r", g=NV * C, k=L // 2)
            wfp = wf[:].rearrange("p (g k r) -> p g k r", g=NV * C, k=L // 2)
            pk1 = big()
            pk1v = pk1[:].rearrange("p (g k r) -> p g k r", g=NV * C,
                                    k=L // 2)
            nc.gpsimd.tensor_copy(pk1v[:, :, :, 0], na1p[:, :, :, 0])
            nc.vector.tensor_tensor(pk1v[:, :, :, 1], na1p[:, :, :, 1],
                                    na1p[:, :, :, 0], op=A.mult)
            nc.vector.tensor_tensor(pk1v[:, :, :, 1], pk1v[:, :, :, 1],
                                    na2p[:, :, :, 1], op=A.add)
            pk2 = big()
            pk2v = pk2[:].rearrange("p (g k r) -> p g k r", g=NV * C,
                                    k=L // 2)
            nc.gpsimd.tensor_copy(pk2v[:, :, :, 0], na2p[:, :, :, 0])
            nc.vector.tensor_tensor(pk2v[:, :, :, 1], na1p[:, :, :, 1],
                                    na2p[:, :, :, 0], op=A.mult)
            pkw = big()
            pkwv = pkw[:].rearrange("p (g k r) -> p g k r", g=NV * C,
                                    k=L // 2)
            nc.gpsimd.tensor_copy(pkwv[:, :, :, 0], wfp[:, :, :, 0])
            nc.vector.tensor_tensor(pkwv[:, :, :, 1], na1p[:, :, :, 1],
                                    wfp[:, :, :, 0], op=A.mult)
            nc.vector.tensor_tensor(pkwv[:, :, :, 1], pkwv[:, :, :, 1],
                                    wfp[:, :, :, 1], op=A.add)

            Yg = Y[:].rearrange("p (g ch i) -> p g ch i", g=NV * C, ch=3)
            G = NV * C

            def bcy(col):
                return Yg[:, :, :, col].unsqueeze(3).to_broadcast(
                    (P, G, 3, 2))

            def bck(pkv, k):
                return pkv[:, :, k, :].unsqueeze(2).to_broadcast(
                    (P, G, 3, 2))

            import os as _os2
            _SKIP = bool(int(_os2.environ.get("KSKIP_SCAN", "0")))
            for k in range(0 if _SKIP else L // 2):
                c0 = 2 * k
                t1 = tmpp.tile([P, G * 3 * 2], dt, tag="sc", name="t1")
                t1v = t1[:].rearrange("p (g ch r) -> p g ch r", g=G, ch=3)
                t2 = tmpp.tile([P, G * 3 * 2], dt, tag="sc", name="t2")
                t2v = t2[:].rearrange("p (g ch r) -> p g ch r", g=G, ch=3)
                nc.vector.tensor_tensor(t1v, bcy(c0 + 1), bck(pk1v, k),
                                        op=A.mult)
                nc.gpsimd.tensor_tensor(t2v, bcy(c0), bck(pk2v, k),
                                        op=A.mult)
                nc.vector.tensor_tensor(t1v, t1v, t2v, op=A.add)
                nc.vector.tensor_tensor(Yg[:, :, :, c0 + 2:c0 + 4], t1v,
                                        bck(pkwv, k), op=A.add)

            # ---------- block transfers ----------
            # T entries [128, NV, C]: h1e h2e h1e2 h2e2 ue ue2
            TmE = keep("TmE", [NV * C * 6])
            TmEv = TmE[:].rearrange("p (e v c) -> p e v c", e=6, v=NV)
            for (e, ch, col) in ((0, 1, L + 1), (1, 2, L + 1), (2, 1, L),
                                 (3, 2, L)):
                nc.vector.tensor_tensor(TmEv[:, e], Yv[:, :, :, ch, col],
                                        Yv[:, :, :, 0, col], op=A.subtract)
            nc.vector.tensor_copy(TmEv[:, 4], Yv[:, :, :, 0, L + 1])
            nc.vector.tensor_copy(TmEv[:, 5], Yv[:, :, :, 0, L])

            # ---------- intra-partition exclusive prefixes over c ----------
            # Pm00 Pm01 Pm10 Pm11 Pu0 Pu1: [128, 6, NV, C]
            PmE = keep("PmE", [6 * NV * C])
            PmEv = PmE[:].rearrange("p (e v c) -> p e v c", e=6, v=NV)
            for e in range(6):
                init = 1.0 if e in (0, 3) else 0.0
                nc.vector.memset(PmEv[:, e, :, 0:1], init)

            def compose_step(dst, dcc, S, scc, Tt, tcc):
                """dst[:,:,dcc] = T[:,:,tcc] o S[:,:,scc] (T applied second)"""
                tq = tmpp.tile([P, 2 * NV], dt, tag="c8", name="tq")
                tqv = tq[:].rearrange("p (e v) -> p e v", e=2)
                tq2 = tmpp.tile([P, 2 * NV], dt, tag="c8", name="tq2")
                tq2v = tq2[:].rearrange("p (e v) -> p e v", e=2)
                # m rows batched: (n00,n01) = T00*(S00,S01) + T01*(S10,S11)
                #                 (n10,n11) = T10*(S00,S01) + T11*(S10,S11)
                for (dr, a1, a2) in ((0, 0, 1), (2, 2, 3)):
                    t00b = Tt[:, a1, :, tcc].unsqueeze(1).to_broadcast(
                        (P, 2, NV))
                    t01b = Tt[:, a2, :, tcc].unsqueeze(1).to_broadcast(
                        (P, 2, NV))
                    nc.vector.tensor_tensor(tqv, t00b, S[:, 0:2, :, scc],
                                            op=A.mult)
                    nc.vector.tensor_tensor(tq2v, t01b, S[:, 2:4, :, scc],
                                            op=A.mult)
                    nc.vector.tensor_tensor(dst[:, dr:dr + 2, :, dcc],
                                            tqv, tq2v, op=A.add)
                # u: newu0 = T00*Su0 + T01*Su1 + Tu0 ; newu1 = T10.. + Tu1
                tq3 = tmpp.tile([P, NV], dt, tag="c4", name="tq3")
                tq4 = tmpp.tile([P, NV], dt, tag="c4", name="tq4")
                for (de, a1, a2, au) in ((4, 0, 1, 4), (5, 2, 3, 5)):
                    nc.gpsimd.tensor_tensor(tq3[:], Tt[:, a1, :, tcc],
                                            S[:, 4, :, scc], op=A.mult)
                    nc.gpsimd.tensor_tensor(tq4[:], Tt[:, a2, :, tcc],
                                            S[:, 5, :, scc], op=A.mult)
                    nc.gpsimd.tensor_tensor(tq3[:], tq3[:], tq4[:], op=A.add)
                    nc.gpsimd.tensor_tensor(dst[:, de, :, dcc], tq3[:],
                                            Tt[:, au, :, tcc], op=A.add)

            for cc in range(1, C):
                compose_step(PmEv, cc, PmEv, cc - 1, TmEv, cc - 1)

            # full-partition transfer F = T_{C-1} o P_{C-1}, packed [128,NV,6]
            Fp = keep("Fp", [NV * 6])
            FpE = Fp[:].rearrange("p (e v c) -> p e v c", e=6, v=NV, c=1)
            compose_step(FpE, 0, PmEv, C - 1, TmEv, C - 1)

            # ---------- Hillis-Steele across partitions via PE shifts -------
            # pack to [128, NV*6] with (v, e) layout for matmul rhs
            Xp = keep("Xp", [NV * 6])
            Xv = Xp[:].rearrange("p (v e) -> p v e", v=NV)
            for e in range(6):
                nc.vector.tensor_copy(Xv[:, :, e], FpE[:, e, :, 0])

            cur = Xp
            for si, s in enumerate((1, 2, 4, 8, 16, 32, 64)):
                sh = tmpp.tile([P, NV * 6], dt, tag="hs", name="sh")
                nc.vector.memset(sh[0:s, :], 0.0)
                nc.sync.dma_start(sh[s:P, :], cur[0:P - s, :])
                shv = sh[:].rearrange("p (v e) -> p v e", v=NV)
                nc.vector.memset(shv[0:s, :, 0:1], 1.0)
                nc.vector.memset(shv[0:s, :, 3:4], 1.0)
                nxt = tmpp.tile([P, NV * 6], dt, tag="hs", name="nxt")
                nxtv = nxt[:].rearrange("p (v e) -> p v e", v=NV)
                curv = cur[:].rearrange("p (v e) -> p v e", v=NV)
                tq = tmpp.tile([P, NV * 2], dt, tag="c8", name="tq")
                tqv = tq[:].rearrange("p (v e) -> p v e", v=NV)
                tq2 = tmpp.tile([P, NV * 2], dt, tag="c8", name="tq2")
                tq2v = tq2[:].rearrange("p (v e) -> p v e", v=NV)
                for (dr, a1, a2) in ((0, 0, 1), (2, 2, 3)):
                    c0b = curv[:, :, a1].unsqueeze(2).to_broadcast((P, NV, 2))
                    c1b = curv[:, :, a2].unsqueeze(2).to_broadcast((P, NV, 2))
                    nc.vector.tensor_tensor(tqv, c0b, shv[:, :, 0:2],
                                            op=A.mult)
                    nc.vector.tensor_tensor(tq2v, c1b, shv[:, :, 2:4],
                                            op=A.mult)
                    nc.vector.tensor_tensor(nxtv[:, :, dr:dr + 2], tqv, tq2v,
                                            op=A.add)
                tq3 = tmpp.tile([P, NV], dt, tag="c4", name="tq3")
                tq4 = tmpp.tile([P, NV], dt, tag="c4", name="tq4")
                for (de, a1, a2) in ((4, 0, 1), (5, 2, 3)):
                    nc.gpsimd.tensor_tensor(tq3[:], curv[:, :, a1],
                                            shv[:, :, 4], op=A.mult)
                    nc.gpsimd.tensor_tensor(tq4[:], curv[:, :, a2],
                                            shv[:, :, 5], op=A.mult)
                    nc.gpsimd.tensor_tensor(tq3[:], tq3[:], tq4[:], op=A.add)
                    nc.gpsimd.tensor_tensor(nxtv[:, :, de], tq3[:],
                                            curv[:, :, de], op=A.add)
                cur = nxt

            # exclusive shift by 1: s0 = shift1(cur); rows<1 -> zeros
            s0 = keep("s0", [NV * 6])
            nc.vector.memset(s0[0:1, :], 0.0)
            nc.sync.dma_start(s0[1:P, :], cur[0:P - 1, :])
            s0v = s0[:].rearrange("p (v e) -> p v e", v=NV)

            # ---------- per-block incoming states: ab = Pm@s0u + Pu --------
            abt = keep("abt", [3 * NV * C])
            abv = abt[:].rearrange("p (e v c) -> p e v c", e=3, v=NV)
            s0u0b = s0v[:, :, 4].unsqueeze(2).to_broadcast((P, NV, C))
            s0u1b = s0v[:, :, 5].unsqueeze(2).to_broadcast((P, NV, C))
            tqa = tmpp.tile([P, NV * C], dt, tag="c32", name="tqa")
            tqav = tqa[:].rearrange("p (v c) -> p v c", v=NV)
            tqb = tmpp.tile([P, NV * C], dt, tag="c32", name="tqb")
            tqbv = tqb[:].rearrange("p (v c) -> p v c", v=NV)
            for (de, a1, a2, au) in ((0, 0, 1, 4), (1, 2, 3, 5)):
                nc.vector.tensor_tensor(tqav, PmEv[:, a1], s0u0b, op=A.mult)
                nc.vector.tensor_tensor(tqbv, PmEv[:, a2], s0u1b, op=A.mult)
                nc.vector.tensor_tensor(tqav, tqav, tqbv, op=A.add)
                nc.vector.tensor_tensor(abv[:, de], tqav, PmEv[:, au],
                                        op=A.add)
            # gamma = 1 - a - b
            nc.vector.tensor_tensor(tqav, abv[:, 0], abv[:, 1], op=A.add)
            nc.vector.tensor_scalar(abv[:, 2], tqav, -1.0, 1.0,
                                    op0=A.mult, op1=A.add)

            # ---------- correction + wet ----------
            yout = big()
            youtv = yout[:].rearrange("p (v c i) -> p v c i", v=NV, c=C)
            a_b = abv[:, 0].unsqueeze(3).to_broadcast((P, NV, C, L))
            b_b = abv[:, 1].unsqueeze(3).to_broadcast((P, NV, C, L))
            g_b = abv[:, 2].unsqueeze(3).to_broadcast((P, NV, C, L))
            ct1 = big()
            ct1v = ct1[:].rearrange("p (v c i) -> p v c i", v=NV, c=C)
            ct2 = big()
            ct2v = ct2[:].rearrange("p (v c i) -> p v c i", v=NV, c=C)
            nc.vector.tensor_tensor(ct1v, Yv[:, :, :, 1, 2:L + 2], a_b,
                                    op=A.mult)
            nc.vector.tensor_tensor(ct2v, Yv[:, :, :, 2, 2:L + 2], b_b,
                                    op=A.mult)
            nc.vector.tensor_tensor(ct1v, ct1v, ct2v, op=A.add)
            nc.vector.tensor_tensor(ct2v, Yv[:, :, :, 0, 2:L + 2], g_b,
                                    op=A.mult)
            nc.vector.tensor_tensor(youtv, ct1v, ct2v, op=A.add)
            wet = big()
            for v in range(NV):
                nc.scalar.activation(vsl(wet, v), vsl(yout, v), F.Tanh,
                                     scale=vc1(v, K_DIST))
            for v in range(NV):
                nc.sync.dma_start(
                    wet_d[v:v + 1, :].rearrange("o (p j) -> p o j", p=P),
                    vsl(wet, v)[:].rearrange("p (o j) -> p o j", o=1))

    nc.compile()
    return nc


def _host_prep(inputs):
    """Per-core input maps. Heavy phase cumsum matches eager CPU-XLA jax."""
    import jax
    import jax.numpy as jnp

    cpu = jax.devices("cpu")[0]
    f0 = np.asarray(inputs["f0_hz"], f32)
    s = np.asarray(inputs["osc_shape"], f32)
    note_on = np.asarray(inputs["note_on_duration"], f32)
    fcm = np.asarray(inputs["fc_mod_sig"], f32)
    qmm = np.asarray(inputs["q_mod_sig"], f32)
    dist = np.asarray(inputs["dist_gain"], f32)
    att = np.asarray(inputs["attack"], f32)
    dec = np.asarray(inputs["decay"], f32)
    sus = np.asarray(inputs["sustain"], f32)
    rel = np.asarray(inputs["release"], f32)
    alpha = np.asarray(inputs["alpha"], f32)
    Bn = f0.shape[0]

    inc = (f0 / f32(SR)).astype(f32)
    incb = jax.device_put(
        np.broadcast_to(inc[:, None], (Bn, T)).astype(f32), cpu)
    cx = np.asarray(jax.jit(lambda a: jnp.cumsum(a, axis=1))(incb))
    p32 = (f32(2 * np.pi) * cx).astype(f32)
    r64 = np.mod(p32.astype(np.float64), 2 * np.pi)
    r64 = np.where(r64 > np.pi, r64 - 2 * np.pi, r64)
    rph = r64.astype(f32)

    partials = (f32(12000.0) / (f0 * np.log10(f0))).astype(f32)
    vc_cols = np.zeros((Bn, NK), f32)
    vc_cols[:, K_TANH] = f32(np.pi) * partials / f32(2.0)
    vc_cols[:, K_S] = s
    vc_cols[:, K_DRYSC] = f32(0.5) * (f32(1.0) - s / f32(2.0))
    vc_cols[:, K_IATT] = f32(1.0) / att
    vc_cols[:, K_ALPHA] = alpha
    vc_cols[:, K_IDEC] = f32(1.0) / dec
    vc_cols[:, K_BDEC] = -(att / dec)
    vc_cols[:, K_N1MS] = -(f32(1.0) - sus)
    vc_cols[:, K_IREL] = f32(1.0) / rel
    vc_cols[:, K_BREL] = -(note_on / rel)
    vc_cols[:, K_DIST] = dist

    tsec = (np.arange(T, dtype=f32) / f32(SR)).astype(f32).reshape(P, 1024)

    in_maps = []
    for core in range(NCORES):
        vs = slice(core * NV, (core + 1) * NV)
        vc = np.ascontiguousarray(
            np.broadcast_to(vc_cols[vs].reshape(1, NV * NK), (P, NV * NK))
        ).astype(f32)
        in_maps.append(dict(
            rph=np.ascontiguousarray(rph[vs]),
            fcm=np.ascontiguousarray(fcm[vs]),
            qmm=np.ascontiguousarray(qmm[vs]),
            tsec=tsec, vc=vc,
        ))
    return in_maps


def run(inputs, trace=False):
    from concourse.bass_utils import run_bass_kernel_spmd

    if "nc" not in _CACHE:
        _CACHE["nc"] = _build_module()
    nc = _CACHE["nc"]
    in_maps = _host_prep(inputs)
    res = run_bass_kernel_spmd(nc, in_maps, core_ids=list(range(NCORES)),
                               trace=trace)
    dry = np.concatenate([r["dry"] for r in res.results], axis=0)
    wet = np.concatenate([r["wet"] for r in res.results], axis=0)
    env = np.concatenate([r["env"] for r in res.results], axis=0)
    return (dry, wet, env), res


def kernel(**inputs):
    out, _ = run(inputs, trace=False)
    return out
